# revision 29
# baseline (speedup 1.0000x reference)
"""GATv2 regressor on 8 Trainium2 NeuronCores (Bass).

Sharding: core c owns dst nodes [c*12500, (c+1)*12500), relabeled locally in
decreasing in-degree order so fixed-slot padding per 128-dst block is tight.
All per-edge indexing is host-precomputed; the device does dense DMA,
dma_gather by src table row, broadcast adds, free-dim reductions and small
matmuls. Softmax skips max-subtraction (scores are O(1)); denominators are
accumulated alongside numerators and divided at the end.
"""
import numpy as np
import ml_dtypes

import concourse.bacc as bacc
import concourse.mybir as mybir
import concourse.tile as tile
from concourse.bass_utils import run_bass_kernel_spmd
from concourse.library_config import mlp as mlp_lib

F32 = mybir.dt.float32
BF16 = mybir.dt.bfloat16
I16 = mybir.dt.int16

N, E, IN, C, H, G = 100000, 1600000, 128, 32, 2, 64
NEG = 0.2
NCORES = 8
SH = 12500
SHP = 12544              # 98*128
NBLK = SHP // 128        # 98
NTAB = SHP * NCORES      # 100352
NCHUNK = 4
CHR = NTAB // NCHUNK     # 25088 table rows per gather chunk

_CACHE = {}


def _wrap_idx(idx):
    n = idx.shape[0]
    w = idx.reshape(n // 16, 16).T
    return np.tile(w, (8, 1)).astype(np.int16)


def host_prep(edge_index, batch):
    src = edge_index[0].astype(np.int64)
    dst = edge_index[1].astype(np.int64)
    core = dst // SH
    dloc = dst % SH

    perms, ranks = [], []
    for c in range(NCORES):
        deg = np.bincount(dloc[core == c], minlength=SH)
        p = np.argsort(-deg, kind="stable")
        r = np.empty(SH, dtype=np.int64)
        r[p] = np.arange(SH)
        perms.append(p)
        ranks.append(r)

    ncore = np.arange(N) // SH
    nloc = np.arange(N) % SH
    trow = np.empty(N, dtype=np.int64)
    for c in range(NCORES):
        m = ncore == c
        trow[m] = c * SHP + ranks[c][nloc[m]]

    erow = np.empty(E, dtype=np.int64)
    for c in range(NCORES):
        m = core == c
        erow[m] = ranks[c][dloc[m]]
    srow = trow[src]
    chunk = srow // CHR

    S = np.ones((NBLK, NCHUNK), dtype=np.int64)
    for c in range(NCORES):
        m = core == c
        cnt = np.bincount((erow[m] * NCHUNK + chunk[m]).astype(np.int64),
                          minlength=SH * NCHUNK).reshape(SH, NCHUNK)
        full = np.zeros((SHP, NCHUNK), dtype=np.int64)
        full[:SH] = cnt
        S = np.maximum(S, full.reshape(NBLK, 128, NCHUNK).max(axis=1))
    Stot = int(S.sum())

    idx_all, msk_all, blc_all, cnt_all = [], [], [], []
    for c in range(NCORES):
        m = np.nonzero(core == c)[0]
        key = erow[m] * NCHUNK + chunk[m]
        order = np.argsort(key, kind="stable")
        ms = m[order]
        rk, ck = erow[ms], chunk[ms]
        gid = (rk * NCHUNK + ck).astype(np.int64)
        first = np.zeros(SH * NCHUNK + 1, dtype=np.int64)
        np.cumsum(np.bincount(gid, minlength=SH * NCHUNK), out=first[1:])
        slot = np.arange(ms.size) - first[gid]

        iarr = np.zeros((Stot * 128,), dtype=np.int16)
        marr = np.zeros((128, Stot), dtype=np.float32)
        carr = np.zeros((NBLK * NCHUNK,), dtype=np.int32)
        off = 0
        bb = rk // 128
        pp = rk % 128
        for b in range(NBLK):
            selb = bb == b
            for k in range(NCHUNK):
                s = int(S[b, k])
                sel = selb & (ck == k)
                p = pp[sel]
                sl = slot[sel]
                # per-core used slot count for this (block, chunk)
                u = int(sl.max()) + 1 if sl.size else 0
                seg = np.zeros(s * 128, dtype=np.int16)
                seg[sl * 128 + p] = (srow[ms[sel]] - k * CHR).astype(np.int16)
                seg[u * 128:] = -1            # trailing slots: skipped by DGE
                carr[b * NCHUNK + k] = u * 128
                iarr[off * 128:(off + s) * 128] = seg
                marr[p, off + sl] = 1.0
                off += s
        idx_all.append(_wrap_idx(iarr))
        msk_all.append(marr)
        cnt_all.append(np.tile(carr.reshape(1, -1), (1, 1)))
        bl = np.full((128, NBLK), 127.0, dtype=np.float32)
        for b in range(NBLK):
            lo = b * 128
            take = min(128, SH - lo)
            bl[:take, b] = batch[c * SH + perms[c][lo:lo + take]]
        blc_all.append(bl)

    return S, Stot, trow, perms, idx_all, msk_all, blc_all, cnt_all


def build_kernel(S):
    Stot = int(S.sum())
    nc = bacc.Bacc("TRN2", target_bir_lowering=False, num_swdge_queues=4,
                   dynamic_dma_scratch_size=40960)

    def dp(name, shape, dt=F32):
        return nc.declare_dram_parameter(name, shape, dt, isOutput=False)

    xT = dp("xT", [IN, NTAB], BF16)
    xTl = dp("xTl", [IN, SHP], BF16)
    w1 = dp("w1", [IN, 128], BF16)           # [Wl1 | Wr1]
    w2 = dp("w2", [64, 64], BF16)            # [Wl2 | Wr2]
    cnts = dp("cnts", [1, NBLK * NCHUNK], mybir.dt.int32)
    att1r = dp("att1r", [128, 64])
    att2r = dp("att2r", [128, 32])
    b1r = dp("b1r", [128, 64])
    b2r = dp("b2r", [128, 32])
    g1wp = dp("g1wp", [32, 32], BF16)
    g1br = dp("g1br", [128, 32])
    g2wr = dp("g2wr", [128, 32])
    l1wp = dp("l1wp", [32, 32], BF16)
    l1br = dp("l1br", [64, 32])
    l2wr = dp("l2wr", [64, 32])
    sc4 = dp("sc4", [128, 4])                  # [g2b, lin2b, 0, 0]
    io64 = dp("io64", [128, 64])
    ones132 = dp("ones132", [1, 32])
    id128 = dp("id128", [128, 128])
    bloc = dp("bloc", [128, NBLK])
    msk = dp("msk", [128, Stot])
    eidx = dp("eidx", [128, (Stot * 128) // 16], I16)

    out_y = nc.declare_dram_parameter("y", [64, 1], F32, isOutput=True)

    add = mybir.AluOpType.add
    mult = mybir.AluOpType.mult
    iseq = mybir.AluOpType.is_equal
    byp = mybir.AluOpType.bypass
    AX = mybir.AxisListType.X
    EXP = mybir.ActivationFunctionType.Exp
    RELU = mybir.ActivationFunctionType.Relu
    LRELU = mybir.ActivationFunctionType.Lrelu

    with tile.TileContext(nc) as tc:
        with (
            tc.tile_pool(name="const", bufs=1) as cp,
            tc.tile_pool(name="gat", bufs=8) as gpool,
            tc.tile_pool(name="wk", bufs=2) as wk,
            tc.tile_pool(name="vtp", bufs=4) as vtp,
            tc.tile_pool(name="ps", bufs=2, space="PSUM") as ps,
            tc.tile_pool(name="psacc", bufs=1, space="PSUM") as psacc,
            tc.tile_pool(name="big", bufs=1) as bigp,
            tc.tile_pool(name="dram", bufs=1, space="DRAM") as dram,
        ):
            tab1 = dram.tile([NTAB, 128], BF16)
            tab2 = dram.tile([NTAB, 128], BF16)
            hT_loc = dram.tile([64, SHP], BF16)
            hT_all = dram.tile([NCORES * 64, SHP], BF16)
            pool_in = dram.tile([48, 64], F32)
            pool_all = dram.tile([48, 64], F32)
            nc.gpsimd.load_library(mlp_lib)

            def lc(t, shape, dt=F32):
                tt = cp.tile(shape, dt, tag=t.name + "_t")
                nc.sync.dma_start(tt[:], t[:])
                return tt

            w1_t = lc(w1, [IN, 128], BF16)
            w2_t = lc(w2, [64, 64], BF16)
            cnts_t = lc(cnts, [1, NBLK * NCHUNK], mybir.dt.int32)
            att1_t = lc(att1r, [128, 64])
            att1b_t = cp.tile([128, 64], BF16, tag="att1b")
            nc.vector.tensor_copy(att1b_t[:], att1_t[:])
            att2_t = lc(att2r, [128, 32])
            att2b_t = cp.tile([128, 32], BF16, tag="att2b")
            nc.vector.tensor_copy(att2b_t[:], att2_t[:])
            b1_t = lc(b1r, [128, 64])
            b2_t = lc(b2r, [128, 32])
            g1w_t = lc(g1wp, [32, 32], BF16)
            g1b_t = lc(g1br, [128, 32])
            g2w_t = lc(g2wr, [128, 32])
            l1w_t = lc(l1wp, [32, 32], BF16)
            l1b_t = lc(l1br, [64, 32])
            l2w_t = lc(l2wr, [64, 32])
            sc4_t = lc(sc4, [128, 4])
            io64_t = lc(io64, [128, 64])
            on132_t = lc(ones132, [1, 32])
            id_t = lc(id128, [128, 128])
            idb_t = cp.tile([128, 128], BF16, tag="idb")
            nc.vector.tensor_copy(idb_t[:], id_t[:])
            bloc_t = lc(bloc, [128, NBLK])
            msk_t = bigp.tile([128, Stot], F32)
            nc.sync.dma_start(msk_t[:], msk[:])

            xr1_t = bigp.tile([128, NBLK * 64], BF16)
            xr2_t = bigp.tile([128, NBLK * 32], BF16)
            hTl_t = bigp.tile([64, SHP], BF16)
            h2all = bigp.tile([128, NBLK * 32], F32)
            gcnt_reg = nc.gpsimd.alloc_register("gcnt")
            for _gz in range(8):
                gz = gpool.tile([128, 16, 128], BF16, tag="g", bufs=8,
                                name=f"gz{_gz}")
                nc.vector.memset(gz[:], 0.0)

            # ---------------- L1 global table + local xr1 ----------------
            for n0 in range(0, NTAB, 512):
                pt = ps.tile([128, 512], F32, tag="mm")
                xin = wk.tile([IN, 512], BF16, tag="xin")
                nc.sync.dma_start(xin[:], xT[:, n0:n0 + 512])
                for j in range(4):
                    nc.tensor.matmul(pt[:, j * 128:(j + 1) * 128],
                                     xin[:, j * 128:(j + 1) * 128], w1_t[:],
                                     start=True, stop=True)
                st = wk.tile([128, 512], BF16, tag="tsb")
                nc.scalar.copy(st[:], pt[:])
                nc.sync.dma_start(
                    tab1[n0:n0 + 512, :].rearrange("(j p) c -> p j c", p=128),
                    st[:].rearrange("p (j c) -> p j c", j=4))
            for n0 in range(0, SHP, 512):
                w_ = min(512, SHP - n0)
                pt = ps.tile([128, 512], F32, tag="mm")
                xin = wk.tile([IN, 512], BF16, tag="xin")
                nc.sync.dma_start(xin[:, :w_], xTl[:, n0:n0 + w_])
                for j in range(w_ // 128):
                    nc.tensor.matmul(pt[:, j * 128:(j + 1) * 128],
                                     xin[:, j * 128:(j + 1) * 128], w1_t[:],
                                     start=True, stop=True)
                st = wk.tile([128, 512], BF16, tag="tsb")
                nc.scalar.copy(st[:, :w_], pt[:, :w_])
                for j in range(w_ // 128):
                    b = n0 // 128 + j
                    nc.vector.tensor_copy(
                        xr1_t[:, b * 64:(b + 1) * 64],
                        st[:, j * 128 + 64:j * 128 + 128])

            tc.strict_bb_all_engine_barrier()

            def edge_layer(tab, xr_t, att_t, bias_t, F, heads, hall, gw, gdt):
                ioff = 0
                soff = 0
                for b in range(NBLK):
                    Ss = [int(S[b, k]) for k in range(NCHUNK)]
                    ST = sum(Ss)
                    acc4 = wk.tile([128, F, NCHUNK], F32, tag="acc4")
                    ixb = wk.tile([128, ST * 8], I16, tag="ix")
                    nc.sync.dma_start(ixb[:], eidx[:, ioff:ioff + ST * 8])
                    scob = wk.tile([128, ST * heads], F32, tag="scob")
                    wexpb = wk.tile([128, ST * heads], F32, tag="wexpb")
                    gs = []
                    iof2 = 0
                    co = 0
                    for k in range(NCHUNK):
                        s = Ss[k]
                        ni = s * 128
                        g = gpool.tile([128, s, gw], gdt, tag="g", bufs=8)
                        nc.gpsimd.reg_load(
                            gcnt_reg,
                            cnts_t[0:1, b * NCHUNK + k:b * NCHUNK + k + 1])
                        nc.gpsimd.dma_gather(
                            g[:], tab[k * CHR:(k + 1) * CHR, 0:gw],
                            ixb[:, iof2:iof2 + ni // 16],
                            ni, gcnt_reg, gw,
                            single_packet=False,
                            queue_num=(b * NCHUNK + k) % 4)
                        gs.append(g)
                        iof2 += ni // 16
                        ioff += ni // 16
                        xb = xr_t[:, b * F:(b + 1) * F]
                        sadd = wk.tile([128, s, F], gdt, tag="sadd")
                        nc.vector.tensor_tensor(
                            sadd[:], g[:, :, 0:F],
                            xb.unsqueeze(1).broadcast_to([128, s, F]), op=add)
                        nc.scalar.activation(sadd[:], sadd[:], LRELU,
                                             alpha=NEG)
                        nc.vector.tensor_tensor(
                            sadd[:], sadd[:],
                            att_t[:, 0:F].unsqueeze(1)
                            .broadcast_to([128, s, F]), op=mult)
                        nc.vector.tensor_reduce(
                            scob[:, co * heads:(co + s) * heads],
                            sadd[:].rearrange("p s (h c) -> p (s h) c", c=32),
                            axis=AX, op=add)
                        co += s
                    nc.vector.tensor_scalar_min(scob[:], scob[:], 30.0)
                    nc.scalar.activation(wexpb[:], scob[:], EXP)
                    wv = wexpb[:].rearrange("p (s h) -> p s h", h=heads)
                    nc.vector.tensor_tensor(
                        wv, wv,
                        msk_t[:, soff:soff + ST].unsqueeze(2)
                        .broadcast_to([128, ST, heads]), op=mult)
                    den = wk.tile([128, heads], F32, tag="den")
                    nc.vector.tensor_reduce(
                        den[:], wexpb[:].rearrange("p (s h) -> p h s", h=heads),
                        axis=AX, op=add)
                    co = 0
                    for k in range(NCHUNK):
                        s = Ss[k]
                        g = gs[k]
                        vt = vtp.tile([128, F, s], BF16, tag="vt")
                        wvk = wv[:, co:co + s, :]
                        for h in range(heads):
                            nc.vector.tensor_tensor(
                                vt[:, h * 32:(h + 1) * 32, :]
                                .rearrange("p c s -> p s c"),
                                g[:, :, h * 32:h * 32 + 32],
                                wvk[:, :, h:h + 1].broadcast_to([128, s, 32]),
                                op=mult)
                        nc.vector.tensor_reduce(
                            acc4[:, :, k:k + 1].rearrange("p c o -> p (c o)"),
                            vt[:], axis=AX, op=add)
                        co += s
                    acc = wk.tile([128, F], F32, tag="acc")
                    nc.vector.tensor_reduce(acc[:], acc4[:], axis=AX, op=add)
                    nc.vector.tensor_scalar_max(den[:], den[:], 1e-30)
                    nc.vector.reciprocal(den[:], den[:])
                    hb = wk.tile([128, F], F32, tag="hb")
                    for h in range(heads):
                        nc.vector.tensor_tensor(
                            hb[:, h * 32:(h + 1) * 32],
                            acc[:, h * 32:(h + 1) * 32],
                            den[:, h:h + 1].broadcast_to([128, 32]), op=mult)
                    nc.vector.tensor_tensor(
                        hb[:], hb[:],
                        bias_t[:, 0:F], op=add)
                    nc.vector.tensor_scalar_max(
                        hall[:, b * F:(b + 1) * F], hb[:], 0.0)
                    soff += ST

            # ---------------- Layer 1 ----------------
            h1all = bigp.tile([128, NBLK * 64], BF16)
            edge_layer(tab1, xr1_t, att1b_t, b1_t, 64, H, h1all, 128, BF16)

            for b in range(NBLK):
                pt = ps.tile([64, 128], BF16, tag="mm")
                nc.tensor.transpose(pt[:], h1all[:, b * 64:(b + 1) * 64],
                                    idb_t[:])
                nc.scalar.copy(hTl_t[:, b * 128:(b + 1) * 128], pt[:])
            nc.sync.dma_start(hT_loc[:], hTl_t[:])

            tc.strict_bb_all_engine_barrier()
            nc.gpsimd.collective_compute(
                "AllGather", byp,
                replica_groups=[list(range(NCORES))],
                ins=[hT_loc.opt()], outs=[hT_all.opt()])
            tc.strict_bb_all_engine_barrier()

            # ---------------- L2 table + local xr2 ----------------
            for c in range(NCORES):
                for n0 in range(0, SHP, 512):
                    w_ = min(512, SHP - n0)
                    hinb = wk.tile([64, 512], BF16, tag="hinb")
                    nc.sync.dma_start(
                        hinb[:, :w_], hT_all[c * 64:(c + 1) * 64, n0:n0 + w_])
                    pt = ps.tile([128, 256], F32, tag="mm")
                    for j in range(w_ // 128):
                        nc.tensor.matmul(pt[:, j * 64:(j + 1) * 64],
                                         hinb[:, j * 128:(j + 1) * 128],
                                         w2_t[:], start=True, stop=True)
                    st = wk.tile([128, 256], BF16, tag="t2sb")
                    nc.scalar.copy(st[:], pt[:])
                    base = c * SHP + n0
                    nc.sync.dma_start(
                        tab2[base:base + w_, 0:64]
                        .rearrange("(j p) c -> p j c", p=128),
                        st[:, :w_ // 2].rearrange("p (j c) -> p j c", c=64))

            for n0 in range(0, SHP, 512):
                w_ = min(512, SHP - n0)
                hinb = wk.tile([64, 512], BF16, tag="hinb")
                nc.vector.tensor_copy(hinb[:, :w_], hTl_t[:, n0:n0 + w_])
                pt = ps.tile([128, 256], F32, tag="mm")
                for j in range(w_ // 128):
                    nc.tensor.matmul(pt[:, j * 64:(j + 1) * 64],
                                     hinb[:, j * 128:(j + 1) * 128], w2_t[:],
                                     start=True, stop=True)
                st = wk.tile([128, 256], BF16, tag="t2sb")
                nc.scalar.copy(st[:], pt[:])
                for j in range(w_ // 128):
                    b = n0 // 128 + j
                    nc.vector.tensor_copy(xr2_t[:, b * 32:(b + 1) * 32],
                                          st[:, j * 64 + 32:j * 64 + 64])

            tc.strict_bb_all_engine_barrier()

            # ---------------- Layer 2 ----------------
            edge_layer(tab2, xr2_t, att2b_t, b2_t, 32, 1, h2all, 128, BF16)

            # ---------------- pooling ----------------
            pp = psacc.tile([34, 64], F32)
            for b in range(NBLK):
                h2 = h2all[:, b * 32:(b + 1) * 32]
                pt = ps.tile([32, 128], F32, tag="mm")
                nc.tensor.transpose(pt[:], h2, id_t[:])
                h2T = wk.tile([32, 128], BF16, tag="h2T")
                nc.vector.tensor_copy(h2T[:], pt[:])
                gp1 = ps.tile([128, 32], F32, tag="mm")
                nc.tensor.matmul(gp1[:], h2T[:], g1w_t[:],
                                 start=True, stop=True)
                ga = wk.tile([128, 32], F32, tag="ga")
                nc.vector.tensor_tensor(
                    ga[:], gp1[:],
                    g1b_t[:], op=add)
                nc.vector.tensor_scalar_max(ga[:], ga[:], 0.0)
                nc.vector.tensor_tensor(
                    ga[:], ga[:],
                    g2w_t[:], op=mult)
                gt = wk.tile([128, 1], F32, tag="gt")
                nc.vector.tensor_reduce(gt[:], ga[:], axis=AX, op=add)
                nc.vector.tensor_tensor(
                    gt[:], gt[:],
                    sc4_t[:, 0:1], op=add)
                ge = wk.tile([128, 1], F32, tag="ge")
                nc.scalar.activation(ge[:], gt[:], EXP)
                vg = wk.tile([128, 34], F32, tag="vg")
                nc.vector.tensor_tensor(
                    vg[:, 0:32], h2, ge[:].broadcast_to([128, 32]), op=mult)
                nc.vector.tensor_copy(vg[:, 32:33], ge[:])
                nc.vector.memset(vg[:, 33:34], 0.0)
                ohg = wk.tile([128, 64], F32, tag="ohg")
                nc.vector.tensor_scalar(
                    ohg[:], io64_t[:],
                    bloc_t[:, b:b + 1], None, op0=iseq)
                vgb = wk.tile([128, 34], BF16, tag="vgb")
                nc.vector.tensor_copy(vgb[:], vg[:])
                ohgb = wk.tile([128, 64], BF16, tag="ohgb")
                nc.vector.tensor_copy(ohgb[:], ohg[:])
                nc.tensor.matmul(pp[:], vgb[:], ohgb[:],
                                 start=(b == 0), stop=(b == NBLK - 1))

            pin = wk.tile([48, 64], F32, tag="pin")
            nc.vector.memset(pin[:], 0.0)
            nc.scalar.copy(pin[0:34, :], pp[:])
            nc.sync.dma_start(pool_in[:], pin[:])

            tc.strict_bb_all_engine_barrier()
            nc.gpsimd.collective_compute(
                "AllReduce", add,
                replica_groups=[list(range(NCORES))],
                ins=[pool_in.opt()], outs=[pool_all.opt()])
            tc.strict_bb_all_engine_barrier()

            pall = wk.tile([48, 64], F32, tag="pall")
            nc.sync.dma_start(pall[:], pool_all[:])
            dn = wk.tile([1, 64], F32, tag="dn")
            nc.vector.reciprocal(dn[:], pall[32:33, :])
            dnr = ps.tile([32, 64], F32, tag="mm")
            nc.tensor.matmul(dnr[:], on132_t[:], dn[:],
                             start=True, stop=True)
            pooledT = wk.tile([32, 64], BF16, tag="pooledT")
            nc.vector.tensor_tensor(
                pooledT[:], pall[0:32, :], dnr[:], op=mult)
            zp = ps.tile([64, 32], F32, tag="mm")
            nc.tensor.matmul(zp[:], pooledT[:], l1w_t[:],
                             start=True, stop=True)
            z = wk.tile([64, 32], F32, tag="z")
            nc.vector.tensor_tensor(
                z[:], zp[:], l1b_t[:], op=add)
            nc.vector.tensor_scalar_max(z[:], z[:], 0.0)
            nc.vector.tensor_tensor(
                z[:], z[:], l2w_t[:], op=mult)
            yv = wk.tile([64, 1], F32, tag="yv")
            nc.vector.tensor_reduce(yv[:], z[:], axis=AX, op=add)
            nc.vector.tensor_tensor(
                yv[:], yv[:], sc4_t[0:64, 1:2], op=add)
            nc.sync.dma_start(out_y[:], yv[:])

    nc.compile()
    return nc


def kernel(**inputs):
    x = np.asarray(inputs["x"], dtype=np.float32)
    edge_index = np.asarray(inputs["edge_index"])
    batch = np.asarray(inputs["batch"])
    key = (int(edge_index[:, ::4099].sum()), int(batch[::997].sum()))
    if key not in _CACHE:
        prep = host_prep(edge_index, batch)
        nc = build_kernel(prep[0])
        _CACHE.clear()
        _CACHE[key] = (prep, nc)
    (S, Stot, trow, perms, idx_all, msk_all, blc_all, cnt_all), nc = _CACHE[key]

    xp = np.zeros((NTAB, IN), dtype=np.float32)
    xp[trow] = x
    xT_full = np.ascontiguousarray(xp.T).astype(ml_dtypes.bfloat16)

    w1c = np.concatenate([inputs["Wl1"], inputs["Wr1"]], 1).astype(ml_dtypes.bfloat16)
    w2c = np.concatenate([inputs["Wl2"], inputs["Wr2"]], 1).astype(ml_dtypes.bfloat16)
    common = {
        "xT": xT_full, "w1": w1c, "w2": w2c,
        "att1r": np.tile(np.asarray(inputs["att1"], np.float32).reshape(1, 64), (128, 1)),
        "att2r": np.tile(np.asarray(inputs["att2"], np.float32).reshape(1, 32), (128, 1)),
        "b1r": np.tile(np.asarray(inputs["b1"], np.float32).reshape(1, 64), (128, 1)),
        "b2r": np.tile(np.asarray(inputs["b2"], np.float32).reshape(1, 32), (128, 1)),
        "g1wp": np.asarray(inputs["g1w"]).astype(ml_dtypes.bfloat16),
        "g1br": np.tile(np.asarray(inputs["g1b"], np.float32).reshape(1, 32), (128, 1)),
        "g2wr": np.tile(np.asarray(inputs["g2w"], np.float32).reshape(1, 32), (128, 1)),
        "l1wp": np.asarray(inputs["lin1w"]).astype(ml_dtypes.bfloat16),
        "l1br": np.tile(np.asarray(inputs["lin1b"], np.float32).reshape(1, 32), (64, 1)),
        "l2wr": np.tile(np.asarray(inputs["lin2w"], np.float32).reshape(1, 32), (64, 1)),
        "sc4": np.tile(np.array([[float(np.ravel(inputs["g2b"])[0]),
                          float(np.ravel(inputs["lin2b"])[0]), 0.0, 0.0]],
                        np.float32), (128, 1)),
        "io64": np.tile(np.arange(64, dtype=np.float32).reshape(1, 64), (128, 1)),
        "ones132": np.ones((1, 32), np.float32),
        "id128": np.eye(128, dtype=np.float32),
    }
    in_maps = []
    for c in range(NCORES):
        m = dict(common)
        m["xTl"] = np.ascontiguousarray(xT_full[:, c * SHP:(c + 1) * SHP])
        m["bloc"] = blc_all[c]
        m["msk"] = msk_all[c]
        m["eidx"] = idx_all[c]
        m["cnts"] = cnt_all[c]
        in_maps.append(m)

    res = run_bass_kernel_spmd(nc, in_maps, list(range(NCORES)))
    return res.results[0]["y"].reshape(G).astype(np.float32)



# revision 34
# speedup vs baseline: 1.0871x; 1.0871x over previous
"""GATv2 regressor on 8 Trainium2 NeuronCores (Bass).

Sharding: core c owns dst nodes [c*12500, (c+1)*12500), relabeled locally in
decreasing in-degree order so fixed-slot padding per 128-dst block is tight.
All per-edge indexing is host-precomputed; the device does dense DMA,
dma_gather by src table row, broadcast adds, free-dim reductions and small
matmuls. Softmax skips max-subtraction (scores are clamped before exp);
denominators are reduced from the masked exp tile and divided at the end.

Both layers' node tables are bf16 with 256B rows so each per-edge gather
descriptor moves one full row. Per-core gather lists end in a -1 suffix with
the exact valid count supplied at runtime through a gpsimd register, so cores
with fewer edges in a (block, chunk) cell skip the cross-core padding rows.
Activations are batched per 128-dst block (one Lrelu span, one Exp span) to
avoid per-chunk activation-table reloads.
"""
import numpy as np
import ml_dtypes

import concourse.bacc as bacc
import concourse.mybir as mybir
import concourse.tile as tile
from concourse.bass_utils import run_bass_kernel_spmd
from concourse.library_config import mlp as mlp_lib

F32 = mybir.dt.float32
BF16 = mybir.dt.bfloat16
I16 = mybir.dt.int16

N, E, IN, C, H, G = 100000, 1600000, 128, 32, 2, 64
NEG = 0.2
NCORES = 8
SH = 12500
SHP = 12544              # 98*128
NBLK = SHP // 128        # 98
NTAB = SHP * NCORES      # 100352
NCHUNK = 4
CHR = NTAB // NCHUNK     # 25088 table rows per gather chunk

_CACHE = {}


def _wrap_idx(idx):
    n = idx.shape[0]
    w = idx.reshape(n // 16, 16).T
    return np.tile(w, (8, 1)).astype(np.int16)


def host_prep(edge_index, batch):
    src = edge_index[0].astype(np.int64)
    dst = edge_index[1].astype(np.int64)
    core = dst // SH
    dloc = dst % SH

    perms, ranks = [], []
    for c in range(NCORES):
        deg = np.bincount(dloc[core == c], minlength=SH)
        p = np.argsort(-deg, kind="stable")
        r = np.empty(SH, dtype=np.int64)
        r[p] = np.arange(SH)
        perms.append(p)
        ranks.append(r)

    ncore = np.arange(N) // SH
    nloc = np.arange(N) % SH
    trow = np.empty(N, dtype=np.int64)
    for c in range(NCORES):
        m = ncore == c
        trow[m] = c * SHP + ranks[c][nloc[m]]

    erow = np.empty(E, dtype=np.int64)
    for c in range(NCORES):
        m = core == c
        erow[m] = ranks[c][dloc[m]]
    srow = trow[src]
    chunk = srow // CHR

    S = np.ones((NBLK, NCHUNK), dtype=np.int64)
    for c in range(NCORES):
        m = core == c
        cnt = np.bincount((erow[m] * NCHUNK + chunk[m]).astype(np.int64),
                          minlength=SH * NCHUNK).reshape(SH, NCHUNK)
        full = np.zeros((SHP, NCHUNK), dtype=np.int64)
        full[:SH] = cnt
        S = np.maximum(S, full.reshape(NBLK, 128, NCHUNK).max(axis=1))
    Stot = int(S.sum())

    idx_all, msk_all, blc_all, cnt_all = [], [], [], []
    for c in range(NCORES):
        m = np.nonzero(core == c)[0]
        key = erow[m] * NCHUNK + chunk[m]
        order = np.argsort(key, kind="stable")
        ms = m[order]
        rk, ck = erow[ms], chunk[ms]
        gid = (rk * NCHUNK + ck).astype(np.int64)
        first = np.zeros(SH * NCHUNK + 1, dtype=np.int64)
        np.cumsum(np.bincount(gid, minlength=SH * NCHUNK), out=first[1:])
        slot = np.arange(ms.size) - first[gid]

        iarr = np.zeros((Stot * 128,), dtype=np.int16)
        marr = np.zeros((128, Stot), dtype=np.float32)
        carr = np.zeros((NBLK * NCHUNK,), dtype=np.int32)
        off = 0
        bb = rk // 128
        pp = rk % 128
        for b in range(NBLK):
            selb = bb == b
            for k in range(NCHUNK):
                s = int(S[b, k])
                sel = selb & (ck == k)
                p = pp[sel]
                sl = slot[sel]
                # per-core used slot count for this (block, chunk)
                u = int(sl.max()) + 1 if sl.size else 0
                seg = np.zeros(s * 128, dtype=np.int16)
                seg[sl * 128 + p] = (srow[ms[sel]] - k * CHR).astype(np.int16)
                seg[u * 128:] = -1            # trailing slots: skipped by DGE
                carr[b * NCHUNK + k] = u * 128
                iarr[off * 128:(off + s) * 128] = seg
                marr[p, off + sl] = 1.0
                off += s
        idx_all.append(_wrap_idx(iarr))
        msk_all.append(marr)
        cnt_all.append(np.tile(carr.reshape(1, -1), (1, 1)))
        bl = np.full((128, NBLK), 127.0, dtype=np.float32)
        for b in range(NBLK):
            lo = b * 128
            take = min(128, SH - lo)
            bl[:take, b] = batch[c * SH + perms[c][lo:lo + take]]
        blc_all.append(bl)

    return S, Stot, trow, perms, idx_all, msk_all, blc_all, cnt_all


def build_kernel(S):
    Stot = int(S.sum())
    nc = bacc.Bacc("TRN2", target_bir_lowering=False, num_swdge_queues=4,
                   dynamic_dma_scratch_size=40960)

    def dp(name, shape, dt=F32):
        return nc.declare_dram_parameter(name, shape, dt, isOutput=False)

    xT = dp("xT", [IN, NTAB], BF16)
    xTl = dp("xTl", [IN, SHP], BF16)
    w1 = dp("w1", [IN, 128], BF16)           # [Wl1 | Wr1]
    w2 = dp("w2", [64, 64], BF16)            # [Wl2 | Wr2]
    cnts = dp("cnts", [1, NBLK * NCHUNK], mybir.dt.int32)
    att1r = dp("att1r", [128, 64])
    att2r = dp("att2r", [128, 32])
    b1r = dp("b1r", [128, 64])
    b2r = dp("b2r", [128, 32])
    g1wp = dp("g1wp", [32, 32], BF16)
    g1br = dp("g1br", [128, 32])
    g2wr = dp("g2wr", [128, 32])
    l1wp = dp("l1wp", [32, 32], BF16)
    l1br = dp("l1br", [64, 32])
    l2wr = dp("l2wr", [64, 32])
    sc4 = dp("sc4", [128, 4])                  # [g2b, lin2b, 0, 0]
    io64 = dp("io64", [128, 64])
    ones132 = dp("ones132", [1, 32])
    id128 = dp("id128", [128, 128])
    bloc = dp("bloc", [128, NBLK])
    msk = dp("msk", [128, Stot])
    eidx = dp("eidx", [128, (Stot * 128) // 16], I16)

    out_y = nc.declare_dram_parameter("y", [64, 1], F32, isOutput=True)

    add = mybir.AluOpType.add
    mult = mybir.AluOpType.mult
    iseq = mybir.AluOpType.is_equal
    byp = mybir.AluOpType.bypass
    AX = mybir.AxisListType.X
    EXP = mybir.ActivationFunctionType.Exp
    RELU = mybir.ActivationFunctionType.Relu
    LRELU = mybir.ActivationFunctionType.Lrelu

    with tile.TileContext(nc) as tc:
        with (
            tc.tile_pool(name="const", bufs=1) as cp,
            tc.tile_pool(name="gat", bufs=4) as gpool,
            tc.tile_pool(name="wk", bufs=2) as wk,
            tc.tile_pool(name="vtp", bufs=3) as vtp,
            tc.tile_pool(name="ps", bufs=2, space="PSUM") as ps,
            tc.tile_pool(name="psacc", bufs=1, space="PSUM") as psacc,
            tc.tile_pool(name="big", bufs=1) as bigp,
            tc.tile_pool(name="dram", bufs=1, space="DRAM") as dram,
        ):
            tab1 = dram.tile([NTAB, 128], BF16)
            tab2 = dram.tile([NTAB, 128], BF16)
            hT_loc = dram.tile([64, SHP], BF16)
            hT_all = dram.tile([NCORES * 64, SHP], BF16)
            pool_in = dram.tile([48, 64], F32)
            pool_all = dram.tile([48, 64], F32)
            nc.gpsimd.load_library(mlp_lib)

            def lc(t, shape, dt=F32):
                tt = cp.tile(shape, dt, tag=t.name + "_t")
                nc.sync.dma_start(tt[:], t[:])
                return tt

            w1_t = lc(w1, [IN, 128], BF16)
            w2_t = lc(w2, [64, 64], BF16)
            cnts_t = lc(cnts, [1, NBLK * NCHUNK], mybir.dt.int32)
            att1_t = lc(att1r, [128, 64])
            att1b_t = cp.tile([128, 64], BF16, tag="att1b")
            nc.vector.tensor_copy(att1b_t[:], att1_t[:])
            att2_t = lc(att2r, [128, 32])
            att2b_t = cp.tile([128, 32], BF16, tag="att2b")
            nc.vector.tensor_copy(att2b_t[:], att2_t[:])
            b1_t = lc(b1r, [128, 64])
            b2_t = lc(b2r, [128, 32])
            g1w_t = lc(g1wp, [32, 32], BF16)
            g1b_t = lc(g1br, [128, 32])
            g2w_t = lc(g2wr, [128, 32])
            l1w_t = lc(l1wp, [32, 32], BF16)
            l1b_t = lc(l1br, [64, 32])
            l2w_t = lc(l2wr, [64, 32])
            sc4_t = lc(sc4, [128, 4])
            io64_t = lc(io64, [128, 64])
            on132_t = lc(ones132, [1, 32])
            id_t = lc(id128, [128, 128])
            idb_t = cp.tile([128, 128], BF16, tag="idb")
            nc.vector.tensor_copy(idb_t[:], id_t[:])
            bloc_t = lc(bloc, [128, NBLK])
            msk_t = bigp.tile([128, Stot], F32)
            nc.sync.dma_start(msk_t[:], msk[:])

            xr1_t = bigp.tile([128, NBLK * 64], BF16)
            xr2_t = bigp.tile([128, NBLK * 32], BF16)
            hTl_t = bigp.tile([64, SHP], BF16)
            h2all = bigp.tile([128, NBLK * 32], F32)
            gcnt_reg = nc.gpsimd.alloc_register("gcnt")
            for _gz in range(6):
                gz = gpool.tile([128, 16, 128], BF16, tag="g", bufs=6,
                                name=f"gz{_gz}")
                nc.vector.memset(gz[:], 0.0)

            # ---------------- L1 global table + local xr1 ----------------
            for n0 in range(0, NTAB, 512):
                pt = ps.tile([128, 512], F32, tag="mm")
                xin = wk.tile([IN, 512], BF16, tag="xin")
                nc.sync.dma_start(xin[:], xT[:, n0:n0 + 512])
                for j in range(4):
                    nc.tensor.matmul(pt[:, j * 128:(j + 1) * 128],
                                     xin[:, j * 128:(j + 1) * 128], w1_t[:],
                                     start=True, stop=True)
                st = wk.tile([128, 512], BF16, tag="tsb")
                nc.scalar.copy(st[:], pt[:])
                nc.sync.dma_start(
                    tab1[n0:n0 + 512, :].rearrange("(j p) c -> p j c", p=128),
                    st[:].rearrange("p (j c) -> p j c", j=4))
            for n0 in range(0, SHP, 512):
                w_ = min(512, SHP - n0)
                pt = ps.tile([128, 512], F32, tag="mm")
                xin = wk.tile([IN, 512], BF16, tag="xin")
                nc.sync.dma_start(xin[:, :w_], xTl[:, n0:n0 + w_])
                for j in range(w_ // 128):
                    nc.tensor.matmul(pt[:, j * 128:(j + 1) * 128],
                                     xin[:, j * 128:(j + 1) * 128], w1_t[:],
                                     start=True, stop=True)
                st = wk.tile([128, 512], BF16, tag="tsb")
                nc.scalar.copy(st[:, :w_], pt[:, :w_])
                for j in range(w_ // 128):
                    b = n0 // 128 + j
                    nc.vector.tensor_copy(
                        xr1_t[:, b * 64:(b + 1) * 64],
                        st[:, j * 128 + 64:j * 128 + 128])

            tc.strict_bb_all_engine_barrier()

            def edge_layer(tab, xr_t, att_t, bias_t, F, heads, hall, gw, gdt):
                ioff = 0
                soff = 0
                for b in range(NBLK):
                    Ss = [int(S[b, k]) for k in range(NCHUNK)]
                    ST = sum(Ss)
                    acc4 = wk.tile([128, F, NCHUNK], F32, tag="acc4")
                    ixb = wk.tile([128, ST * 8], I16, tag="ix")
                    nc.sync.dma_start(ixb[:], eidx[:, ioff:ioff + ST * 8])
                    scob = wk.tile([128, ST * heads], F32, tag="scob")
                    wexpb = wk.tile([128, ST * heads], F32, tag="wexpb")
                    gs = []
                    iof2 = 0
                    co = 0
                    for k in range(NCHUNK):
                        s = Ss[k]
                        ni = s * 128
                        g = gpool.tile([128, s, gw], gdt, tag="g", bufs=6)
                        nc.gpsimd.reg_load(
                            gcnt_reg,
                            cnts_t[0:1, b * NCHUNK + k:b * NCHUNK + k + 1])
                        nc.gpsimd.dma_gather(
                            g[:], tab[k * CHR:(k + 1) * CHR, 0:gw],
                            ixb[:, iof2:iof2 + ni // 16],
                            ni, gcnt_reg, gw,
                            single_packet=False,
                            queue_num=(b * NCHUNK + k) % 4)
                        gs.append(g)
                        iof2 += ni // 16
                        ioff += ni // 16
                        xb = xr_t[:, b * F:(b + 1) * F]
                        sadd = wk.tile([128, s, F], gdt, tag="sadd")
                        nc.vector.tensor_tensor(
                            sadd[:], g[:, :, 0:F],
                            xb.unsqueeze(1).broadcast_to([128, s, F]), op=add)
                        nc.scalar.activation(sadd[:], sadd[:], LRELU,
                                             alpha=NEG)
                        nc.vector.tensor_tensor(
                            sadd[:], sadd[:],
                            att_t[:, 0:F].unsqueeze(1)
                            .broadcast_to([128, s, F]), op=mult)
                        nc.vector.tensor_reduce(
                            scob[:, co * heads:(co + s) * heads],
                            sadd[:].rearrange("p s (h c) -> p (s h) c", c=32),
                            axis=AX, op=add)
                        co += s
                    nc.vector.tensor_scalar_min(scob[:], scob[:], 30.0)
                    nc.scalar.activation(wexpb[:], scob[:], EXP)
                    wv = wexpb[:].rearrange("p (s h) -> p s h", h=heads)
                    nc.vector.tensor_tensor(
                        wv, wv,
                        msk_t[:, soff:soff + ST].unsqueeze(2)
                        .broadcast_to([128, ST, heads]), op=mult)
                    den = wk.tile([128, heads], F32, tag="den")
                    nc.vector.tensor_reduce(
                        den[:], wexpb[:].rearrange("p (s h) -> p h s", h=heads),
                        axis=AX, op=add)
                    co = 0
                    for k in range(NCHUNK):
                        s = Ss[k]
                        g = gs[k]
                        vt = vtp.tile([128, F, s], F32, tag="vt")
                        wvk = wv[:, co:co + s, :]
                        for h in range(heads):
                            nc.vector.tensor_tensor(
                                vt[:, h * 32:(h + 1) * 32, :]
                                .rearrange("p c s -> p s c"),
                                g[:, :, h * 32:h * 32 + 32],
                                wvk[:, :, h:h + 1].broadcast_to([128, s, 32]),
                                op=mult)
                        nc.vector.tensor_reduce(
                            acc4[:, :, k:k + 1].rearrange("p c o -> p (c o)"),
                            vt[:], axis=AX, op=add)
                        co += s
                    acc = wk.tile([128, F], F32, tag="acc")
                    nc.vector.tensor_reduce(acc[:], acc4[:], axis=AX, op=add)
                    nc.vector.tensor_scalar_max(den[:], den[:], 1e-30)
                    nc.vector.reciprocal(den[:], den[:])
                    hb = wk.tile([128, F], F32, tag="hb")
                    for h in range(heads):
                        nc.vector.tensor_tensor(
                            hb[:, h * 32:(h + 1) * 32],
                            acc[:, h * 32:(h + 1) * 32],
                            den[:, h:h + 1].broadcast_to([128, 32]), op=mult)
                    nc.vector.tensor_tensor(
                        hb[:], hb[:],
                        bias_t[:, 0:F], op=add)
                    nc.vector.tensor_scalar_max(
                        hall[:, b * F:(b + 1) * F], hb[:], 0.0)
                    soff += ST

            # ---------------- Layer 1 ----------------
            h1all = bigp.tile([128, NBLK * 64], BF16)
            edge_layer(tab1, xr1_t, att1b_t, b1_t, 64, H, h1all, 128, BF16)

            for b in range(NBLK):
                pt = ps.tile([64, 128], BF16, tag="mm")
                nc.tensor.transpose(pt[:], h1all[:, b * 64:(b + 1) * 64],
                                    idb_t[:])
                nc.scalar.copy(hTl_t[:, b * 128:(b + 1) * 128], pt[:])
            nc.sync.dma_start(hT_loc[:], hTl_t[:])

            tc.strict_bb_all_engine_barrier()
            nc.gpsimd.collective_compute(
                "AllGather", byp,
                replica_groups=[list(range(NCORES))],
                ins=[hT_loc.opt()], outs=[hT_all.opt()])
            tc.strict_bb_all_engine_barrier()

            # ---------------- L2 table + local xr2 ----------------
            for c in range(NCORES):
                for n0 in range(0, SHP, 512):
                    w_ = min(512, SHP - n0)
                    hinb = wk.tile([64, 512], BF16, tag="hinb")
                    nc.sync.dma_start(
                        hinb[:, :w_], hT_all[c * 64:(c + 1) * 64, n0:n0 + w_])
                    pt = ps.tile([128, 256], F32, tag="mm")
                    for j in range(w_ // 128):
                        nc.tensor.matmul(pt[:, j * 64:(j + 1) * 64],
                                         hinb[:, j * 128:(j + 1) * 128],
                                         w2_t[:], start=True, stop=True)
                    st = wk.tile([128, 256], BF16, tag="t2sb")
                    nc.scalar.copy(st[:], pt[:])
                    base = c * SHP + n0
                    nc.sync.dma_start(
                        tab2[base:base + w_, 0:64]
                        .rearrange("(j p) c -> p j c", p=128),
                        st[:, :w_ // 2].rearrange("p (j c) -> p j c", c=64))

            for n0 in range(0, SHP, 512):
                w_ = min(512, SHP - n0)
                hinb = wk.tile([64, 512], BF16, tag="hinb")
                nc.vector.tensor_copy(hinb[:, :w_], hTl_t[:, n0:n0 + w_])
                pt = ps.tile([128, 256], F32, tag="mm")
                for j in range(w_ // 128):
                    nc.tensor.matmul(pt[:, j * 64:(j + 1) * 64],
                                     hinb[:, j * 128:(j + 1) * 128], w2_t[:],
                                     start=True, stop=True)
                st = wk.tile([128, 256], BF16, tag="t2sb")
                nc.scalar.copy(st[:], pt[:])
                for j in range(w_ // 128):
                    b = n0 // 128 + j
                    nc.vector.tensor_copy(xr2_t[:, b * 32:(b + 1) * 32],
                                          st[:, j * 64 + 32:j * 64 + 64])

            tc.strict_bb_all_engine_barrier()

            # ---------------- Layer 2 ----------------
            edge_layer(tab2, xr2_t, att2b_t, b2_t, 32, 1, h2all, 128, BF16)

            # ---------------- pooling ----------------
            pp = psacc.tile([34, 64], F32)
            for b in range(NBLK):
                h2 = h2all[:, b * 32:(b + 1) * 32]
                pt = ps.tile([32, 128], F32, tag="mm")
                nc.tensor.transpose(pt[:], h2, id_t[:])
                h2T = wk.tile([32, 128], BF16, tag="h2T")
                nc.vector.tensor_copy(h2T[:], pt[:])
                gp1 = ps.tile([128, 32], F32, tag="mm")
                nc.tensor.matmul(gp1[:], h2T[:], g1w_t[:],
                                 start=True, stop=True)
                ga = wk.tile([128, 32], F32, tag="ga")
                nc.vector.tensor_tensor(
                    ga[:], gp1[:],
                    g1b_t[:], op=add)
                nc.vector.tensor_scalar_max(ga[:], ga[:], 0.0)
                nc.vector.tensor_tensor(
                    ga[:], ga[:],
                    g2w_t[:], op=mult)
                gt = wk.tile([128, 1], F32, tag="gt")
                nc.vector.tensor_reduce(gt[:], ga[:], axis=AX, op=add)
                nc.vector.tensor_tensor(
                    gt[:], gt[:],
                    sc4_t[:, 0:1], op=add)
                ge = wk.tile([128, 1], F32, tag="ge")
                nc.scalar.activation(ge[:], gt[:], EXP)
                vg = wk.tile([128, 34], F32, tag="vg")
                nc.vector.tensor_tensor(
                    vg[:, 0:32], h2, ge[:].broadcast_to([128, 32]), op=mult)
                nc.vector.tensor_copy(vg[:, 32:33], ge[:])
                nc.vector.memset(vg[:, 33:34], 0.0)
                ohg = wk.tile([128, 64], F32, tag="ohg")
                nc.vector.tensor_scalar(
                    ohg[:], io64_t[:],
                    bloc_t[:, b:b + 1], None, op0=iseq)
                vgb = wk.tile([128, 34], BF16, tag="vgb")
                nc.vector.tensor_copy(vgb[:], vg[:])
                ohgb = wk.tile([128, 64], BF16, tag="ohgb")
                nc.vector.tensor_copy(ohgb[:], ohg[:])
                nc.tensor.matmul(pp[:], vgb[:], ohgb[:],
                                 start=(b == 0), stop=(b == NBLK - 1))

            pin = wk.tile([48, 64], F32, tag="pin")
            nc.vector.memset(pin[:], 0.0)
            nc.scalar.copy(pin[0:34, :], pp[:])
            nc.sync.dma_start(pool_in[:], pin[:])

            tc.strict_bb_all_engine_barrier()
            nc.gpsimd.collective_compute(
                "AllReduce", add,
                replica_groups=[list(range(NCORES))],
                ins=[pool_in.opt()], outs=[pool_all.opt()])
            tc.strict_bb_all_engine_barrier()

            pall = wk.tile([48, 64], F32, tag="pall")
            nc.sync.dma_start(pall[:], pool_all[:])
            dn = wk.tile([1, 64], F32, tag="dn")
            nc.vector.reciprocal(dn[:], pall[32:33, :])
            dnr = ps.tile([32, 64], F32, tag="mm")
            nc.tensor.matmul(dnr[:], on132_t[:], dn[:],
                             start=True, stop=True)
            pooledT = wk.tile([32, 64], BF16, tag="pooledT")
            nc.vector.tensor_tensor(
                pooledT[:], pall[0:32, :], dnr[:], op=mult)
            zp = ps.tile([64, 32], F32, tag="mm")
            nc.tensor.matmul(zp[:], pooledT[:], l1w_t[:],
                             start=True, stop=True)
            z = wk.tile([64, 32], F32, tag="z")
            nc.vector.tensor_tensor(
                z[:], zp[:], l1b_t[:], op=add)
            nc.vector.tensor_scalar_max(z[:], z[:], 0.0)
            nc.vector.tensor_tensor(
                z[:], z[:], l2w_t[:], op=mult)
            yv = wk.tile([64, 1], F32, tag="yv")
            nc.vector.tensor_reduce(yv[:], z[:], axis=AX, op=add)
            nc.vector.tensor_tensor(
                yv[:], yv[:], sc4_t[0:64, 1:2], op=add)
            nc.sync.dma_start(out_y[:], yv[:])

    nc.compile()
    return nc


def kernel(**inputs):
    x = np.asarray(inputs["x"], dtype=np.float32)
    edge_index = np.asarray(inputs["edge_index"])
    batch = np.asarray(inputs["batch"])
    key = (int(edge_index[:, ::4099].sum()), int(batch[::997].sum()))
    if key not in _CACHE:
        prep = host_prep(edge_index, batch)
        nc = build_kernel(prep[0])
        _CACHE.clear()
        _CACHE[key] = (prep, nc)
    (S, Stot, trow, perms, idx_all, msk_all, blc_all, cnt_all), nc = _CACHE[key]

    xp = np.zeros((NTAB, IN), dtype=np.float32)
    xp[trow] = x
    xT_full = np.ascontiguousarray(xp.T).astype(ml_dtypes.bfloat16)

    w1c = np.concatenate([inputs["Wl1"], inputs["Wr1"]], 1).astype(ml_dtypes.bfloat16)
    w2c = np.concatenate([inputs["Wl2"], inputs["Wr2"]], 1).astype(ml_dtypes.bfloat16)
    common = {
        "xT": xT_full, "w1": w1c, "w2": w2c,
        "att1r": np.tile(np.asarray(inputs["att1"], np.float32).reshape(1, 64), (128, 1)),
        "att2r": np.tile(np.asarray(inputs["att2"], np.float32).reshape(1, 32), (128, 1)),
        "b1r": np.tile(np.asarray(inputs["b1"], np.float32).reshape(1, 64), (128, 1)),
        "b2r": np.tile(np.asarray(inputs["b2"], np.float32).reshape(1, 32), (128, 1)),
        "g1wp": np.asarray(inputs["g1w"]).astype(ml_dtypes.bfloat16),
        "g1br": np.tile(np.asarray(inputs["g1b"], np.float32).reshape(1, 32), (128, 1)),
        "g2wr": np.tile(np.asarray(inputs["g2w"], np.float32).reshape(1, 32), (128, 1)),
        "l1wp": np.asarray(inputs["lin1w"]).astype(ml_dtypes.bfloat16),
        "l1br": np.tile(np.asarray(inputs["lin1b"], np.float32).reshape(1, 32), (64, 1)),
        "l2wr": np.tile(np.asarray(inputs["lin2w"], np.float32).reshape(1, 32), (64, 1)),
        "sc4": np.tile(np.array([[float(np.ravel(inputs["g2b"])[0]),
                          float(np.ravel(inputs["lin2b"])[0]), 0.0, 0.0]],
                        np.float32), (128, 1)),
        "io64": np.tile(np.arange(64, dtype=np.float32).reshape(1, 64), (128, 1)),
        "ones132": np.ones((1, 32), np.float32),
        "id128": np.eye(128, dtype=np.float32),
    }
    in_maps = []
    for c in range(NCORES):
        m = dict(common)
        m["xTl"] = np.ascontiguousarray(xT_full[:, c * SHP:(c + 1) * SHP])
        m["bloc"] = blc_all[c]
        m["msk"] = msk_all[c]
        m["eidx"] = idx_all[c]
        m["cnts"] = cnt_all[c]
        in_maps.append(m)

    res = run_bass_kernel_spmd(nc, in_maps, list(range(NCORES)))
    return res.results[0]["y"].reshape(G).astype(np.float32)



# revision 40
# speedup vs baseline: 1.1895x; 1.0942x over previous
"""GATv2 regressor on 8 Trainium2 NeuronCores (Bass).

Sharding: core c owns dst nodes [c*12500, (c+1)*12500), relabeled locally in
decreasing in-degree order so fixed-slot padding per 128-dst block is tight.
All per-edge indexing is host-precomputed; the device does dense DMA,
dma_gather by src table row, broadcast adds, free-dim reductions and small
matmuls. Softmax skips max-subtraction (scores are clamped before exp);
denominators are reduced from the masked exp tile and divided at the end.

Both layers' node tables are bf16 with 256B rows so each per-edge gather
descriptor moves one full row. Per-core gather lists end in a -1 suffix with
the exact valid count supplied at runtime through a gpsimd register, so cores
with fewer edges in a (block, chunk) cell skip the cross-core padding rows.
Activations are batched per 128-dst block (one Lrelu span, one Exp span) to
avoid per-chunk activation-table reloads.
"""
import numpy as np
import ml_dtypes

import concourse.bacc as bacc
import concourse.mybir as mybir
import concourse.tile as tile
from concourse.bass_utils import run_bass_kernel_spmd
from concourse.library_config import mlp as mlp_lib

F32 = mybir.dt.float32
BF16 = mybir.dt.bfloat16
I16 = mybir.dt.int16

N, E, IN, C, H, G = 100000, 1600000, 128, 32, 2, 64
NEG = 0.2
NCORES = 8
SH = 12500
SHP = 12544              # 98*128
NBLK = SHP // 128        # 98
NTAB = SHP * NCORES      # 100352
NCHUNK = 4
WWID = 32768             # gather window width (int16 index reach)
WBASE = np.array([0, 22528, 45056, 67584])   # overlapping window bases
_WB = np.array([22528, 32768, 45056, 55296, 67584, 77824])  # region bounds

_CACHE = {}


def _wrap_idx(idx):
    n = idx.shape[0]
    w = idx.reshape(n // 16, 16).T
    return np.tile(w, (8, 1)).astype(np.int16)


def _assign_windows(sr, rk):
    """2-choice balance: edges in window overlaps go to the less-loaded
    window of their dst, flattening per-(dst, window) counts."""
    reg = np.searchsorted(_WB, sr, side="right")
    C7 = np.zeros((SH, 7), np.int64)
    np.add.at(C7, (rk, reg), 1)
    e = C7[:, [0, 2, 4, 6]].astype(np.float64)
    o = C7[:, [1, 3, 5]].astype(np.float64)
    a = o / 2
    for _ in range(8):
        l0 = e[:, 0] + a[:, 0]
        l1 = e[:, 1] + (o[:, 0] - a[:, 0]) + a[:, 1]
        l2 = e[:, 2] + (o[:, 1] - a[:, 1]) + a[:, 2]
        l3 = e[:, 3] + (o[:, 2] - a[:, 2])
        a[:, 0] = np.clip(a[:, 0] + (l1 - l0) / 2, 0, o[:, 0])
        a[:, 1] = np.clip(a[:, 1] + (l2 - l1) / 2, 0, o[:, 1])
        a[:, 2] = np.clip(a[:, 2] + (l3 - l2) / 2, 0, o[:, 2])
    A = np.minimum(np.rint(a).astype(np.int64), C7[:, [1, 3, 5]])
    key = rk * 7 + reg
    order = np.argsort(key, kind="stable")
    first = np.zeros(SH * 7 + 1, np.int64)
    np.cumsum(np.bincount(key, minlength=SH * 7), out=first[1:])
    rig = np.empty(sr.size, np.int64)
    rig[order] = np.arange(sr.size) - first[key[order]]
    w = np.empty(sr.size, np.int64)
    excl = (reg % 2 == 0)
    w[excl] = reg[excl] // 2
    ov = ~excl
    ovi = (reg[ov] - 1) // 2
    left = rig[ov] < A[rk[ov], ovi]
    w[ov] = np.where(left, ovi, ovi + 1)
    return w


def host_prep(edge_index, batch):
    src = edge_index[0].astype(np.int64)
    dst = edge_index[1].astype(np.int64)
    core = dst // SH
    dloc = dst % SH

    perms, ranks = [], []
    for c in range(NCORES):
        deg = np.bincount(dloc[core == c], minlength=SH)
        p = np.argsort(-deg, kind="stable")
        r = np.empty(SH, dtype=np.int64)
        r[p] = np.arange(SH)
        perms.append(p)
        ranks.append(r)

    ncore = np.arange(N) // SH
    nloc = np.arange(N) % SH
    trow = np.empty(N, dtype=np.int64)
    for c in range(NCORES):
        m = ncore == c
        trow[m] = c * SHP + ranks[c][nloc[m]]

    erow = np.empty(E, dtype=np.int64)
    for c in range(NCORES):
        m = core == c
        erow[m] = ranks[c][dloc[m]]
    srow = trow[src]
    wofe = np.empty(E, dtype=np.int64)
    for c in range(NCORES):
        m = core == c
        wofe[m] = _assign_windows(srow[m], erow[m])

    S = np.ones((NBLK, NCHUNK), dtype=np.int64)
    for c in range(NCORES):
        m = core == c
        cnt = np.bincount((erow[m] * NCHUNK + wofe[m]).astype(np.int64),
                          minlength=SH * NCHUNK).reshape(SH, NCHUNK)
        full = np.zeros((SHP, NCHUNK), dtype=np.int64)
        full[:SH] = cnt
        S = np.maximum(S, full.reshape(NBLK, 128, NCHUNK).max(axis=1))
    Stot = int(S.sum())

    idx_all, msk_all, blc_all, cnt_all = [], [], [], []
    for c in range(NCORES):
        m = np.nonzero(core == c)[0]
        key = erow[m] * NCHUNK + wofe[m]
        order = np.argsort(key, kind="stable")
        ms = m[order]
        rk, ck = erow[ms], wofe[ms]
        gid = (rk * NCHUNK + ck).astype(np.int64)
        first = np.zeros(SH * NCHUNK + 1, dtype=np.int64)
        np.cumsum(np.bincount(gid, minlength=SH * NCHUNK), out=first[1:])
        slot = np.arange(ms.size) - first[gid]

        iarr = np.zeros((Stot * 128,), dtype=np.int16)
        marr = np.zeros((128, Stot), dtype=np.float32)
        carr = np.zeros((NBLK * NCHUNK,), dtype=np.int32)
        off = 0
        bb = rk // 128
        pp = rk % 128
        for b in range(NBLK):
            selb = bb == b
            for k in range(NCHUNK):
                s = int(S[b, k])
                sel = selb & (ck == k)
                p = pp[sel]
                sl = slot[sel]
                # per-core used slot count for this (block, chunk)
                u = int(sl.max()) + 1 if sl.size else 0
                seg = np.zeros(s * 128, dtype=np.int16)
                seg[sl * 128 + p] = (srow[ms[sel]] - WBASE[k]).astype(np.int16)
                seg[u * 128:] = -1            # trailing slots: skipped by DGE
                carr[b * NCHUNK + k] = u * 128
                iarr[off * 128:(off + s) * 128] = seg
                marr[p, off + sl] = 1.0
                off += s
        idx_all.append(_wrap_idx(iarr))
        msk_all.append(marr)
        cnt_all.append(np.tile(carr.reshape(1, -1), (1, 1)))
        bl = np.full((128, NBLK), 127.0, dtype=np.float32)
        for b in range(NBLK):
            lo = b * 128
            take = min(128, SH - lo)
            bl[:take, b] = batch[c * SH + perms[c][lo:lo + take]]
        blc_all.append(bl)

    return S, Stot, trow, perms, idx_all, msk_all, blc_all, cnt_all


def build_kernel(S):
    Stot = int(S.sum())
    nc = bacc.Bacc("TRN2", target_bir_lowering=False, num_swdge_queues=4,
                   dynamic_dma_scratch_size=40960)

    def dp(name, shape, dt=F32):
        return nc.declare_dram_parameter(name, shape, dt, isOutput=False)

    xT = dp("xT", [IN, NTAB], BF16)
    xTl = dp("xTl", [IN, SHP], BF16)
    w1 = dp("w1", [IN, 128], BF16)           # [Wl1 | Wr1]
    w2 = dp("w2", [64, 64], BF16)            # [Wl2 | Wr2]
    cnts = dp("cnts", [1, NBLK * NCHUNK], mybir.dt.int32)
    att1r = dp("att1r", [128, 64])
    att2r = dp("att2r", [128, 32])
    b1r = dp("b1r", [128, 64])
    b2r = dp("b2r", [128, 32])
    g1wp = dp("g1wp", [32, 32], BF16)
    g1br = dp("g1br", [128, 32])
    g2wr = dp("g2wr", [128, 32])
    l1wp = dp("l1wp", [32, 32], BF16)
    l1br = dp("l1br", [64, 32])
    l2wr = dp("l2wr", [64, 32])
    sc4 = dp("sc4", [128, 4])                  # [g2b, lin2b, 0, 0]
    io64 = dp("io64", [128, 64])
    ones132 = dp("ones132", [1, 32])
    id128 = dp("id128", [128, 128])
    bloc = dp("bloc", [128, NBLK])
    msk = dp("msk", [128, Stot])
    eidx = dp("eidx", [128, (Stot * 128) // 16], I16)

    out_y = nc.declare_dram_parameter("y", [64, 1], F32, isOutput=True)

    add = mybir.AluOpType.add
    mult = mybir.AluOpType.mult
    iseq = mybir.AluOpType.is_equal
    byp = mybir.AluOpType.bypass
    AX = mybir.AxisListType.X
    EXP = mybir.ActivationFunctionType.Exp
    RELU = mybir.ActivationFunctionType.Relu
    LRELU = mybir.ActivationFunctionType.Lrelu

    with tile.TileContext(nc) as tc:
        with (
            tc.tile_pool(name="const", bufs=1) as cp,
            tc.tile_pool(name="gat", bufs=4) as gpool,
            tc.tile_pool(name="wk", bufs=2) as wk,
            tc.tile_pool(name="vtp", bufs=3) as vtp,
            tc.tile_pool(name="ps", bufs=2, space="PSUM") as ps,
            tc.tile_pool(name="psacc", bufs=1, space="PSUM") as psacc,
            tc.tile_pool(name="big", bufs=1) as bigp,
            tc.tile_pool(name="dram", bufs=1, space="DRAM") as dram,
        ):
            tab1 = dram.tile([NTAB, 128], BF16)
            tab2 = dram.tile([NTAB, 128], BF16)
            hT_loc = dram.tile([64, SHP], BF16)
            hT_all = dram.tile([NCORES * 64, SHP], BF16)
            pool_in = dram.tile([48, 64], F32)
            pool_all = dram.tile([48, 64], F32)
            nc.gpsimd.load_library(mlp_lib)

            def lc(t, shape, dt=F32):
                tt = cp.tile(shape, dt, tag=t.name + "_t")
                nc.sync.dma_start(tt[:], t[:])
                return tt

            w1_t = lc(w1, [IN, 128], BF16)
            w2_t = lc(w2, [64, 64], BF16)
            cnts_t = lc(cnts, [1, NBLK * NCHUNK], mybir.dt.int32)
            att1_t = lc(att1r, [128, 64])
            att1b_t = cp.tile([128, 64], BF16, tag="att1b")
            nc.vector.tensor_copy(att1b_t[:], att1_t[:])
            att2_t = lc(att2r, [128, 32])
            att2b_t = cp.tile([128, 32], BF16, tag="att2b")
            nc.vector.tensor_copy(att2b_t[:], att2_t[:])
            b1_t = lc(b1r, [128, 64])
            b2_t = lc(b2r, [128, 32])
            g1w_t = lc(g1wp, [32, 32], BF16)
            g1b_t = lc(g1br, [128, 32])
            g2w_t = lc(g2wr, [128, 32])
            l1w_t = lc(l1wp, [32, 32], BF16)
            l1b_t = lc(l1br, [64, 32])
            l2w_t = lc(l2wr, [64, 32])
            sc4_t = lc(sc4, [128, 4])
            io64_t = lc(io64, [128, 64])
            on132_t = lc(ones132, [1, 32])
            id_t = lc(id128, [128, 128])
            idb_t = cp.tile([128, 128], BF16, tag="idb")
            nc.vector.tensor_copy(idb_t[:], id_t[:])
            bloc_t = lc(bloc, [128, NBLK])
            msk_t = bigp.tile([128, Stot], F32)
            nc.sync.dma_start(msk_t[:], msk[:])

            xr1_t = bigp.tile([128, NBLK * 64], BF16)
            xr2_t = bigp.tile([128, NBLK * 32], BF16)
            hTl_t = bigp.tile([64, SHP], BF16)
            h2all = bigp.tile([128, NBLK * 32], F32)
            gcnt_reg = nc.gpsimd.alloc_register("gcnt")
            for _gz in range(6):
                gz = gpool.tile([128, 16, 128], BF16, tag="g", bufs=6,
                                name=f"gz{_gz}")
                nc.vector.memset(gz[:], 0.0)

            # ---------------- L1 global table + local xr1 ----------------
            for n0 in range(0, NTAB, 512):
                pt = ps.tile([128, 512], F32, tag="mm")
                xin = wk.tile([IN, 512], BF16, tag="xin")
                nc.sync.dma_start(xin[:], xT[:, n0:n0 + 512])
                for j in range(4):
                    nc.tensor.matmul(pt[:, j * 128:(j + 1) * 128],
                                     xin[:, j * 128:(j + 1) * 128], w1_t[:],
                                     start=True, stop=True)
                st = wk.tile([128, 512], BF16, tag="tsb")
                nc.scalar.copy(st[:], pt[:])
                nc.sync.dma_start(
                    tab1[n0:n0 + 512, :].rearrange("(j p) c -> p j c", p=128),
                    st[:].rearrange("p (j c) -> p j c", j=4))
            for n0 in range(0, SHP, 512):
                w_ = min(512, SHP - n0)
                pt = ps.tile([128, 512], F32, tag="mm")
                xin = wk.tile([IN, 512], BF16, tag="xin")
                nc.sync.dma_start(xin[:, :w_], xTl[:, n0:n0 + w_])
                for j in range(w_ // 128):
                    nc.tensor.matmul(pt[:, j * 128:(j + 1) * 128],
                                     xin[:, j * 128:(j + 1) * 128], w1_t[:],
                                     start=True, stop=True)
                st = wk.tile([128, 512], BF16, tag="tsb")
                nc.scalar.copy(st[:, :w_], pt[:, :w_])
                for j in range(w_ // 128):
                    b = n0 // 128 + j
                    nc.vector.tensor_copy(
                        xr1_t[:, b * 64:(b + 1) * 64],
                        st[:, j * 128 + 64:j * 128 + 128])

            tc.strict_bb_all_engine_barrier()

            def edge_layer(tab, xr_t, att_t, bias_t, F, heads, hall, gw, gdt):
                ioff = 0
                soff = 0
                for b in range(NBLK):
                    Ss = [int(S[b, k]) for k in range(NCHUNK)]
                    ST = sum(Ss)
                    acc4 = wk.tile([128, F, NCHUNK], F32, tag="acc4")
                    ixb = wk.tile([128, ST * 8], I16, tag="ix")
                    nc.sync.dma_start(ixb[:], eidx[:, ioff:ioff + ST * 8])
                    scob = wk.tile([128, ST * heads], F32, tag="scob")
                    wexpb = wk.tile([128, ST * heads], F32, tag="wexpb")
                    gs = []
                    iof2 = 0
                    co = 0
                    for k in range(NCHUNK):
                        s = Ss[k]
                        ni = s * 128
                        g = gpool.tile([128, s, gw], gdt, tag="g", bufs=6)
                        nc.gpsimd.reg_load(
                            gcnt_reg,
                            cnts_t[0:1, b * NCHUNK + k:b * NCHUNK + k + 1])
                        nc.gpsimd.dma_gather(
                            g[:], tab[int(WBASE[k]):int(WBASE[k]) + WWID, 0:gw],
                            ixb[:, iof2:iof2 + ni // 16],
                            ni, gcnt_reg, gw,
                            single_packet=False,
                            queue_num=(b * NCHUNK + k) % 4)
                        gs.append(g)
                        iof2 += ni // 16
                        ioff += ni // 16
                        xb = xr_t[:, b * F:(b + 1) * F]
                        sadd = wk.tile([128, s, F], gdt, tag="sadd")
                        nc.vector.tensor_tensor(
                            sadd[:], g[:, :, 0:F],
                            xb.unsqueeze(1).broadcast_to([128, s, F]), op=add)
                        nc.scalar.activation(sadd[:], sadd[:], LRELU,
                                             alpha=NEG)
                        nc.vector.tensor_tensor(
                            sadd[:], sadd[:],
                            att_t[:, 0:F].unsqueeze(1)
                            .broadcast_to([128, s, F]), op=mult)
                        nc.vector.tensor_reduce(
                            scob[:, co * heads:(co + s) * heads],
                            sadd[:].rearrange("p s (h c) -> p (s h) c", c=32),
                            axis=AX, op=add)
                        co += s
                    nc.vector.tensor_scalar_min(scob[:], scob[:], 30.0)
                    nc.scalar.activation(wexpb[:], scob[:], EXP)
                    wv = wexpb[:].rearrange("p (s h) -> p s h", h=heads)
                    nc.vector.tensor_tensor(
                        wv, wv,
                        msk_t[:, soff:soff + ST].unsqueeze(2)
                        .broadcast_to([128, ST, heads]), op=mult)
                    den = wk.tile([128, heads], F32, tag="den")
                    nc.vector.tensor_reduce(
                        den[:], wexpb[:].rearrange("p (s h) -> p h s", h=heads),
                        axis=AX, op=add)
                    co = 0
                    for k in range(NCHUNK):
                        s = Ss[k]
                        g = gs[k]
                        vt = vtp.tile([128, F, s], F32, tag="vt")
                        wvk = wv[:, co:co + s, :]
                        for h in range(heads):
                            nc.vector.tensor_tensor(
                                vt[:, h * 32:(h + 1) * 32, :]
                                .rearrange("p c s -> p s c"),
                                g[:, :, h * 32:h * 32 + 32],
                                wvk[:, :, h:h + 1].broadcast_to([128, s, 32]),
                                op=mult)
                        nc.vector.tensor_reduce(
                            acc4[:, :, k:k + 1].rearrange("p c o -> p (c o)"),
                            vt[:], axis=AX, op=add)
                        co += s
                    acc = wk.tile([128, F], F32, tag="acc")
                    nc.vector.tensor_reduce(acc[:], acc4[:], axis=AX, op=add)
                    nc.vector.tensor_scalar_max(den[:], den[:], 1e-30)
                    nc.vector.reciprocal(den[:], den[:])
                    hb = wk.tile([128, F], F32, tag="hb")
                    for h in range(heads):
                        nc.vector.tensor_tensor(
                            hb[:, h * 32:(h + 1) * 32],
                            acc[:, h * 32:(h + 1) * 32],
                            den[:, h:h + 1].broadcast_to([128, 32]), op=mult)
                    nc.vector.tensor_tensor(
                        hb[:], hb[:],
                        bias_t[:, 0:F], op=add)
                    nc.vector.tensor_scalar_max(
                        hall[:, b * F:(b + 1) * F], hb[:], 0.0)
                    soff += ST

            # ---------------- Layer 1 ----------------
            h1all = bigp.tile([128, NBLK * 64], BF16)
            edge_layer(tab1, xr1_t, att1b_t, b1_t, 64, H, h1all, 128, BF16)

            for b in range(NBLK):
                pt = ps.tile([64, 128], BF16, tag="mm")
                nc.tensor.transpose(pt[:], h1all[:, b * 64:(b + 1) * 64],
                                    idb_t[:])
                nc.scalar.copy(hTl_t[:, b * 128:(b + 1) * 128], pt[:])
            nc.sync.dma_start(hT_loc[:], hTl_t[:])

            tc.strict_bb_all_engine_barrier()
            nc.gpsimd.collective_compute(
                "AllGather", byp,
                replica_groups=[list(range(NCORES))],
                ins=[hT_loc.opt()], outs=[hT_all.opt()])
            tc.strict_bb_all_engine_barrier()

            # ---------------- L2 table + local xr2 ----------------
            for c in range(NCORES):
                for n0 in range(0, SHP, 512):
                    w_ = min(512, SHP - n0)
                    hinb = wk.tile([64, 512], BF16, tag="hinb")
                    nc.sync.dma_start(
                        hinb[:, :w_], hT_all[c * 64:(c + 1) * 64, n0:n0 + w_])
                    pt = ps.tile([128, 256], F32, tag="mm")
                    for j in range(w_ // 128):
                        nc.tensor.matmul(pt[:, j * 64:(j + 1) * 64],
                                         hinb[:, j * 128:(j + 1) * 128],
                                         w2_t[:], start=True, stop=True)
                    st = wk.tile([128, 256], BF16, tag="t2sb")
                    nc.scalar.copy(st[:], pt[:])
                    base = c * SHP + n0
                    nc.sync.dma_start(
                        tab2[base:base + w_, 0:64]
                        .rearrange("(j p) c -> p j c", p=128),
                        st[:, :w_ // 2].rearrange("p (j c) -> p j c", c=64))

            for n0 in range(0, SHP, 512):
                w_ = min(512, SHP - n0)
                hinb = wk.tile([64, 512], BF16, tag="hinb")
                nc.vector.tensor_copy(hinb[:, :w_], hTl_t[:, n0:n0 + w_])
                pt = ps.tile([128, 256], F32, tag="mm")
                for j in range(w_ // 128):
                    nc.tensor.matmul(pt[:, j * 64:(j + 1) * 64],
                                     hinb[:, j * 128:(j + 1) * 128], w2_t[:],
                                     start=True, stop=True)
                st = wk.tile([128, 256], BF16, tag="t2sb")
                nc.scalar.copy(st[:], pt[:])
                for j in range(w_ // 128):
                    b = n0 // 128 + j
                    nc.vector.tensor_copy(xr2_t[:, b * 32:(b + 1) * 32],
                                          st[:, j * 64 + 32:j * 64 + 64])

            tc.strict_bb_all_engine_barrier()

            # ---------------- Layer 2 ----------------
            edge_layer(tab2, xr2_t, att2b_t, b2_t, 32, 1, h2all, 128, BF16)

            # ---------------- pooling ----------------
            pp = psacc.tile([34, 64], F32)
            for b in range(NBLK):
                h2 = h2all[:, b * 32:(b + 1) * 32]
                pt = ps.tile([32, 128], F32, tag="mm")
                nc.tensor.transpose(pt[:], h2, id_t[:])
                h2T = wk.tile([32, 128], BF16, tag="h2T")
                nc.vector.tensor_copy(h2T[:], pt[:])
                gp1 = ps.tile([128, 32], F32, tag="mm")
                nc.tensor.matmul(gp1[:], h2T[:], g1w_t[:],
                                 start=True, stop=True)
                ga = wk.tile([128, 32], F32, tag="ga")
                nc.vector.tensor_tensor(
                    ga[:], gp1[:],
                    g1b_t[:], op=add)
                nc.vector.tensor_scalar_max(ga[:], ga[:], 0.0)
                nc.vector.tensor_tensor(
                    ga[:], ga[:],
                    g2w_t[:], op=mult)
                gt = wk.tile([128, 1], F32, tag="gt")
                nc.vector.tensor_reduce(gt[:], ga[:], axis=AX, op=add)
                nc.vector.tensor_tensor(
                    gt[:], gt[:],
                    sc4_t[:, 0:1], op=add)
                ge = wk.tile([128, 1], F32, tag="ge")
                nc.scalar.activation(ge[:], gt[:], EXP)
                vg = wk.tile([128, 34], F32, tag="vg")
                nc.vector.tensor_tensor(
                    vg[:, 0:32], h2, ge[:].broadcast_to([128, 32]), op=mult)
                nc.vector.tensor_copy(vg[:, 32:33], ge[:])
                nc.vector.memset(vg[:, 33:34], 0.0)
                ohg = wk.tile([128, 64], F32, tag="ohg")
                nc.vector.tensor_scalar(
                    ohg[:], io64_t[:],
                    bloc_t[:, b:b + 1], None, op0=iseq)
                vgb = wk.tile([128, 34], BF16, tag="vgb")
                nc.vector.tensor_copy(vgb[:], vg[:])
                ohgb = wk.tile([128, 64], BF16, tag="ohgb")
                nc.vector.tensor_copy(ohgb[:], ohg[:])
                nc.tensor.matmul(pp[:], vgb[:], ohgb[:],
                                 start=(b == 0), stop=(b == NBLK - 1))

            pin = wk.tile([48, 64], F32, tag="pin")
            nc.vector.memset(pin[:], 0.0)
            nc.scalar.copy(pin[0:34, :], pp[:])
            nc.sync.dma_start(pool_in[:], pin[:])

            tc.strict_bb_all_engine_barrier()
            nc.gpsimd.collective_compute(
                "AllReduce", add,
                replica_groups=[list(range(NCORES))],
                ins=[pool_in.opt()], outs=[pool_all.opt()])
            tc.strict_bb_all_engine_barrier()

            pall = wk.tile([48, 64], F32, tag="pall")
            nc.sync.dma_start(pall[:], pool_all[:])
            dn = wk.tile([1, 64], F32, tag="dn")
            nc.vector.reciprocal(dn[:], pall[32:33, :])
            dnr = ps.tile([32, 64], F32, tag="mm")
            nc.tensor.matmul(dnr[:], on132_t[:], dn[:],
                             start=True, stop=True)
            pooledT = wk.tile([32, 64], BF16, tag="pooledT")
            nc.vector.tensor_tensor(
                pooledT[:], pall[0:32, :], dnr[:], op=mult)
            zp = ps.tile([64, 32], F32, tag="mm")
            nc.tensor.matmul(zp[:], pooledT[:], l1w_t[:],
                             start=True, stop=True)
            z = wk.tile([64, 32], F32, tag="z")
            nc.vector.tensor_tensor(
                z[:], zp[:], l1b_t[:], op=add)
            nc.vector.tensor_scalar_max(z[:], z[:], 0.0)
            nc.vector.tensor_tensor(
                z[:], z[:], l2w_t[:], op=mult)
            yv = wk.tile([64, 1], F32, tag="yv")
            nc.vector.tensor_reduce(yv[:], z[:], axis=AX, op=add)
            nc.vector.tensor_tensor(
                yv[:], yv[:], sc4_t[0:64, 1:2], op=add)
            nc.sync.dma_start(out_y[:], yv[:])

    nc.compile()
    return nc


def kernel(**inputs):
    x = np.asarray(inputs["x"], dtype=np.float32)
    edge_index = np.asarray(inputs["edge_index"])
    batch = np.asarray(inputs["batch"])
    key = (int(edge_index[:, ::4099].sum()), int(batch[::997].sum()))
    if key not in _CACHE:
        prep = host_prep(edge_index, batch)
        nc = build_kernel(prep[0])
        _CACHE.clear()
        _CACHE[key] = (prep, nc)
    (S, Stot, trow, perms, idx_all, msk_all, blc_all, cnt_all), nc = _CACHE[key]

    xp = np.zeros((NTAB, IN), dtype=np.float32)
    xp[trow] = x
    xT_full = np.ascontiguousarray(xp.T).astype(ml_dtypes.bfloat16)

    w1c = np.concatenate([inputs["Wl1"], inputs["Wr1"]], 1).astype(ml_dtypes.bfloat16)
    w2c = np.concatenate([inputs["Wl2"], inputs["Wr2"]], 1).astype(ml_dtypes.bfloat16)
    common = {
        "xT": xT_full, "w1": w1c, "w2": w2c,
        "att1r": np.tile(np.asarray(inputs["att1"], np.float32).reshape(1, 64), (128, 1)),
        "att2r": np.tile(np.asarray(inputs["att2"], np.float32).reshape(1, 32), (128, 1)),
        "b1r": np.tile(np.asarray(inputs["b1"], np.float32).reshape(1, 64), (128, 1)),
        "b2r": np.tile(np.asarray(inputs["b2"], np.float32).reshape(1, 32), (128, 1)),
        "g1wp": np.asarray(inputs["g1w"]).astype(ml_dtypes.bfloat16),
        "g1br": np.tile(np.asarray(inputs["g1b"], np.float32).reshape(1, 32), (128, 1)),
        "g2wr": np.tile(np.asarray(inputs["g2w"], np.float32).reshape(1, 32), (128, 1)),
        "l1wp": np.asarray(inputs["lin1w"]).astype(ml_dtypes.bfloat16),
        "l1br": np.tile(np.asarray(inputs["lin1b"], np.float32).reshape(1, 32), (64, 1)),
        "l2wr": np.tile(np.asarray(inputs["lin2w"], np.float32).reshape(1, 32), (64, 1)),
        "sc4": np.tile(np.array([[float(np.ravel(inputs["g2b"])[0]),
                          float(np.ravel(inputs["lin2b"])[0]), 0.0, 0.0]],
                        np.float32), (128, 1)),
        "io64": np.tile(np.arange(64, dtype=np.float32).reshape(1, 64), (128, 1)),
        "ones132": np.ones((1, 32), np.float32),
        "id128": np.eye(128, dtype=np.float32),
    }
    in_maps = []
    for c in range(NCORES):
        m = dict(common)
        m["xTl"] = np.ascontiguousarray(xT_full[:, c * SHP:(c + 1) * SHP])
        m["bloc"] = blc_all[c]
        m["msk"] = msk_all[c]
        m["eidx"] = idx_all[c]
        m["cnts"] = cnt_all[c]
        in_maps.append(m)

    res = run_bass_kernel_spmd(nc, in_maps, list(range(NCORES)))
    return res.results[0]["y"].reshape(G).astype(np.float32)



# revision 44
# speedup vs baseline: 1.2076x; 1.0152x over previous
"""GATv2 regressor on 8 Trainium2 NeuronCores (Bass).

Sharding: core c owns dst nodes [c*12500, (c+1)*12500), relabeled locally in
decreasing in-degree order so fixed-slot padding per 128-dst block is tight.
All per-edge indexing is host-precomputed; the device does dense DMA,
dma_gather by src table row, broadcast adds, free-dim reductions and small
matmuls. Softmax skips max-subtraction (scores are clamped before exp);
denominators are reduced from the masked exp tile and divided at the end.

Both layers' node tables are bf16 with 256B rows so each per-edge gather
descriptor moves one full row. Per-core gather lists end in a -1 suffix with
the exact valid count supplied at runtime through a gpsimd register, so cores
with fewer edges in a (block, chunk) cell skip the cross-core padding rows.
Activations are batched per 128-dst block (one Lrelu span, one Exp span) to
avoid per-chunk activation-table reloads.
"""
import numpy as np
import ml_dtypes

import concourse.bacc as bacc
import concourse.mybir as mybir
import concourse.tile as tile
from concourse.bass_utils import run_bass_kernel_spmd
from concourse.library_config import mlp as mlp_lib

F32 = mybir.dt.float32
BF16 = mybir.dt.bfloat16
I16 = mybir.dt.int16

N, E, IN, C, H, G = 100000, 1600000, 128, 32, 2, 64
NEG = 0.2
NCORES = 8
SH = 12500
SHP = 12544              # 98*128
NBLK = SHP // 128        # 98
NTAB = SHP * NCORES      # 100352
NCHUNK = 4
WWID = 32768             # gather window width (int16 index reach)
WBASE = np.array([0, 22528, 45056, 67584])   # overlapping window bases
_WB = np.array([22528, 32768, 45056, 55296, 67584, 77824])  # region bounds

_CACHE = {}


def _wrap_idx(idx):
    n = idx.shape[0]
    w = idx.reshape(n // 16, 16).T
    return np.tile(w, (8, 1)).astype(np.int16)


def _assign_windows(sr, rk):
    """2-choice balance: edges in window overlaps go to the less-loaded
    window of their dst, flattening per-(dst, window) counts."""
    reg = np.searchsorted(_WB, sr, side="right")
    C7 = np.zeros((SH, 7), np.int64)
    np.add.at(C7, (rk, reg), 1)
    e = C7[:, [0, 2, 4, 6]].astype(np.float64)
    o = C7[:, [1, 3, 5]].astype(np.float64)
    a = o / 2
    for _ in range(8):
        l0 = e[:, 0] + a[:, 0]
        l1 = e[:, 1] + (o[:, 0] - a[:, 0]) + a[:, 1]
        l2 = e[:, 2] + (o[:, 1] - a[:, 1]) + a[:, 2]
        l3 = e[:, 3] + (o[:, 2] - a[:, 2])
        a[:, 0] = np.clip(a[:, 0] + (l1 - l0) / 2, 0, o[:, 0])
        a[:, 1] = np.clip(a[:, 1] + (l2 - l1) / 2, 0, o[:, 1])
        a[:, 2] = np.clip(a[:, 2] + (l3 - l2) / 2, 0, o[:, 2])
    A = np.minimum(np.rint(a).astype(np.int64), C7[:, [1, 3, 5]])
    key = rk * 7 + reg
    order = np.argsort(key, kind="stable")
    first = np.zeros(SH * 7 + 1, np.int64)
    np.cumsum(np.bincount(key, minlength=SH * 7), out=first[1:])
    rig = np.empty(sr.size, np.int64)
    rig[order] = np.arange(sr.size) - first[key[order]]
    w = np.empty(sr.size, np.int64)
    excl = (reg % 2 == 0)
    w[excl] = reg[excl] // 2
    ov = ~excl
    ovi = (reg[ov] - 1) // 2
    left = rig[ov] < A[rk[ov], ovi]
    w[ov] = np.where(left, ovi, ovi + 1)
    return w


def host_prep(edge_index, batch):
    src = edge_index[0].astype(np.int64)
    dst = edge_index[1].astype(np.int64)
    core = dst // SH
    dloc = dst % SH

    perms, ranks = [], []
    for c in range(NCORES):
        deg = np.bincount(dloc[core == c], minlength=SH)
        p = np.argsort(-deg, kind="stable")
        r = np.empty(SH, dtype=np.int64)
        r[p] = np.arange(SH)
        perms.append(p)
        ranks.append(r)

    ncore = np.arange(N) // SH
    nloc = np.arange(N) % SH
    trow = np.empty(N, dtype=np.int64)
    for c in range(NCORES):
        m = ncore == c
        trow[m] = c * SHP + ranks[c][nloc[m]]

    erow = np.empty(E, dtype=np.int64)
    for c in range(NCORES):
        m = core == c
        erow[m] = ranks[c][dloc[m]]
    srow = trow[src]
    wofe = np.empty(E, dtype=np.int64)
    for c in range(NCORES):
        m = core == c
        wofe[m] = _assign_windows(srow[m], erow[m])

    S = np.ones((NBLK, NCHUNK), dtype=np.int64)
    for c in range(NCORES):
        m = core == c
        cnt = np.bincount((erow[m] * NCHUNK + wofe[m]).astype(np.int64),
                          minlength=SH * NCHUNK).reshape(SH, NCHUNK)
        full = np.zeros((SHP, NCHUNK), dtype=np.int64)
        full[:SH] = cnt
        S = np.maximum(S, full.reshape(NBLK, 128, NCHUNK).max(axis=1))
    Stot = int(S.sum())

    idx_all, msk_all, blc_all, cnt_all = [], [], [], []
    for c in range(NCORES):
        m = np.nonzero(core == c)[0]
        key = erow[m] * NCHUNK + wofe[m]
        order = np.argsort(key, kind="stable")
        ms = m[order]
        rk, ck = erow[ms], wofe[ms]
        gid = (rk * NCHUNK + ck).astype(np.int64)
        first = np.zeros(SH * NCHUNK + 1, dtype=np.int64)
        np.cumsum(np.bincount(gid, minlength=SH * NCHUNK), out=first[1:])
        slot = np.arange(ms.size) - first[gid]

        iarr = np.zeros((Stot * 128,), dtype=np.int16)
        marr = np.zeros((128, Stot), dtype=np.float32)
        carr = np.zeros((NBLK * NCHUNK,), dtype=np.int32)
        off = 0
        bb = rk // 128
        pp = rk % 128
        for b in range(NBLK):
            selb = bb == b
            for k in range(NCHUNK):
                s = int(S[b, k])
                sel = selb & (ck == k)
                p = pp[sel]
                sl = slot[sel]
                # per-core used slot count for this (block, chunk)
                u = int(sl.max()) + 1 if sl.size else 0
                seg = np.zeros(s * 128, dtype=np.int16)
                seg[sl * 128 + p] = (srow[ms[sel]] - WBASE[k]).astype(np.int16)
                seg[u * 128:] = -1            # trailing slots: skipped by DGE
                carr[b * NCHUNK + k] = u * 128
                iarr[off * 128:(off + s) * 128] = seg
                marr[p, off + sl] = 1.0
                off += s
        idx_all.append(_wrap_idx(iarr))
        msk_all.append(marr)
        cnt_all.append(np.tile(carr.reshape(1, -1), (1, 1)))
        bl = np.full((128, NBLK), 127.0, dtype=np.float32)
        for b in range(NBLK):
            lo = b * 128
            take = min(128, SH - lo)
            bl[:take, b] = batch[c * SH + perms[c][lo:lo + take]]
        blc_all.append(bl)

    return S, Stot, trow, perms, idx_all, msk_all, blc_all, cnt_all


def build_kernel(S):
    Stot = int(S.sum())
    nc = bacc.Bacc("TRN2", target_bir_lowering=False, num_swdge_queues=4,
                   dynamic_dma_scratch_size=40960)

    def dp(name, shape, dt=F32):
        return nc.declare_dram_parameter(name, shape, dt, isOutput=False)

    xT = dp("xT", [IN, NTAB], BF16)
    xTl = dp("xTl", [IN, SHP], BF16)
    w1 = dp("w1", [IN, 128], BF16)           # [Wl1 | Wr1]
    w2 = dp("w2", [64, 64], BF16)            # [Wl2 | Wr2]
    cnts = dp("cnts", [1, NBLK * NCHUNK], mybir.dt.int32)
    att1r = dp("att1r", [128, 64])
    att2r = dp("att2r", [128, 32])
    b1r = dp("b1r", [128, 64])
    b2r = dp("b2r", [128, 32])
    g1wp = dp("g1wp", [32, 32], BF16)
    g1br = dp("g1br", [128, 32])
    g2wr = dp("g2wr", [128, 32])
    l1wp = dp("l1wp", [32, 32], BF16)
    l1br = dp("l1br", [64, 32])
    l2wr = dp("l2wr", [64, 32])
    sc4 = dp("sc4", [128, 4])                  # [g2b, lin2b, 0, 0]
    io64 = dp("io64", [128, 64])
    ones132 = dp("ones132", [1, 32])
    id128 = dp("id128", [128, 128])
    bloc = dp("bloc", [128, NBLK])
    msk = dp("msk", [128, Stot])
    eidx = dp("eidx", [128, (Stot * 128) // 16], I16)

    out_y = nc.declare_dram_parameter("y", [64, 1], F32, isOutput=True)

    add = mybir.AluOpType.add
    mult = mybir.AluOpType.mult
    iseq = mybir.AluOpType.is_equal
    byp = mybir.AluOpType.bypass
    AX = mybir.AxisListType.X
    EXP = mybir.ActivationFunctionType.Exp
    RELU = mybir.ActivationFunctionType.Relu
    LRELU = mybir.ActivationFunctionType.Lrelu

    with tile.TileContext(nc) as tc:
        with (
            tc.tile_pool(name="const", bufs=1) as cp,
            tc.tile_pool(name="gat", bufs=4) as gpool,
            tc.tile_pool(name="wk", bufs=2) as wk,
            tc.tile_pool(name="vtp", bufs=3) as vtp,
            tc.tile_pool(name="ps", bufs=2, space="PSUM") as ps,
            tc.tile_pool(name="psacc", bufs=1, space="PSUM") as psacc,
            tc.tile_pool(name="big", bufs=1) as bigp,
            tc.tile_pool(name="dram", bufs=1, space="DRAM") as dram,
        ):
            tab1 = dram.tile([NTAB, 128], BF16)
            tab2 = dram.tile([NTAB, 128], BF16)
            hT_loc = dram.tile([64, SHP], BF16)
            hT_all = dram.tile([NCORES * 64, SHP], BF16)
            pool_in = dram.tile([48, 64], F32)
            pool_all = dram.tile([48, 64], F32)
            nc.gpsimd.load_library(mlp_lib)

            def lc(t, shape, dt=F32):
                tt = cp.tile(shape, dt, tag=t.name + "_t")
                nc.sync.dma_start(tt[:], t[:])
                return tt

            w1_t = lc(w1, [IN, 128], BF16)
            w2_t = lc(w2, [64, 64], BF16)
            cnts_t = lc(cnts, [1, NBLK * NCHUNK], mybir.dt.int32)
            att1_t = lc(att1r, [128, 64])
            att1b_t = cp.tile([128, 64], BF16, tag="att1b")
            nc.vector.tensor_copy(att1b_t[:], att1_t[:])
            att2_t = lc(att2r, [128, 32])
            att2b_t = cp.tile([128, 32], BF16, tag="att2b")
            nc.vector.tensor_copy(att2b_t[:], att2_t[:])
            b1_t = lc(b1r, [128, 64])
            b2_t = lc(b2r, [128, 32])
            g1w_t = lc(g1wp, [32, 32], BF16)
            g1b_t = lc(g1br, [128, 32])
            g2w_t = lc(g2wr, [128, 32])
            l1w_t = lc(l1wp, [32, 32], BF16)
            l1b_t = lc(l1br, [64, 32])
            l2w_t = lc(l2wr, [64, 32])
            sc4_t = lc(sc4, [128, 4])
            io64_t = lc(io64, [128, 64])
            on132_t = lc(ones132, [1, 32])
            id_t = lc(id128, [128, 128])
            idb_t = cp.tile([128, 128], BF16, tag="idb")
            nc.vector.tensor_copy(idb_t[:], id_t[:])
            bloc_t = lc(bloc, [128, NBLK])
            msk_t = bigp.tile([128, Stot], F32)
            nc.sync.dma_start(msk_t[:], msk[:])

            xr1_t = bigp.tile([128, NBLK * 64], BF16)
            xr2_t = bigp.tile([128, NBLK * 32], BF16)
            hTl_t = bigp.tile([64, SHP], BF16)
            h2all = bigp.tile([128, NBLK * 32], F32)
            gcnt_reg = nc.gpsimd.alloc_register("gcnt")
            for _gz in range(6):
                gz = gpool.tile([128, 16, 128], BF16, tag="g", bufs=6,
                                name=f"gz{_gz}")
                nc.vector.memset(gz[:], 0.0)

            # ---------------- L1 global table + local xr1 ----------------
            for n0 in range(0, NTAB, 512):
                pt = ps.tile([128, 512], F32, tag="mm")
                xin = wk.tile([IN, 512], BF16, tag="xin")
                nc.sync.dma_start(xin[:], xT[:, n0:n0 + 512])
                for j in range(4):
                    nc.tensor.matmul(pt[:, j * 128:(j + 1) * 128],
                                     xin[:, j * 128:(j + 1) * 128], w1_t[:],
                                     start=True, stop=True)
                st = wk.tile([128, 512], BF16, tag="tsb")
                nc.scalar.copy(st[:], pt[:])
                nc.sync.dma_start(
                    tab1[n0:n0 + 512, :].rearrange("(j p) c -> p j c", p=128),
                    st[:].rearrange("p (j c) -> p j c", j=4))
            for n0 in range(0, SHP, 512):
                w_ = min(512, SHP - n0)
                pt = ps.tile([128, 512], F32, tag="mm")
                xin = wk.tile([IN, 512], BF16, tag="xin")
                nc.sync.dma_start(xin[:, :w_], xTl[:, n0:n0 + w_])
                for j in range(w_ // 128):
                    nc.tensor.matmul(pt[:, j * 128:(j + 1) * 128],
                                     xin[:, j * 128:(j + 1) * 128], w1_t[:],
                                     start=True, stop=True)
                st = wk.tile([128, 512], BF16, tag="tsb")
                nc.scalar.copy(st[:, :w_], pt[:, :w_])
                for j in range(w_ // 128):
                    b = n0 // 128 + j
                    nc.vector.tensor_copy(
                        xr1_t[:, b * 64:(b + 1) * 64],
                        st[:, j * 128 + 64:j * 128 + 128])

            tc.strict_bb_all_engine_barrier()

            def edge_layer(tab, xr_t, att_t, bias_t, F, heads, hall, gw, gdt,
                           post_block=None):
                ioff = 0
                soff = 0
                for b in range(NBLK):
                    Ss = [int(S[b, k]) for k in range(NCHUNK)]
                    ST = sum(Ss)
                    acc4 = wk.tile([128, F, NCHUNK], F32, tag="acc4")
                    ixb = wk.tile([128, ST * 8], I16, tag="ix")
                    nc.sync.dma_start(ixb[:], eidx[:, ioff:ioff + ST * 8])
                    scob = wk.tile([128, ST * heads], F32, tag="scob")
                    wexpb = wk.tile([128, ST * heads], F32, tag="wexpb")
                    gs = []
                    iof2 = 0
                    co = 0
                    for k in range(NCHUNK):
                        s = Ss[k]
                        ni = s * 128
                        g = gpool.tile([128, s, gw], gdt, tag="g", bufs=6)
                        nc.gpsimd.reg_load(
                            gcnt_reg,
                            cnts_t[0:1, b * NCHUNK + k:b * NCHUNK + k + 1])
                        nc.gpsimd.dma_gather(
                            g[:], tab[int(WBASE[k]):int(WBASE[k]) + WWID, 0:gw],
                            ixb[:, iof2:iof2 + ni // 16],
                            ni, gcnt_reg, gw,
                            single_packet=False,
                            queue_num=(b * NCHUNK + k) % 4)
                        gs.append(g)
                        iof2 += ni // 16
                        ioff += ni // 16
                        xb = xr_t[:, b * F:(b + 1) * F]
                        sadd = wk.tile([128, s, F], gdt, tag="sadd")
                        nc.vector.tensor_tensor(
                            sadd[:], g[:, :, 0:F],
                            xb.unsqueeze(1).broadcast_to([128, s, F]), op=add)
                        nc.scalar.activation(sadd[:], sadd[:], LRELU,
                                             alpha=NEG)
                        nc.vector.tensor_tensor(
                            sadd[:], sadd[:],
                            att_t[:, 0:F].unsqueeze(1)
                            .broadcast_to([128, s, F]), op=mult)
                        nc.vector.tensor_reduce(
                            scob[:, co * heads:(co + s) * heads],
                            sadd[:].rearrange("p s (h c) -> p (s h) c", c=32),
                            axis=AX, op=add)
                        co += s
                    nc.vector.tensor_scalar_min(scob[:], scob[:], 30.0)
                    nc.scalar.activation(wexpb[:], scob[:], EXP)
                    wv = wexpb[:].rearrange("p (s h) -> p s h", h=heads)
                    nc.vector.tensor_tensor(
                        wv, wv,
                        msk_t[:, soff:soff + ST].unsqueeze(2)
                        .broadcast_to([128, ST, heads]), op=mult)
                    den = wk.tile([128, heads], F32, tag="den")
                    nc.vector.tensor_reduce(
                        den[:], wexpb[:].rearrange("p (s h) -> p h s", h=heads),
                        axis=AX, op=add)
                    co = 0
                    for k in range(NCHUNK):
                        s = Ss[k]
                        g = gs[k]
                        vt = vtp.tile([128, F, s], F32, tag="vt")
                        wvk = wv[:, co:co + s, :]
                        for h in range(heads):
                            nc.vector.tensor_tensor(
                                vt[:, h * 32:(h + 1) * 32, :]
                                .rearrange("p c s -> p s c"),
                                g[:, :, h * 32:h * 32 + 32],
                                wvk[:, :, h:h + 1].broadcast_to([128, s, 32]),
                                op=mult)
                        nc.vector.tensor_reduce(
                            acc4[:, :, k:k + 1].rearrange("p c o -> p (c o)"),
                            vt[:], axis=AX, op=add)
                        co += s
                    acc = wk.tile([128, F], F32, tag="acc")
                    nc.vector.tensor_reduce(acc[:], acc4[:], axis=AX, op=add)
                    nc.vector.tensor_scalar_max(den[:], den[:], 1e-30)
                    nc.vector.reciprocal(den[:], den[:])
                    hb = wk.tile([128, F], F32, tag="hb")
                    for h in range(heads):
                        nc.vector.tensor_tensor(
                            hb[:, h * 32:(h + 1) * 32],
                            acc[:, h * 32:(h + 1) * 32],
                            den[:, h:h + 1].broadcast_to([128, 32]), op=mult)
                    nc.vector.tensor_tensor(
                        hb[:], hb[:],
                        bias_t[:, 0:F], op=add)
                    nc.vector.tensor_scalar_max(
                        hall[:, b * F:(b + 1) * F], hb[:], 0.0)
                    if post_block is not None:
                        post_block(b)
                    soff += ST

            # ---------------- Layer 1 ----------------
            h1all = bigp.tile([128, NBLK * 64], BF16)

            def l1_post(b):
                pt = ps.tile([64, 128], BF16, tag="mm", name="ptT")
                nc.tensor.transpose(pt[:], h1all[:, b * 64:(b + 1) * 64],
                                    idb_t[:])
                nc.scalar.copy(hTl_t[:, b * 128:(b + 1) * 128], pt[:])

            edge_layer(tab1, xr1_t, att1b_t, b1_t, 64, H, h1all, 128, BF16,
                       post_block=l1_post)
            nc.sync.dma_start(hT_loc[:], hTl_t[:])

            tc.strict_bb_all_engine_barrier()
            nc.gpsimd.collective_compute(
                "AllGather", byp,
                replica_groups=[list(range(NCORES))],
                ins=[hT_loc.opt()], outs=[hT_all.opt()])
            tc.strict_bb_all_engine_barrier()

            # ---------------- L2 table + local xr2 ----------------
            for c in range(NCORES):
                for n0 in range(0, SHP, 512):
                    w_ = min(512, SHP - n0)
                    hinb = wk.tile([64, 512], BF16, tag="hinb")
                    nc.sync.dma_start(
                        hinb[:, :w_], hT_all[c * 64:(c + 1) * 64, n0:n0 + w_])
                    pt = ps.tile([128, 256], F32, tag="mm")
                    for j in range(w_ // 128):
                        nc.tensor.matmul(pt[:, j * 64:(j + 1) * 64],
                                         hinb[:, j * 128:(j + 1) * 128],
                                         w2_t[:], start=True, stop=True)
                    st = wk.tile([128, 256], BF16, tag="t2sb")
                    nc.scalar.copy(st[:], pt[:])
                    base = c * SHP + n0
                    nc.sync.dma_start(
                        tab2[base:base + w_, 0:64]
                        .rearrange("(j p) c -> p j c", p=128),
                        st[:, :w_ // 2].rearrange("p (j c) -> p j c", c=64))

            for n0 in range(0, SHP, 512):
                w_ = min(512, SHP - n0)
                hinb = wk.tile([64, 512], BF16, tag="hinb")
                nc.vector.tensor_copy(hinb[:, :w_], hTl_t[:, n0:n0 + w_])
                pt = ps.tile([128, 256], F32, tag="mm")
                for j in range(w_ // 128):
                    nc.tensor.matmul(pt[:, j * 64:(j + 1) * 64],
                                     hinb[:, j * 128:(j + 1) * 128], w2_t[:],
                                     start=True, stop=True)
                st = wk.tile([128, 256], BF16, tag="t2sb")
                nc.scalar.copy(st[:], pt[:])
                for j in range(w_ // 128):
                    b = n0 // 128 + j
                    nc.vector.tensor_copy(xr2_t[:, b * 32:(b + 1) * 32],
                                          st[:, j * 64 + 32:j * 64 + 64])

            tc.strict_bb_all_engine_barrier()

            # ---------------- Layer 2 + fused pooling ----------------
            pp = psacc.tile([34, 64], F32)

            def l2_post(b):
                h2 = h2all[:, b * 32:(b + 1) * 32]
                pt = ps.tile([32, 128], F32, tag="mm", name="ptP")
                nc.tensor.transpose(pt[:], h2, id_t[:])
                h2T = wk.tile([32, 128], BF16, tag="h2T")
                nc.vector.tensor_copy(h2T[:], pt[:])
                gp1 = ps.tile([128, 32], F32, tag="mm", name="gp1")
                nc.tensor.matmul(gp1[:], h2T[:], g1w_t[:],
                                 start=True, stop=True)
                ga = wk.tile([128, 32], F32, tag="ga")
                nc.vector.tensor_tensor(
                    ga[:], gp1[:],
                    g1b_t[:], op=add)
                nc.vector.tensor_scalar_max(ga[:], ga[:], 0.0)
                nc.vector.tensor_tensor(
                    ga[:], ga[:],
                    g2w_t[:], op=mult)
                gt = wk.tile([128, 1], F32, tag="gt")
                nc.vector.tensor_reduce(gt[:], ga[:], axis=AX, op=add)
                nc.vector.tensor_tensor(
                    gt[:], gt[:],
                    sc4_t[:, 0:1], op=add)
                ge = wk.tile([128, 1], F32, tag="ge")
                nc.scalar.activation(ge[:], gt[:], EXP)
                vg = wk.tile([128, 34], F32, tag="vg")
                nc.vector.tensor_tensor(
                    vg[:, 0:32], h2, ge[:].broadcast_to([128, 32]), op=mult)
                nc.vector.tensor_copy(vg[:, 32:33], ge[:])
                nc.vector.memset(vg[:, 33:34], 0.0)
                ohg = wk.tile([128, 64], F32, tag="ohg")
                nc.vector.tensor_scalar(
                    ohg[:], io64_t[:],
                    bloc_t[:, b:b + 1], None, op0=iseq)
                vgb = wk.tile([128, 34], BF16, tag="vgb")
                nc.vector.tensor_copy(vgb[:], vg[:])
                ohgb = wk.tile([128, 64], BF16, tag="ohgb")
                nc.vector.tensor_copy(ohgb[:], ohg[:])
                nc.tensor.matmul(pp[:], vgb[:], ohgb[:],
                                 start=(b == 0), stop=(b == NBLK - 1))

            edge_layer(tab2, xr2_t, att2b_t, b2_t, 32, 1, h2all, 128, BF16,
                       post_block=l2_post)

            pin = wk.tile([48, 64], F32, tag="pin")
            nc.vector.memset(pin[:], 0.0)
            nc.scalar.copy(pin[0:34, :], pp[:])
            nc.sync.dma_start(pool_in[:], pin[:])

            tc.strict_bb_all_engine_barrier()
            nc.gpsimd.collective_compute(
                "AllReduce", add,
                replica_groups=[list(range(NCORES))],
                ins=[pool_in.opt()], outs=[pool_all.opt()])
            tc.strict_bb_all_engine_barrier()

            pall = wk.tile([48, 64], F32, tag="pall")
            nc.sync.dma_start(pall[:], pool_all[:])
            dn = wk.tile([1, 64], F32, tag="dn")
            nc.vector.reciprocal(dn[:], pall[32:33, :])
            dnr = ps.tile([32, 64], F32, tag="mm")
            nc.tensor.matmul(dnr[:], on132_t[:], dn[:],
                             start=True, stop=True)
            pooledT = wk.tile([32, 64], BF16, tag="pooledT")
            nc.vector.tensor_tensor(
                pooledT[:], pall[0:32, :], dnr[:], op=mult)
            zp = ps.tile([64, 32], F32, tag="mm")
            nc.tensor.matmul(zp[:], pooledT[:], l1w_t[:],
                             start=True, stop=True)
            z = wk.tile([64, 32], F32, tag="z")
            nc.vector.tensor_tensor(
                z[:], zp[:], l1b_t[:], op=add)
            nc.vector.tensor_scalar_max(z[:], z[:], 0.0)
            nc.vector.tensor_tensor(
                z[:], z[:], l2w_t[:], op=mult)
            yv = wk.tile([64, 1], F32, tag="yv")
            nc.vector.tensor_reduce(yv[:], z[:], axis=AX, op=add)
            nc.vector.tensor_tensor(
                yv[:], yv[:], sc4_t[0:64, 1:2], op=add)
            nc.sync.dma_start(out_y[:], yv[:])

    nc.compile()
    return nc


def kernel(**inputs):
    x = np.asarray(inputs["x"], dtype=np.float32)
    edge_index = np.asarray(inputs["edge_index"])
    batch = np.asarray(inputs["batch"])
    key = (int(edge_index[:, ::4099].sum()), int(batch[::997].sum()))
    if key not in _CACHE:
        prep = host_prep(edge_index, batch)
        nc = build_kernel(prep[0])
        _CACHE.clear()
        _CACHE[key] = (prep, nc)
    (S, Stot, trow, perms, idx_all, msk_all, blc_all, cnt_all), nc = _CACHE[key]

    xp = np.zeros((NTAB, IN), dtype=np.float32)
    xp[trow] = x
    xT_full = np.ascontiguousarray(xp.T).astype(ml_dtypes.bfloat16)

    w1c = np.concatenate([inputs["Wl1"], inputs["Wr1"]], 1).astype(ml_dtypes.bfloat16)
    w2c = np.concatenate([inputs["Wl2"], inputs["Wr2"]], 1).astype(ml_dtypes.bfloat16)
    common = {
        "xT": xT_full, "w1": w1c, "w2": w2c,
        "att1r": np.tile(np.asarray(inputs["att1"], np.float32).reshape(1, 64), (128, 1)),
        "att2r": np.tile(np.asarray(inputs["att2"], np.float32).reshape(1, 32), (128, 1)),
        "b1r": np.tile(np.asarray(inputs["b1"], np.float32).reshape(1, 64), (128, 1)),
        "b2r": np.tile(np.asarray(inputs["b2"], np.float32).reshape(1, 32), (128, 1)),
        "g1wp": np.asarray(inputs["g1w"]).astype(ml_dtypes.bfloat16),
        "g1br": np.tile(np.asarray(inputs["g1b"], np.float32).reshape(1, 32), (128, 1)),
        "g2wr": np.tile(np.asarray(inputs["g2w"], np.float32).reshape(1, 32), (128, 1)),
        "l1wp": np.asarray(inputs["lin1w"]).astype(ml_dtypes.bfloat16),
        "l1br": np.tile(np.asarray(inputs["lin1b"], np.float32).reshape(1, 32), (64, 1)),
        "l2wr": np.tile(np.asarray(inputs["lin2w"], np.float32).reshape(1, 32), (64, 1)),
        "sc4": np.tile(np.array([[float(np.ravel(inputs["g2b"])[0]),
                          float(np.ravel(inputs["lin2b"])[0]), 0.0, 0.0]],
                        np.float32), (128, 1)),
        "io64": np.tile(np.arange(64, dtype=np.float32).reshape(1, 64), (128, 1)),
        "ones132": np.ones((1, 32), np.float32),
        "id128": np.eye(128, dtype=np.float32),
    }
    in_maps = []
    for c in range(NCORES):
        m = dict(common)
        m["xTl"] = np.ascontiguousarray(xT_full[:, c * SHP:(c + 1) * SHP])
        m["bloc"] = blc_all[c]
        m["msk"] = msk_all[c]
        m["eidx"] = idx_all[c]
        m["cnts"] = cnt_all[c]
        in_maps.append(m)

    res = run_bass_kernel_spmd(nc, in_maps, list(range(NCORES)))
    return res.results[0]["y"].reshape(G).astype(np.float32)



# revision 47
# speedup vs baseline: 1.2203x; 1.0105x over previous
"""GATv2 regressor on 8 Trainium2 NeuronCores (Bass).

Sharding: core c owns dst nodes [c*12500, (c+1)*12500), relabeled locally in
decreasing in-degree order so fixed-slot padding per 128-dst block is tight.
All per-edge indexing is host-precomputed; the device does dense DMA,
dma_gather by src table row, broadcast adds, free-dim reductions and small
matmuls. Softmax skips max-subtraction (scores are clamped before exp);
denominators are reduced from the masked exp tile and divided at the end.

Both layers' node tables are bf16 with 256B rows so each per-edge gather
descriptor moves one full row. Per-core gather lists end in a -1 suffix with
the exact valid count supplied at runtime through a gpsimd register, so cores
with fewer edges in a (block, chunk) cell skip the cross-core padding rows.
Activations are batched per 128-dst block (one Lrelu span, one Exp span) to
avoid per-chunk activation-table reloads.
"""
import numpy as np
import ml_dtypes

import concourse.bacc as bacc
import concourse.mybir as mybir
import concourse.tile as tile
from concourse.bass_utils import run_bass_kernel_spmd
from concourse.library_config import mlp as mlp_lib

F32 = mybir.dt.float32
BF16 = mybir.dt.bfloat16
I16 = mybir.dt.int16

N, E, IN, C, H, G = 100000, 1600000, 128, 32, 2, 64
NEG = 0.2
NCORES = 8
SH = 12500
SHP = 12544              # 98*128
NBLK = SHP // 128        # 98
NTAB = SHP * NCORES      # 100352
NCHUNK = 4
WWID = 32768             # gather window width (int16 index reach)
WBASE = np.array([0, 22528, 45056, 67584])   # overlapping window bases
_WB = np.array([22528, 32768, 45056, 55296, 67584, 77824])  # region bounds

_CACHE = {}


def _wrap_idx(idx):
    n = idx.shape[0]
    w = idx.reshape(n // 16, 16).T
    return np.tile(w, (8, 1)).astype(np.int16)


def _assign_windows(sr, rk):
    """2-choice balance: edges in window overlaps go to the less-loaded
    window of their dst, flattening per-(dst, window) counts."""
    reg = np.searchsorted(_WB, sr, side="right")
    C7 = np.zeros((SH, 7), np.int64)
    np.add.at(C7, (rk, reg), 1)
    e = C7[:, [0, 2, 4, 6]].astype(np.float64)
    o = C7[:, [1, 3, 5]].astype(np.float64)
    a = o / 2
    for _ in range(8):
        l0 = e[:, 0] + a[:, 0]
        l1 = e[:, 1] + (o[:, 0] - a[:, 0]) + a[:, 1]
        l2 = e[:, 2] + (o[:, 1] - a[:, 1]) + a[:, 2]
        l3 = e[:, 3] + (o[:, 2] - a[:, 2])
        a[:, 0] = np.clip(a[:, 0] + (l1 - l0) / 2, 0, o[:, 0])
        a[:, 1] = np.clip(a[:, 1] + (l2 - l1) / 2, 0, o[:, 1])
        a[:, 2] = np.clip(a[:, 2] + (l3 - l2) / 2, 0, o[:, 2])
    A = np.minimum(np.rint(a).astype(np.int64), C7[:, [1, 3, 5]])
    key = rk * 7 + reg
    order = np.argsort(key, kind="stable")
    first = np.zeros(SH * 7 + 1, np.int64)
    np.cumsum(np.bincount(key, minlength=SH * 7), out=first[1:])
    rig = np.empty(sr.size, np.int64)
    rig[order] = np.arange(sr.size) - first[key[order]]
    w = np.empty(sr.size, np.int64)
    excl = (reg % 2 == 0)
    w[excl] = reg[excl] // 2
    ov = ~excl
    ovi = (reg[ov] - 1) // 2
    left = rig[ov] < A[rk[ov], ovi]
    w[ov] = np.where(left, ovi, ovi + 1)
    return w


def host_prep(edge_index, batch):
    src = edge_index[0].astype(np.int64)
    dst = edge_index[1].astype(np.int64)
    core = dst // SH
    dloc = dst % SH

    perms, ranks = [], []
    for c in range(NCORES):
        deg = np.bincount(dloc[core == c], minlength=SH)
        p = np.argsort(-deg, kind="stable")
        r = np.empty(SH, dtype=np.int64)
        r[p] = np.arange(SH)
        perms.append(p)
        ranks.append(r)

    ncore = np.arange(N) // SH
    nloc = np.arange(N) % SH
    trow = np.empty(N, dtype=np.int64)
    for c in range(NCORES):
        m = ncore == c
        trow[m] = c * SHP + ranks[c][nloc[m]]

    erow = np.empty(E, dtype=np.int64)
    for c in range(NCORES):
        m = core == c
        erow[m] = ranks[c][dloc[m]]
    srow = trow[src]
    wofe = np.empty(E, dtype=np.int64)
    for c in range(NCORES):
        m = core == c
        wofe[m] = _assign_windows(srow[m], erow[m])

    S = np.ones((NBLK, NCHUNK), dtype=np.int64)
    for c in range(NCORES):
        m = core == c
        cnt = np.bincount((erow[m] * NCHUNK + wofe[m]).astype(np.int64),
                          minlength=SH * NCHUNK).reshape(SH, NCHUNK)
        full = np.zeros((SHP, NCHUNK), dtype=np.int64)
        full[:SH] = cnt
        S = np.maximum(S, full.reshape(NBLK, 128, NCHUNK).max(axis=1))
    Stot = int(S.sum())

    idx_all, msk_all, blc_all, cnt_all = [], [], [], []
    for c in range(NCORES):
        m = np.nonzero(core == c)[0]
        key = erow[m] * NCHUNK + wofe[m]
        order = np.argsort(key, kind="stable")
        ms = m[order]
        rk, ck = erow[ms], wofe[ms]
        gid = (rk * NCHUNK + ck).astype(np.int64)
        first = np.zeros(SH * NCHUNK + 1, dtype=np.int64)
        np.cumsum(np.bincount(gid, minlength=SH * NCHUNK), out=first[1:])
        slot = np.arange(ms.size) - first[gid]

        iarr = np.zeros((Stot * 128,), dtype=np.int16)
        marr = np.zeros((128, Stot), dtype=np.float32)
        carr = np.zeros((NBLK * NCHUNK,), dtype=np.int32)
        off = 0
        bb = rk // 128
        pp = rk % 128
        for b in range(NBLK):
            selb = bb == b
            for k in range(NCHUNK):
                s = int(S[b, k])
                sel = selb & (ck == k)
                p = pp[sel]
                sl = slot[sel]
                # per-core used slot count for this (block, chunk)
                u = int(sl.max()) + 1 if sl.size else 0
                seg = np.zeros(s * 128, dtype=np.int16)
                seg[sl * 128 + p] = (srow[ms[sel]] - WBASE[k]).astype(np.int16)
                seg[u * 128:] = -1            # trailing slots: skipped by DGE
                carr[b * NCHUNK + k] = u * 128
                iarr[off * 128:(off + s) * 128] = seg
                marr[p, off + sl] = 1.0
                off += s
        idx_all.append(_wrap_idx(iarr))
        msk_all.append(marr)
        cnt_all.append(np.tile(carr.reshape(1, -1), (1, 1)))
        bl = np.full((128, NBLK), 127.0, dtype=np.float32)
        for b in range(NBLK):
            lo = b * 128
            take = min(128, SH - lo)
            bl[:take, b] = batch[c * SH + perms[c][lo:lo + take]]
        blc_all.append(bl)

    return S, Stot, trow, perms, idx_all, msk_all, blc_all, cnt_all


def build_kernel(S):
    Stot = int(S.sum())
    nc = bacc.Bacc("TRN2", target_bir_lowering=False, num_swdge_queues=4,
                   dynamic_dma_scratch_size=40960)

    def dp(name, shape, dt=F32):
        return nc.declare_dram_parameter(name, shape, dt, isOutput=False)

    xT = dp("xT", [IN, NTAB], BF16)
    xTl = dp("xTl", [IN, SHP], BF16)
    w1 = dp("w1", [IN, 128], BF16)           # [Wl1 | Wr1]
    w2 = dp("w2", [64, 64], BF16)            # [Wl2 | Wr2]
    cnts = dp("cnts", [1, NBLK * NCHUNK], mybir.dt.int32)
    att1r = dp("att1r", [128, 64])
    att2r = dp("att2r", [128, 32])
    b1r = dp("b1r", [128, 64])
    b2r = dp("b2r", [128, 32])
    g1wp = dp("g1wp", [32, 32], BF16)
    g1br = dp("g1br", [128, 32])
    g2wr = dp("g2wr", [128, 32])
    l1wp = dp("l1wp", [32, 32], BF16)
    l1br = dp("l1br", [64, 32])
    l2wr = dp("l2wr", [64, 32])
    sc4 = dp("sc4", [128, 4])                  # [g2b, lin2b, 0, 0]
    io64 = dp("io64", [128, 64])
    ones132 = dp("ones132", [1, 32])
    id128 = dp("id128", [128, 128])
    bloc = dp("bloc", [128, NBLK])
    msk = dp("msk", [128, Stot])
    eidx = dp("eidx", [128, (Stot * 128) // 16], I16)

    out_y = nc.declare_dram_parameter("y", [64, 1], F32, isOutput=True)

    add = mybir.AluOpType.add
    mult = mybir.AluOpType.mult
    iseq = mybir.AluOpType.is_equal
    byp = mybir.AluOpType.bypass
    AX = mybir.AxisListType.X
    EXP = mybir.ActivationFunctionType.Exp
    RELU = mybir.ActivationFunctionType.Relu
    LRELU = mybir.ActivationFunctionType.Lrelu

    with tile.TileContext(nc) as tc:
        with (
            tc.tile_pool(name="const", bufs=1) as cp,
            tc.tile_pool(name="gat", bufs=4) as gpool,
            tc.tile_pool(name="wk", bufs=2) as wk,
            tc.tile_pool(name="vtp", bufs=3) as vtp,
            tc.tile_pool(name="ps", bufs=2, space="PSUM") as ps,
            tc.tile_pool(name="psacc", bufs=1, space="PSUM") as psacc,
            tc.tile_pool(name="big", bufs=1) as bigp,
            tc.tile_pool(name="dram", bufs=1, space="DRAM") as dram,
        ):
            tab1 = dram.tile([NTAB, 128], BF16)
            tab2 = dram.tile([NTAB, 128], BF16)
            hT_loc = dram.tile([64, SHP], BF16)
            hT_all = dram.tile([NCORES * 64, SHP], BF16)
            pool_in = dram.tile([48, 64], F32)
            pool_all = dram.tile([48, 64], F32)
            nc.gpsimd.load_library(mlp_lib)

            def lc(t, shape, dt=F32):
                tt = cp.tile(shape, dt, tag=t.name + "_t")
                nc.sync.dma_start(tt[:], t[:])
                return tt

            w1_t = lc(w1, [IN, 128], BF16)
            w2_t = lc(w2, [64, 64], BF16)
            cnts_t = lc(cnts, [1, NBLK * NCHUNK], mybir.dt.int32)
            att1_t = lc(att1r, [128, 64])
            att1b_t = cp.tile([128, 64], BF16, tag="att1b")
            nc.vector.tensor_copy(att1b_t[:], att1_t[:])
            att2_t = lc(att2r, [128, 32])
            att2b_t = cp.tile([128, 32], BF16, tag="att2b")
            nc.vector.tensor_copy(att2b_t[:], att2_t[:])
            b1_t = lc(b1r, [128, 64])
            b2_t = lc(b2r, [128, 32])
            g1w_t = lc(g1wp, [32, 32], BF16)
            g1b_t = lc(g1br, [128, 32])
            g2w_t = lc(g2wr, [128, 32])
            l1w_t = lc(l1wp, [32, 32], BF16)
            l1b_t = lc(l1br, [64, 32])
            l2w_t = lc(l2wr, [64, 32])
            sc4_t = lc(sc4, [128, 4])
            io64_t = lc(io64, [128, 64])
            on132_t = lc(ones132, [1, 32])
            id_t = lc(id128, [128, 128])
            idb_t = cp.tile([128, 128], BF16, tag="idb")
            nc.vector.tensor_copy(idb_t[:], id_t[:])
            bloc_t = lc(bloc, [128, NBLK])
            msk_t = bigp.tile([128, Stot], F32)
            nc.sync.dma_start(msk_t[:], msk[:])

            xr1_t = bigp.tile([128, NBLK * 64], BF16)
            xr2_t = bigp.tile([128, NBLK * 32], BF16)
            hTl_t = bigp.tile([64, SHP], BF16)
            h2all = bigp.tile([128, NBLK * 32], F32)
            gcnt_regs = [nc.gpsimd.alloc_register(f"gcnt{i}")
                         for i in range(NCHUNK)]
            for _gz in range(6):
                gz = gpool.tile([128, 16, 128], BF16, tag="g", bufs=6,
                                name=f"gz{_gz}")
                nc.vector.memset(gz[:], 0.0)

            # ---------------- L1 global table + local xr1 ----------------
            for n0 in range(0, NTAB, 512):
                pt = ps.tile([128, 512], F32, tag="mm")
                xin = wk.tile([IN, 512], BF16, tag="xin")
                nc.sync.dma_start(xin[:], xT[:, n0:n0 + 512])
                for j in range(4):
                    nc.tensor.matmul(pt[:, j * 128:(j + 1) * 128],
                                     xin[:, j * 128:(j + 1) * 128], w1_t[:],
                                     start=True, stop=True)
                st = wk.tile([128, 512], BF16, tag="tsb")
                nc.scalar.copy(st[:], pt[:])
                nc.sync.dma_start(
                    tab1[n0:n0 + 512, :].rearrange("(j p) c -> p j c", p=128),
                    st[:].rearrange("p (j c) -> p j c", j=4))
            for n0 in range(0, SHP, 512):
                w_ = min(512, SHP - n0)
                pt = ps.tile([128, 512], F32, tag="mm")
                xin = wk.tile([IN, 512], BF16, tag="xin")
                nc.sync.dma_start(xin[:, :w_], xTl[:, n0:n0 + w_])
                for j in range(w_ // 128):
                    nc.tensor.matmul(pt[:, j * 128:(j + 1) * 128],
                                     xin[:, j * 128:(j + 1) * 128], w1_t[:],
                                     start=True, stop=True)
                st = wk.tile([128, 512], BF16, tag="tsb")
                nc.scalar.copy(st[:, :w_], pt[:, :w_])
                for j in range(w_ // 128):
                    b = n0 // 128 + j
                    nc.vector.tensor_copy(
                        xr1_t[:, b * 64:(b + 1) * 64],
                        st[:, j * 128 + 64:j * 128 + 128])

            tc.strict_bb_all_engine_barrier()

            def edge_layer(tab, xr_t, att_t, bias_t, F, heads, hall, gw, gdt,
                           post_block=None):
                ioff = 0
                soff = 0
                for b in range(NBLK):
                    Ss = [int(S[b, k]) for k in range(NCHUNK)]
                    ST = sum(Ss)
                    acc4 = wk.tile([128, F, NCHUNK], F32, tag="acc4")
                    ixb = wk.tile([128, ST * 8], I16, tag="ix")
                    nc.sync.dma_start(ixb[:], eidx[:, ioff:ioff + ST * 8])
                    scob = wk.tile([128, ST * heads], F32, tag="scob")
                    wexpb = wk.tile([128, ST * heads], F32, tag="wexpb")
                    nc.gpsimd.reg_load(
                        gcnt_regs,
                        cnts_t[0:1, b * NCHUNK:(b + 1) * NCHUNK])
                    gs = []
                    iof2 = 0
                    co = 0
                    for k in range(NCHUNK):
                        s = Ss[k]
                        ni = s * 128
                        g = gpool.tile([128, s, gw], gdt, tag="g", bufs=6)
                        nc.gpsimd.dma_gather(
                            g[:], tab[int(WBASE[k]):int(WBASE[k]) + WWID, 0:gw],
                            ixb[:, iof2:iof2 + ni // 16],
                            ni, gcnt_regs[k], gw,
                            single_packet=False,
                            queue_num=(b * NCHUNK + k) % 4)
                        gs.append(g)
                        iof2 += ni // 16
                        ioff += ni // 16
                        xb = xr_t[:, b * F:(b + 1) * F]
                        sadd = wk.tile([128, s, F], gdt, tag="sadd")
                        nc.vector.tensor_tensor(
                            sadd[:], g[:, :, 0:F],
                            xb.unsqueeze(1).broadcast_to([128, s, F]), op=add)
                        nc.scalar.activation(sadd[:], sadd[:], LRELU,
                                             alpha=NEG)
                        nc.vector.tensor_tensor(
                            sadd[:], sadd[:],
                            att_t[:, 0:F].unsqueeze(1)
                            .broadcast_to([128, s, F]), op=mult)
                        nc.vector.tensor_reduce(
                            scob[:, co * heads:(co + s) * heads],
                            sadd[:].rearrange("p s (h c) -> p (s h) c", c=32),
                            axis=AX, op=add)
                        co += s
                    nc.scalar.activation(wexpb[:], scob[:], EXP)
                    wv = wexpb[:].rearrange("p (s h) -> p s h", h=heads)
                    nc.vector.tensor_tensor(
                        wv, wv,
                        msk_t[:, soff:soff + ST].unsqueeze(2)
                        .broadcast_to([128, ST, heads]), op=mult)
                    den = wk.tile([128, heads], F32, tag="den")
                    nc.vector.tensor_reduce(
                        den[:], wexpb[:].rearrange("p (s h) -> p h s", h=heads),
                        axis=AX, op=add)
                    co = 0
                    for k in range(NCHUNK):
                        s = Ss[k]
                        g = gs[k]
                        vt = vtp.tile([128, F, s], F32, tag="vt")
                        wvk = wv[:, co:co + s, :]
                        for h in range(heads):
                            nc.vector.tensor_tensor(
                                vt[:, h * 32:(h + 1) * 32, :]
                                .rearrange("p c s -> p s c"),
                                g[:, :, h * 32:h * 32 + 32],
                                wvk[:, :, h:h + 1].broadcast_to([128, s, 32]),
                                op=mult)
                        nc.vector.tensor_reduce(
                            acc4[:, :, k:k + 1].rearrange("p c o -> p (c o)"),
                            vt[:], axis=AX, op=add)
                        co += s
                    acc = wk.tile([128, F], F32, tag="acc")
                    nc.vector.tensor_reduce(acc[:], acc4[:], axis=AX, op=add)
                    nc.vector.tensor_scalar_max(den[:], den[:], 1e-30)
                    nc.vector.reciprocal(den[:], den[:])
                    hb = wk.tile([128, F], F32, tag="hb")
                    for h in range(heads):
                        nc.vector.tensor_tensor(
                            hb[:, h * 32:(h + 1) * 32],
                            acc[:, h * 32:(h + 1) * 32],
                            den[:, h:h + 1].broadcast_to([128, 32]), op=mult)
                    nc.vector.tensor_tensor(
                        hb[:], hb[:],
                        bias_t[:, 0:F], op=add)
                    nc.vector.tensor_scalar_max(
                        hall[:, b * F:(b + 1) * F], hb[:], 0.0)
                    if post_block is not None:
                        post_block(b)
                    soff += ST

            # ---------------- Layer 1 ----------------
            h1all = bigp.tile([128, NBLK * 64], BF16)

            def l1_post(b):
                pt = ps.tile([64, 128], BF16, tag="mm", name="ptT")
                nc.tensor.transpose(pt[:], h1all[:, b * 64:(b + 1) * 64],
                                    idb_t[:])
                nc.scalar.copy(hTl_t[:, b * 128:(b + 1) * 128], pt[:])

            edge_layer(tab1, xr1_t, att1b_t, b1_t, 64, H, h1all, 128, BF16,
                       post_block=l1_post)
            nc.sync.dma_start(hT_loc[:], hTl_t[:])

            tc.strict_bb_all_engine_barrier()
            nc.gpsimd.collective_compute(
                "AllGather", byp,
                replica_groups=[list(range(NCORES))],
                ins=[hT_loc.opt()], outs=[hT_all.opt()])
            tc.strict_bb_all_engine_barrier()

            # ---------------- L2 table + local xr2 ----------------
            for c in range(NCORES):
                for n0 in range(0, SHP, 512):
                    w_ = min(512, SHP - n0)
                    hinb = wk.tile([64, 512], BF16, tag="hinb")
                    nc.sync.dma_start(
                        hinb[:, :w_], hT_all[c * 64:(c + 1) * 64, n0:n0 + w_])
                    pt = ps.tile([128, 256], F32, tag="mm")
                    for j in range(w_ // 128):
                        nc.tensor.matmul(pt[:, j * 64:(j + 1) * 64],
                                         hinb[:, j * 128:(j + 1) * 128],
                                         w2_t[:], start=True, stop=True)
                    st = wk.tile([128, 256], BF16, tag="t2sb")
                    nc.scalar.copy(st[:], pt[:])
                    base = c * SHP + n0
                    nc.sync.dma_start(
                        tab2[base:base + w_, 0:64]
                        .rearrange("(j p) c -> p j c", p=128),
                        st[:, :w_ // 2].rearrange("p (j c) -> p j c", c=64))

            for n0 in range(0, SHP, 512):
                w_ = min(512, SHP - n0)
                hinb = wk.tile([64, 512], BF16, tag="hinb")
                nc.vector.tensor_copy(hinb[:, :w_], hTl_t[:, n0:n0 + w_])
                pt = ps.tile([128, 256], F32, tag="mm")
                for j in range(w_ // 128):
                    nc.tensor.matmul(pt[:, j * 64:(j + 1) * 64],
                                     hinb[:, j * 128:(j + 1) * 128], w2_t[:],
                                     start=True, stop=True)
                st = wk.tile([128, 256], BF16, tag="t2sb")
                nc.scalar.copy(st[:], pt[:])
                for j in range(w_ // 128):
                    b = n0 // 128 + j
                    nc.vector.tensor_copy(xr2_t[:, b * 32:(b + 1) * 32],
                                          st[:, j * 64 + 32:j * 64 + 64])

            tc.strict_bb_all_engine_barrier()

            # ---------------- Layer 2 + fused pooling ----------------
            pp = psacc.tile([34, 64], F32)

            def l2_post(b):
                h2 = h2all[:, b * 32:(b + 1) * 32]
                pt = ps.tile([32, 128], F32, tag="mm", name="ptP")
                nc.tensor.transpose(pt[:], h2, id_t[:])
                h2T = wk.tile([32, 128], BF16, tag="h2T")
                nc.vector.tensor_copy(h2T[:], pt[:])
                gp1 = ps.tile([128, 32], F32, tag="mm", name="gp1")
                nc.tensor.matmul(gp1[:], h2T[:], g1w_t[:],
                                 start=True, stop=True)
                ga = wk.tile([128, 32], F32, tag="ga")
                nc.vector.tensor_tensor(
                    ga[:], gp1[:],
                    g1b_t[:], op=add)
                nc.vector.tensor_scalar_max(ga[:], ga[:], 0.0)
                nc.vector.tensor_tensor(
                    ga[:], ga[:],
                    g2w_t[:], op=mult)
                gt = wk.tile([128, 1], F32, tag="gt")
                nc.vector.tensor_reduce(gt[:], ga[:], axis=AX, op=add)
                nc.vector.tensor_tensor(
                    gt[:], gt[:],
                    sc4_t[:, 0:1], op=add)
                ge = wk.tile([128, 1], F32, tag="ge")
                nc.scalar.activation(ge[:], gt[:], EXP)
                vg = wk.tile([128, 34], F32, tag="vg")
                nc.vector.tensor_tensor(
                    vg[:, 0:32], h2, ge[:].broadcast_to([128, 32]), op=mult)
                nc.vector.tensor_copy(vg[:, 32:33], ge[:])
                nc.vector.memset(vg[:, 33:34], 0.0)
                ohg = wk.tile([128, 64], F32, tag="ohg")
                nc.vector.tensor_scalar(
                    ohg[:], io64_t[:],
                    bloc_t[:, b:b + 1], None, op0=iseq)
                vgb = wk.tile([128, 34], BF16, tag="vgb")
                nc.vector.tensor_copy(vgb[:], vg[:])
                ohgb = wk.tile([128, 64], BF16, tag="ohgb")
                nc.vector.tensor_copy(ohgb[:], ohg[:])
                nc.tensor.matmul(pp[:], vgb[:], ohgb[:],
                                 start=(b == 0), stop=(b == NBLK - 1))

            edge_layer(tab2, xr2_t, att2b_t, b2_t, 32, 1, h2all, 128, BF16,
                       post_block=l2_post)

            pin = wk.tile([48, 64], F32, tag="pin")
            nc.vector.memset(pin[:], 0.0)
            nc.scalar.copy(pin[0:34, :], pp[:])
            nc.sync.dma_start(pool_in[:], pin[:])

            tc.strict_bb_all_engine_barrier()
            nc.gpsimd.collective_compute(
                "AllReduce", add,
                replica_groups=[list(range(NCORES))],
                ins=[pool_in.opt()], outs=[pool_all.opt()])
            tc.strict_bb_all_engine_barrier()

            pall = wk.tile([48, 64], F32, tag="pall")
            nc.sync.dma_start(pall[:], pool_all[:])
            dn = wk.tile([1, 64], F32, tag="dn")
            nc.vector.reciprocal(dn[:], pall[32:33, :])
            dnr = ps.tile([32, 64], F32, tag="mm")
            nc.tensor.matmul(dnr[:], on132_t[:], dn[:],
                             start=True, stop=True)
            pooledT = wk.tile([32, 64], BF16, tag="pooledT")
            nc.vector.tensor_tensor(
                pooledT[:], pall[0:32, :], dnr[:], op=mult)
            zp = ps.tile([64, 32], F32, tag="mm")
            nc.tensor.matmul(zp[:], pooledT[:], l1w_t[:],
                             start=True, stop=True)
            z = wk.tile([64, 32], F32, tag="z")
            nc.vector.tensor_tensor(
                z[:], zp[:], l1b_t[:], op=add)
            nc.vector.tensor_scalar_max(z[:], z[:], 0.0)
            nc.vector.tensor_tensor(
                z[:], z[:], l2w_t[:], op=mult)
            yv = wk.tile([64, 1], F32, tag="yv")
            nc.vector.tensor_reduce(yv[:], z[:], axis=AX, op=add)
            nc.vector.tensor_tensor(
                yv[:], yv[:], sc4_t[0:64, 1:2], op=add)
            nc.sync.dma_start(out_y[:], yv[:])

    nc.compile()
    return nc


def kernel(**inputs):
    x = np.asarray(inputs["x"], dtype=np.float32)
    edge_index = np.asarray(inputs["edge_index"])
    batch = np.asarray(inputs["batch"])
    key = (int(edge_index[:, ::4099].sum()), int(batch[::997].sum()))
    if key not in _CACHE:
        prep = host_prep(edge_index, batch)
        nc = build_kernel(prep[0])
        _CACHE.clear()
        _CACHE[key] = (prep, nc)
    (S, Stot, trow, perms, idx_all, msk_all, blc_all, cnt_all), nc = _CACHE[key]

    xp = np.zeros((NTAB, IN), dtype=np.float32)
    xp[trow] = x
    xT_full = np.ascontiguousarray(xp.T).astype(ml_dtypes.bfloat16)

    w1c = np.concatenate([inputs["Wl1"], inputs["Wr1"]], 1).astype(ml_dtypes.bfloat16)
    w2c = np.concatenate([inputs["Wl2"], inputs["Wr2"]], 1).astype(ml_dtypes.bfloat16)
    common = {
        "xT": xT_full, "w1": w1c, "w2": w2c,
        "att1r": np.tile(np.asarray(inputs["att1"], np.float32).reshape(1, 64), (128, 1)),
        "att2r": np.tile(np.asarray(inputs["att2"], np.float32).reshape(1, 32), (128, 1)),
        "b1r": np.tile(np.asarray(inputs["b1"], np.float32).reshape(1, 64), (128, 1)),
        "b2r": np.tile(np.asarray(inputs["b2"], np.float32).reshape(1, 32), (128, 1)),
        "g1wp": np.asarray(inputs["g1w"]).astype(ml_dtypes.bfloat16),
        "g1br": np.tile(np.asarray(inputs["g1b"], np.float32).reshape(1, 32), (128, 1)),
        "g2wr": np.tile(np.asarray(inputs["g2w"], np.float32).reshape(1, 32), (128, 1)),
        "l1wp": np.asarray(inputs["lin1w"]).astype(ml_dtypes.bfloat16),
        "l1br": np.tile(np.asarray(inputs["lin1b"], np.float32).reshape(1, 32), (64, 1)),
        "l2wr": np.tile(np.asarray(inputs["lin2w"], np.float32).reshape(1, 32), (64, 1)),
        "sc4": np.tile(np.array([[float(np.ravel(inputs["g2b"])[0]),
                          float(np.ravel(inputs["lin2b"])[0]), 0.0, 0.0]],
                        np.float32), (128, 1)),
        "io64": np.tile(np.arange(64, dtype=np.float32).reshape(1, 64), (128, 1)),
        "ones132": np.ones((1, 32), np.float32),
        "id128": np.eye(128, dtype=np.float32),
    }
    in_maps = []
    for c in range(NCORES):
        m = dict(common)
        m["xTl"] = np.ascontiguousarray(xT_full[:, c * SHP:(c + 1) * SHP])
        m["bloc"] = blc_all[c]
        m["msk"] = msk_all[c]
        m["eidx"] = idx_all[c]
        m["cnts"] = cnt_all[c]
        in_maps.append(m)

    res = run_bass_kernel_spmd(nc, in_maps, list(range(NCORES)))
    return res.results[0]["y"].reshape(G).astype(np.float32)



# revision 48
# speedup vs baseline: 1.3499x; 1.1062x over previous
"""GATv2 regressor on 8 Trainium2 NeuronCores (Bass).

Sharding: core c owns dst nodes [c*12500, (c+1)*12500), relabeled locally in
decreasing in-degree order so fixed-slot padding per 128-dst block is tight.
All per-edge indexing is host-precomputed; the device does dense DMA,
dma_gather by src table row, broadcast adds, free-dim reductions and small
matmuls. Softmax skips max-subtraction (scores are clamped before exp);
denominators are reduced from the masked exp tile and divided at the end.

Both layers' node tables are bf16 with 256B rows so each per-edge gather
descriptor moves one full row. Per-core gather lists end in a -1 suffix with
the exact valid count supplied at runtime through a gpsimd register, so cores
with fewer edges in a (block, chunk) cell skip the cross-core padding rows.
Activations are batched per 128-dst block (one Lrelu span, one Exp span) to
avoid per-chunk activation-table reloads.
"""
import numpy as np
import ml_dtypes

import concourse.bacc as bacc
import concourse.mybir as mybir
import concourse.tile as tile
from concourse.bass_utils import run_bass_kernel_spmd
from concourse.library_config import mlp as mlp_lib

F32 = mybir.dt.float32
BF16 = mybir.dt.bfloat16
I16 = mybir.dt.int16

N, E, IN, C, H, G = 100000, 1600000, 128, 32, 2, 64
NEG = 0.2
NCORES = 8
SH = 12500
SHP = 12544              # 98*128
NBLK = SHP // 128        # 98
NTAB = SHP * NCORES      # 100352
NCHUNK = 4
WWID = 32768             # gather window width (int16 index reach)
WBASE = np.array([0, 22528, 45056, 67584])   # overlapping window bases
_WB = np.array([22528, 32768, 45056, 55296, 67584, 77824])  # region bounds

_CACHE = {}


def _wrap_idx(idx):
    n = idx.shape[0]
    w = idx.reshape(n // 16, 16).T
    return np.tile(w, (8, 1)).astype(np.int16)


def _assign_windows(sr, rk):
    """2-choice balance: edges in window overlaps go to the less-loaded
    window of their dst, flattening per-(dst, window) counts."""
    reg = np.searchsorted(_WB, sr, side="right")
    C7 = np.zeros((SH, 7), np.int64)
    np.add.at(C7, (rk, reg), 1)
    e = C7[:, [0, 2, 4, 6]].astype(np.float64)
    o = C7[:, [1, 3, 5]].astype(np.float64)
    a = o / 2
    for _ in range(8):
        l0 = e[:, 0] + a[:, 0]
        l1 = e[:, 1] + (o[:, 0] - a[:, 0]) + a[:, 1]
        l2 = e[:, 2] + (o[:, 1] - a[:, 1]) + a[:, 2]
        l3 = e[:, 3] + (o[:, 2] - a[:, 2])
        a[:, 0] = np.clip(a[:, 0] + (l1 - l0) / 2, 0, o[:, 0])
        a[:, 1] = np.clip(a[:, 1] + (l2 - l1) / 2, 0, o[:, 1])
        a[:, 2] = np.clip(a[:, 2] + (l3 - l2) / 2, 0, o[:, 2])
    A = np.minimum(np.rint(a).astype(np.int64), C7[:, [1, 3, 5]])
    key = rk * 7 + reg
    order = np.argsort(key, kind="stable")
    first = np.zeros(SH * 7 + 1, np.int64)
    np.cumsum(np.bincount(key, minlength=SH * 7), out=first[1:])
    rig = np.empty(sr.size, np.int64)
    rig[order] = np.arange(sr.size) - first[key[order]]
    w = np.empty(sr.size, np.int64)
    excl = (reg % 2 == 0)
    w[excl] = reg[excl] // 2
    ov = ~excl
    ovi = (reg[ov] - 1) // 2
    left = rig[ov] < A[rk[ov], ovi]
    w[ov] = np.where(left, ovi, ovi + 1)
    return w


def host_prep(edge_index, batch):
    src = edge_index[0].astype(np.int64)
    dst = edge_index[1].astype(np.int64)
    core = dst // SH
    dloc = dst % SH

    perms, ranks = [], []
    for c in range(NCORES):
        deg = np.bincount(dloc[core == c], minlength=SH)
        p = np.argsort(-deg, kind="stable")
        r = np.empty(SH, dtype=np.int64)
        r[p] = np.arange(SH)
        perms.append(p)
        ranks.append(r)

    ncore = np.arange(N) // SH
    nloc = np.arange(N) % SH
    trow = np.empty(N, dtype=np.int64)
    for c in range(NCORES):
        m = ncore == c
        trow[m] = c * SHP + ranks[c][nloc[m]]

    erow = np.empty(E, dtype=np.int64)
    for c in range(NCORES):
        m = core == c
        erow[m] = ranks[c][dloc[m]]
    srow = trow[src]
    wofe = np.empty(E, dtype=np.int64)
    for c in range(NCORES):
        m = core == c
        wofe[m] = _assign_windows(srow[m], erow[m])

    S = np.ones((NBLK, NCHUNK), dtype=np.int64)
    for c in range(NCORES):
        m = core == c
        cnt = np.bincount((erow[m] * NCHUNK + wofe[m]).astype(np.int64),
                          minlength=SH * NCHUNK).reshape(SH, NCHUNK)
        full = np.zeros((SHP, NCHUNK), dtype=np.int64)
        full[:SH] = cnt
        S = np.maximum(S, full.reshape(NBLK, 128, NCHUNK).max(axis=1))
    Stot = int(S.sum())

    idx_all, msk_all, blc_all, cnt_all = [], [], [], []
    for c in range(NCORES):
        m = np.nonzero(core == c)[0]
        key = erow[m] * NCHUNK + wofe[m]
        order = np.argsort(key, kind="stable")
        ms = m[order]
        rk, ck = erow[ms], wofe[ms]
        gid = (rk * NCHUNK + ck).astype(np.int64)
        first = np.zeros(SH * NCHUNK + 1, dtype=np.int64)
        np.cumsum(np.bincount(gid, minlength=SH * NCHUNK), out=first[1:])
        slot = np.arange(ms.size) - first[gid]

        iarr = np.zeros((Stot * 128,), dtype=np.int16)
        marr = np.zeros((128, Stot), dtype=np.float32)
        carr = np.zeros((NBLK * NCHUNK,), dtype=np.int32)
        off = 0
        bb = rk // 128
        pp = rk % 128
        for b in range(NBLK):
            selb = bb == b
            for k in range(NCHUNK):
                s = int(S[b, k])
                sel = selb & (ck == k)
                p = pp[sel]
                sl = slot[sel]
                # per-core used slot count for this (block, chunk)
                u = int(sl.max()) + 1 if sl.size else 0
                seg = np.zeros(s * 128, dtype=np.int16)
                seg[sl * 128 + p] = (srow[ms[sel]] - WBASE[k]).astype(np.int16)
                seg[u * 128:] = -1            # trailing slots: skipped by DGE
                carr[b * NCHUNK + k] = u * 128
                iarr[off * 128:(off + s) * 128] = seg
                marr[p, off + sl] = 1.0
                off += s
        idx_all.append(_wrap_idx(iarr))
        msk_all.append(marr)
        cnt_all.append(np.tile(carr.reshape(1, -1), (1, 1)))
        bl = np.full((128, NBLK), 127.0, dtype=np.float32)
        for b in range(NBLK):
            lo = b * 128
            take = min(128, SH - lo)
            bl[:take, b] = batch[c * SH + perms[c][lo:lo + take]]
        blc_all.append(bl)

    return S, Stot, trow, perms, idx_all, msk_all, blc_all, cnt_all


def build_kernel(S):
    Stot = int(S.sum())
    nc = bacc.Bacc("TRN2", target_bir_lowering=False, num_swdge_queues=4,
                   dynamic_dma_scratch_size=40960)

    def dp(name, shape, dt=F32):
        return nc.declare_dram_parameter(name, shape, dt, isOutput=False)

    xT = dp("xT", [IN, NTAB], BF16)
    xTl = dp("xTl", [IN, SHP], BF16)
    w1 = dp("w1", [IN, 128], BF16)           # [Wl1 | Wr1]
    w2 = dp("w2", [64, 64], BF16)            # [Wl2 | Wr2]
    cnts = dp("cnts", [1, NBLK * NCHUNK], mybir.dt.int32)
    att1r = dp("att1r", [128, 64])
    att2r = dp("att2r", [128, 32])
    b1r = dp("b1r", [128, 64])
    b2r = dp("b2r", [128, 32])
    g1wp = dp("g1wp", [32, 32], BF16)
    g1br = dp("g1br", [128, 32])
    g2wr = dp("g2wr", [128, 32])
    l1wp = dp("l1wp", [32, 32], BF16)
    l1br = dp("l1br", [64, 32])
    l2wr = dp("l2wr", [64, 32])
    sc4 = dp("sc4", [128, 4])                  # [g2b, lin2b, 0, 0]
    io64 = dp("io64", [128, 64])
    ones132 = dp("ones132", [1, 32])
    id128 = dp("id128", [128, 128])
    bloc = dp("bloc", [128, NBLK])
    msk = dp("msk", [128, Stot])
    eidx = dp("eidx", [128, (Stot * 128) // 16], I16)

    out_y = nc.declare_dram_parameter("y", [64, 1], F32, isOutput=True)

    add = mybir.AluOpType.add
    mult = mybir.AluOpType.mult
    iseq = mybir.AluOpType.is_equal
    byp = mybir.AluOpType.bypass
    AX = mybir.AxisListType.X
    EXP = mybir.ActivationFunctionType.Exp
    RELU = mybir.ActivationFunctionType.Relu
    LRELU = mybir.ActivationFunctionType.Lrelu

    with tile.TileContext(nc) as tc:
        with (
            tc.tile_pool(name="const", bufs=1) as cp,
            tc.tile_pool(name="gat", bufs=4) as gpool,
            tc.tile_pool(name="wk", bufs=2) as wk,
            tc.tile_pool(name="vtp", bufs=3) as vtp,
            tc.tile_pool(name="ps", bufs=2, space="PSUM") as ps,
            tc.tile_pool(name="psacc", bufs=1, space="PSUM") as psacc,
            tc.tile_pool(name="big", bufs=1) as bigp,
            tc.tile_pool(name="dram", bufs=1, space="DRAM") as dram,
        ):
            tab1 = dram.tile([NTAB, 128], BF16)
            tab2 = dram.tile([NTAB, 128], BF16)
            hT_loc = dram.tile([64, SHP], BF16)
            hT_all = dram.tile([NCORES * 64, SHP], BF16)
            pool_in = dram.tile([48, 64], F32)
            pool_all = dram.tile([48, 64], F32)
            nc.gpsimd.load_library(mlp_lib)

            def lc(t, shape, dt=F32):
                tt = cp.tile(shape, dt, tag=t.name + "_t")
                nc.sync.dma_start(tt[:], t[:])
                return tt

            w1_t = lc(w1, [IN, 128], BF16)
            w2_t = lc(w2, [64, 64], BF16)
            cnts_t = lc(cnts, [1, NBLK * NCHUNK], mybir.dt.int32)
            att1_t = lc(att1r, [128, 64])
            att1b_t = cp.tile([128, 64], BF16, tag="att1b")
            nc.vector.tensor_copy(att1b_t[:], att1_t[:])
            att2_t = lc(att2r, [128, 32])
            att2b_t = cp.tile([128, 32], BF16, tag="att2b")
            nc.vector.tensor_copy(att2b_t[:], att2_t[:])
            b1_t = lc(b1r, [128, 64])
            b2_t = lc(b2r, [128, 32])
            g1w_t = lc(g1wp, [32, 32], BF16)
            g1b_t = lc(g1br, [128, 32])
            g2w_t = lc(g2wr, [128, 32])
            l1w_t = lc(l1wp, [32, 32], BF16)
            l1b_t = lc(l1br, [64, 32])
            l2w_t = lc(l2wr, [64, 32])
            sc4_t = lc(sc4, [128, 4])
            io64_t = lc(io64, [128, 64])
            on132_t = lc(ones132, [1, 32])
            id_t = lc(id128, [128, 128])
            idb_t = cp.tile([128, 128], BF16, tag="idb")
            nc.vector.tensor_copy(idb_t[:], id_t[:])
            bloc_t = lc(bloc, [128, NBLK])
            msk_t = bigp.tile([128, Stot], F32)
            nc.sync.dma_start(msk_t[:], msk[:])

            xr1_t = bigp.tile([128, NBLK * 64], BF16)
            xr2_t = bigp.tile([128, NBLK * 32], BF16)
            hTl_t = bigp.tile([64, SHP], BF16)
            h2all = bigp.tile([128, NBLK * 32], F32)
            gcnt_regs = [nc.gpsimd.alloc_register(f"gcnt{i}")
                         for i in range(NCHUNK)]
            for _gz in range(8):
                gz = gpool.tile([128, 16, 128], BF16, tag="g", bufs=8,
                                name=f"gz{_gz}")
                nc.vector.memset(gz[:], 0.0)

            # ---------------- L1 global table + local xr1 ----------------
            for n0 in range(0, NTAB, 512):
                pt = ps.tile([128, 512], F32, tag="mm")
                xin = wk.tile([IN, 512], BF16, tag="xin")
                nc.sync.dma_start(xin[:], xT[:, n0:n0 + 512])
                for j in range(4):
                    nc.tensor.matmul(pt[:, j * 128:(j + 1) * 128],
                                     xin[:, j * 128:(j + 1) * 128], w1_t[:],
                                     start=True, stop=True)
                st = wk.tile([128, 512], BF16, tag="tsb")
                nc.scalar.copy(st[:], pt[:])
                nc.sync.dma_start(
                    tab1[n0:n0 + 512, :].rearrange("(j p) c -> p j c", p=128),
                    st[:].rearrange("p (j c) -> p j c", j=4))
            for n0 in range(0, SHP, 512):
                w_ = min(512, SHP - n0)
                pt = ps.tile([128, 512], F32, tag="mm")
                xin = wk.tile([IN, 512], BF16, tag="xin")
                nc.sync.dma_start(xin[:, :w_], xTl[:, n0:n0 + w_])
                for j in range(w_ // 128):
                    nc.tensor.matmul(pt[:, j * 128:(j + 1) * 128],
                                     xin[:, j * 128:(j + 1) * 128], w1_t[:],
                                     start=True, stop=True)
                st = wk.tile([128, 512], BF16, tag="tsb")
                nc.scalar.copy(st[:, :w_], pt[:, :w_])
                for j in range(w_ // 128):
                    b = n0 // 128 + j
                    nc.vector.tensor_copy(
                        xr1_t[:, b * 64:(b + 1) * 64],
                        st[:, j * 128 + 64:j * 128 + 128])

            tc.strict_bb_all_engine_barrier()

            def edge_layer(tab, xr_t, att_t, bias_t, F, heads, hall, gw, gdt,
                           post_block=None):
                ioff = 0
                soff = 0
                for b in range(NBLK):
                    Ss = [int(S[b, k]) for k in range(NCHUNK)]
                    ST = sum(Ss)
                    acc4 = wk.tile([128, F, NCHUNK], F32, tag="acc4")
                    ixb = wk.tile([128, ST * 8], I16, tag="ix")
                    nc.sync.dma_start(ixb[:], eidx[:, ioff:ioff + ST * 8])
                    scob = wk.tile([128, ST * heads], F32, tag="scob")
                    wexpb = wk.tile([128, ST * heads], F32, tag="wexpb")
                    nc.gpsimd.reg_load(
                        gcnt_regs,
                        cnts_t[0:1, b * NCHUNK:(b + 1) * NCHUNK])
                    gs = []
                    iof2 = 0
                    co = 0
                    for k in range(NCHUNK):
                        s = Ss[k]
                        ni = s * 128
                        g = gpool.tile([128, s, gw], gdt, tag="g", bufs=8)
                        nc.gpsimd.dma_gather(
                            g[:], tab[int(WBASE[k]):int(WBASE[k]) + WWID, 0:gw],
                            ixb[:, iof2:iof2 + ni // 16],
                            ni, gcnt_regs[k], gw,
                            single_packet=False,
                            queue_num=(b * NCHUNK + k) % 4)
                        gs.append(g)
                        iof2 += ni // 16
                        ioff += ni // 16
                        xb = xr_t[:, b * F:(b + 1) * F]
                        sadd = wk.tile([128, s, F], gdt, tag="sadd")
                        nc.vector.tensor_tensor(
                            sadd[:], g[:, :, 0:F],
                            xb.unsqueeze(1).broadcast_to([128, s, F]), op=add)
                        nc.scalar.activation(sadd[:], sadd[:], LRELU,
                                             alpha=NEG)
                        nc.vector.tensor_tensor(
                            sadd[:], sadd[:],
                            att_t[:, 0:F].unsqueeze(1)
                            .broadcast_to([128, s, F]), op=mult)
                        nc.vector.tensor_reduce(
                            scob[:, co * heads:(co + s) * heads],
                            sadd[:].rearrange("p s (h c) -> p (s h) c", c=32),
                            axis=AX, op=add)
                        co += s
                    nc.scalar.activation(wexpb[:], scob[:], EXP)
                    wv = wexpb[:].rearrange("p (s h) -> p s h", h=heads)
                    nc.vector.tensor_tensor(
                        wv, wv,
                        msk_t[:, soff:soff + ST].unsqueeze(2)
                        .broadcast_to([128, ST, heads]), op=mult)
                    den = wk.tile([128, heads], F32, tag="den")
                    nc.vector.tensor_reduce(
                        den[:], wexpb[:].rearrange("p (s h) -> p h s", h=heads),
                        axis=AX, op=add)
                    co = 0
                    for k in range(NCHUNK):
                        s = Ss[k]
                        g = gs[k]
                        vt = vtp.tile([128, F, s], F32, tag="vt")
                        wvk = wv[:, co:co + s, :]
                        for h in range(heads):
                            nc.vector.tensor_tensor(
                                vt[:, h * 32:(h + 1) * 32, :]
                                .rearrange("p c s -> p s c"),
                                g[:, :, h * 32:h * 32 + 32],
                                wvk[:, :, h:h + 1].broadcast_to([128, s, 32]),
                                op=mult)
                        nc.vector.tensor_reduce(
                            acc4[:, :, k:k + 1].rearrange("p c o -> p (c o)"),
                            vt[:], axis=AX, op=add)
                        co += s
                    acc = wk.tile([128, F], F32, tag="acc")
                    nc.vector.tensor_reduce(acc[:], acc4[:], axis=AX, op=add)
                    nc.vector.tensor_scalar_max(den[:], den[:], 1e-30)
                    nc.vector.reciprocal(den[:], den[:])
                    hb = wk.tile([128, F], F32, tag="hb")
                    for h in range(heads):
                        nc.vector.tensor_tensor(
                            hb[:, h * 32:(h + 1) * 32],
                            acc[:, h * 32:(h + 1) * 32],
                            den[:, h:h + 1].broadcast_to([128, 32]), op=mult)
                    nc.vector.tensor_tensor(
                        hb[:], hb[:],
                        bias_t[:, 0:F], op=add)
                    nc.vector.tensor_scalar_max(
                        hall[:, b * F:(b + 1) * F], hb[:], 0.0)
                    if post_block is not None:
                        post_block(b)
                    soff += ST

            # ---------------- Layer 1 ----------------
            h1all = bigp.tile([128, NBLK * 64], BF16)

            def l1_post(b):
                pt = ps.tile([64, 128], BF16, tag="mm", name="ptT")
                nc.tensor.transpose(pt[:], h1all[:, b * 64:(b + 1) * 64],
                                    idb_t[:])
                nc.scalar.copy(hTl_t[:, b * 128:(b + 1) * 128], pt[:])

            edge_layer(tab1, xr1_t, att1b_t, b1_t, 64, H, h1all, 128, BF16,
                       post_block=l1_post)
            nc.sync.dma_start(hT_loc[:], hTl_t[:])

            tc.strict_bb_all_engine_barrier()
            nc.gpsimd.collective_compute(
                "AllGather", byp,
                replica_groups=[list(range(NCORES))],
                ins=[hT_loc.opt()], outs=[hT_all.opt()])
            tc.strict_bb_all_engine_barrier()

            # ---------------- L2 table + local xr2 ----------------
            for c in range(NCORES):
                for n0 in range(0, SHP, 512):
                    w_ = min(512, SHP - n0)
                    hinb = wk.tile([64, 512], BF16, tag="hinb")
                    nc.sync.dma_start(
                        hinb[:, :w_], hT_all[c * 64:(c + 1) * 64, n0:n0 + w_])
                    pt = ps.tile([128, 256], F32, tag="mm")
                    for j in range(w_ // 128):
                        nc.tensor.matmul(pt[:, j * 64:(j + 1) * 64],
                                         hinb[:, j * 128:(j + 1) * 128],
                                         w2_t[:], start=True, stop=True)
                    st = wk.tile([128, 256], BF16, tag="t2sb")
                    nc.scalar.copy(st[:], pt[:])
                    base = c * SHP + n0
                    nc.sync.dma_start(
                        tab2[base:base + w_, 0:64]
                        .rearrange("(j p) c -> p j c", p=128),
                        st[:, :w_ // 2].rearrange("p (j c) -> p j c", c=64))

            for n0 in range(0, SHP, 512):
                w_ = min(512, SHP - n0)
                hinb = wk.tile([64, 512], BF16, tag="hinb")
                nc.vector.tensor_copy(hinb[:, :w_], hTl_t[:, n0:n0 + w_])
                pt = ps.tile([128, 256], F32, tag="mm")
                for j in range(w_ // 128):
                    nc.tensor.matmul(pt[:, j * 64:(j + 1) * 64],
                                     hinb[:, j * 128:(j + 1) * 128], w2_t[:],
                                     start=True, stop=True)
                st = wk.tile([128, 256], BF16, tag="t2sb")
                nc.scalar.copy(st[:], pt[:])
                for j in range(w_ // 128):
                    b = n0 // 128 + j
                    nc.vector.tensor_copy(xr2_t[:, b * 32:(b + 1) * 32],
                                          st[:, j * 64 + 32:j * 64 + 64])

            tc.strict_bb_all_engine_barrier()

            # ---------------- Layer 2 + fused pooling ----------------
            pp = psacc.tile([34, 64], F32)

            def l2_post(b):
                h2 = h2all[:, b * 32:(b + 1) * 32]
                pt = ps.tile([32, 128], F32, tag="mm", name="ptP")
                nc.tensor.transpose(pt[:], h2, id_t[:])
                h2T = wk.tile([32, 128], BF16, tag="h2T")
                nc.vector.tensor_copy(h2T[:], pt[:])
                gp1 = ps.tile([128, 32], F32, tag="mm", name="gp1")
                nc.tensor.matmul(gp1[:], h2T[:], g1w_t[:],
                                 start=True, stop=True)
                ga = wk.tile([128, 32], F32, tag="ga")
                nc.vector.tensor_tensor(
                    ga[:], gp1[:],
                    g1b_t[:], op=add)
                nc.vector.tensor_scalar_max(ga[:], ga[:], 0.0)
                nc.vector.tensor_tensor(
                    ga[:], ga[:],
                    g2w_t[:], op=mult)
                gt = wk.tile([128, 1], F32, tag="gt")
                nc.vector.tensor_reduce(gt[:], ga[:], axis=AX, op=add)
                nc.vector.tensor_tensor(
                    gt[:], gt[:],
                    sc4_t[:, 0:1], op=add)
                ge = wk.tile([128, 1], F32, tag="ge")
                nc.scalar.activation(ge[:], gt[:], EXP)
                vg = wk.tile([128, 34], F32, tag="vg")
                nc.vector.tensor_tensor(
                    vg[:, 0:32], h2, ge[:].broadcast_to([128, 32]), op=mult)
                nc.vector.tensor_copy(vg[:, 32:33], ge[:])
                nc.vector.memset(vg[:, 33:34], 0.0)
                ohg = wk.tile([128, 64], F32, tag="ohg")
                nc.vector.tensor_scalar(
                    ohg[:], io64_t[:],
                    bloc_t[:, b:b + 1], None, op0=iseq)
                vgb = wk.tile([128, 34], BF16, tag="vgb")
                nc.vector.tensor_copy(vgb[:], vg[:])
                ohgb = wk.tile([128, 64], BF16, tag="ohgb")
                nc.vector.tensor_copy(ohgb[:], ohg[:])
                nc.tensor.matmul(pp[:], vgb[:], ohgb[:],
                                 start=(b == 0), stop=(b == NBLK - 1))

            edge_layer(tab2, xr2_t, att2b_t, b2_t, 32, 1, h2all, 128, BF16,
                       post_block=l2_post)

            pin = wk.tile([48, 64], F32, tag="pin")
            nc.vector.memset(pin[:], 0.0)
            nc.scalar.copy(pin[0:34, :], pp[:])
            nc.sync.dma_start(pool_in[:], pin[:])

            tc.strict_bb_all_engine_barrier()
            nc.gpsimd.collective_compute(
                "AllReduce", add,
                replica_groups=[list(range(NCORES))],
                ins=[pool_in.opt()], outs=[pool_all.opt()])
            tc.strict_bb_all_engine_barrier()

            pall = wk.tile([48, 64], F32, tag="pall")
            nc.sync.dma_start(pall[:], pool_all[:])
            dn = wk.tile([1, 64], F32, tag="dn")
            nc.vector.reciprocal(dn[:], pall[32:33, :])
            dnr = ps.tile([32, 64], F32, tag="mm")
            nc.tensor.matmul(dnr[:], on132_t[:], dn[:],
                             start=True, stop=True)
            pooledT = wk.tile([32, 64], BF16, tag="pooledT")
            nc.vector.tensor_tensor(
                pooledT[:], pall[0:32, :], dnr[:], op=mult)
            zp = ps.tile([64, 32], F32, tag="mm")
            nc.tensor.matmul(zp[:], pooledT[:], l1w_t[:],
                             start=True, stop=True)
            z = wk.tile([64, 32], F32, tag="z")
            nc.vector.tensor_tensor(
                z[:], zp[:], l1b_t[:], op=add)
            nc.vector.tensor_scalar_max(z[:], z[:], 0.0)
            nc.vector.tensor_tensor(
                z[:], z[:], l2w_t[:], op=mult)
            yv = wk.tile([64, 1], F32, tag="yv")
            nc.vector.tensor_reduce(yv[:], z[:], axis=AX, op=add)
            nc.vector.tensor_tensor(
                yv[:], yv[:], sc4_t[0:64, 1:2], op=add)
            nc.sync.dma_start(out_y[:], yv[:])

    nc.compile()
    return nc


def kernel(**inputs):
    x = np.asarray(inputs["x"], dtype=np.float32)
    edge_index = np.asarray(inputs["edge_index"])
    batch = np.asarray(inputs["batch"])
    key = (int(edge_index[:, ::4099].sum()), int(batch[::997].sum()))
    if key not in _CACHE:
        prep = host_prep(edge_index, batch)
        nc = build_kernel(prep[0])
        _CACHE.clear()
        _CACHE[key] = (prep, nc)
    (S, Stot, trow, perms, idx_all, msk_all, blc_all, cnt_all), nc = _CACHE[key]

    xp = np.zeros((NTAB, IN), dtype=np.float32)
    xp[trow] = x
    xT_full = np.ascontiguousarray(xp.T).astype(ml_dtypes.bfloat16)

    w1c = np.concatenate([inputs["Wl1"], inputs["Wr1"]], 1).astype(ml_dtypes.bfloat16)
    w2c = np.concatenate([inputs["Wl2"], inputs["Wr2"]], 1).astype(ml_dtypes.bfloat16)
    common = {
        "xT": xT_full, "w1": w1c, "w2": w2c,
        "att1r": np.tile(np.asarray(inputs["att1"], np.float32).reshape(1, 64), (128, 1)),
        "att2r": np.tile(np.asarray(inputs["att2"], np.float32).reshape(1, 32), (128, 1)),
        "b1r": np.tile(np.asarray(inputs["b1"], np.float32).reshape(1, 64), (128, 1)),
        "b2r": np.tile(np.asarray(inputs["b2"], np.float32).reshape(1, 32), (128, 1)),
        "g1wp": np.asarray(inputs["g1w"]).astype(ml_dtypes.bfloat16),
        "g1br": np.tile(np.asarray(inputs["g1b"], np.float32).reshape(1, 32), (128, 1)),
        "g2wr": np.tile(np.asarray(inputs["g2w"], np.float32).reshape(1, 32), (128, 1)),
        "l1wp": np.asarray(inputs["lin1w"]).astype(ml_dtypes.bfloat16),
        "l1br": np.tile(np.asarray(inputs["lin1b"], np.float32).reshape(1, 32), (64, 1)),
        "l2wr": np.tile(np.asarray(inputs["lin2w"], np.float32).reshape(1, 32), (64, 1)),
        "sc4": np.tile(np.array([[float(np.ravel(inputs["g2b"])[0]),
                          float(np.ravel(inputs["lin2b"])[0]), 0.0, 0.0]],
                        np.float32), (128, 1)),
        "io64": np.tile(np.arange(64, dtype=np.float32).reshape(1, 64), (128, 1)),
        "ones132": np.ones((1, 32), np.float32),
        "id128": np.eye(128, dtype=np.float32),
    }
    in_maps = []
    for c in range(NCORES):
        m = dict(common)
        m["xTl"] = np.ascontiguousarray(xT_full[:, c * SHP:(c + 1) * SHP])
        m["bloc"] = blc_all[c]
        m["msk"] = msk_all[c]
        m["eidx"] = idx_all[c]
        m["cnts"] = cnt_all[c]
        in_maps.append(m)

    res = run_bass_kernel_spmd(nc, in_maps, list(range(NCORES)))
    return res.results[0]["y"].reshape(G).astype(np.float32)



# revision 50
# speedup vs baseline: 1.4853x; 1.1003x over previous
"""GATv2 regressor on 8 Trainium2 NeuronCores (Bass).

Sharding: core c owns dst nodes [c*12500, (c+1)*12500), relabeled locally in
decreasing in-degree order so fixed-slot padding per 128-dst block is tight.
All per-edge indexing is host-precomputed; the device does dense DMA,
dma_gather by src table row, broadcast adds, free-dim reductions and small
matmuls. Softmax skips max-subtraction (scores are O(1) by construction);
denominators are reduced from the masked exp tile and divided at the end.

Both layers' node tables are bf16 with 256B rows so each per-edge gather
descriptor moves one full row. Per-core gather lists end in a -1 suffix with
the exact valid count supplied at runtime through a gpsimd register, so cores
with fewer edges in a (block, chunk) cell skip the cross-core padding rows.
Activations are batched per 128-dst block (one Lrelu span, one Exp span) to
avoid per-chunk activation-table reloads.
"""
import numpy as np
import ml_dtypes

import concourse.bacc as bacc
import concourse.mybir as mybir
import concourse.tile as tile
from concourse.bass_utils import run_bass_kernel_spmd
from concourse.library_config import mlp as mlp_lib

F32 = mybir.dt.float32
BF16 = mybir.dt.bfloat16
I16 = mybir.dt.int16

N, E, IN, C, H, G = 100000, 1600000, 128, 32, 2, 64
NEG = 0.2
NCORES = 8
SH = 12500
SHP = 12544              # 98*128
NBLK = SHP // 128        # 98
NTAB = SHP * NCORES      # 100352
NCHUNK = 4
WWID = 32768             # gather window width (int16 index reach)
WBASE = np.array([0, 22528, 45056, 67584])   # overlapping window bases
_WB = np.array([22528, 32768, 45056, 55296, 67584, 77824])  # region bounds

_CACHE = {}


def _wrap_idx(idx):
    n = idx.shape[0]
    w = idx.reshape(n // 16, 16).T
    return np.tile(w, (8, 1)).astype(np.int16)


def _assign_windows(sr, rk):
    """2-choice balance: edges in window overlaps go to the less-loaded
    window of their dst, flattening per-(dst, window) counts."""
    reg = np.searchsorted(_WB, sr, side="right")
    C7 = np.zeros((SH, 7), np.int64)
    np.add.at(C7, (rk, reg), 1)
    e = C7[:, [0, 2, 4, 6]].astype(np.float64)
    o = C7[:, [1, 3, 5]].astype(np.float64)
    a = o / 2
    for _ in range(8):
        l0 = e[:, 0] + a[:, 0]
        l1 = e[:, 1] + (o[:, 0] - a[:, 0]) + a[:, 1]
        l2 = e[:, 2] + (o[:, 1] - a[:, 1]) + a[:, 2]
        l3 = e[:, 3] + (o[:, 2] - a[:, 2])
        a[:, 0] = np.clip(a[:, 0] + (l1 - l0) / 2, 0, o[:, 0])
        a[:, 1] = np.clip(a[:, 1] + (l2 - l1) / 2, 0, o[:, 1])
        a[:, 2] = np.clip(a[:, 2] + (l3 - l2) / 2, 0, o[:, 2])
    A = np.minimum(np.rint(a).astype(np.int64), C7[:, [1, 3, 5]])
    key = rk * 7 + reg
    order = np.argsort(key, kind="stable")
    first = np.zeros(SH * 7 + 1, np.int64)
    np.cumsum(np.bincount(key, minlength=SH * 7), out=first[1:])
    rig = np.empty(sr.size, np.int64)
    rig[order] = np.arange(sr.size) - first[key[order]]
    w = np.empty(sr.size, np.int64)
    excl = (reg % 2 == 0)
    w[excl] = reg[excl] // 2
    ov = ~excl
    ovi = (reg[ov] - 1) // 2
    left = rig[ov] < A[rk[ov], ovi]
    w[ov] = np.where(left, ovi, ovi + 1)
    return w


def host_prep(edge_index, batch):
    src = edge_index[0].astype(np.int64)
    dst = edge_index[1].astype(np.int64)
    core = dst // SH
    dloc = dst % SH

    perms, ranks = [], []
    for c in range(NCORES):
        deg = np.bincount(dloc[core == c], minlength=SH)
        p = np.argsort(-deg, kind="stable")
        r = np.empty(SH, dtype=np.int64)
        r[p] = np.arange(SH)
        perms.append(p)
        ranks.append(r)

    ncore = np.arange(N) // SH
    nloc = np.arange(N) % SH
    trow = np.empty(N, dtype=np.int64)
    for c in range(NCORES):
        m = ncore == c
        trow[m] = c * SHP + ranks[c][nloc[m]]

    erow = np.empty(E, dtype=np.int64)
    for c in range(NCORES):
        m = core == c
        erow[m] = ranks[c][dloc[m]]
    srow = trow[src]
    wofe = np.empty(E, dtype=np.int64)
    for c in range(NCORES):
        m = core == c
        wofe[m] = _assign_windows(srow[m], erow[m])

    S = np.ones((NBLK, NCHUNK), dtype=np.int64)
    for c in range(NCORES):
        m = core == c
        cnt = np.bincount((erow[m] * NCHUNK + wofe[m]).astype(np.int64),
                          minlength=SH * NCHUNK).reshape(SH, NCHUNK)
        full = np.zeros((SHP, NCHUNK), dtype=np.int64)
        full[:SH] = cnt
        S = np.maximum(S, full.reshape(NBLK, 128, NCHUNK).max(axis=1))
    Stot = int(S.sum())

    idx_all, msk_all, blc_all, cnt_all = [], [], [], []
    for c in range(NCORES):
        m = np.nonzero(core == c)[0]
        key = erow[m] * NCHUNK + wofe[m]
        order = np.argsort(key, kind="stable")
        ms = m[order]
        rk, ck = erow[ms], wofe[ms]
        gid = (rk * NCHUNK + ck).astype(np.int64)
        first = np.zeros(SH * NCHUNK + 1, dtype=np.int64)
        np.cumsum(np.bincount(gid, minlength=SH * NCHUNK), out=first[1:])
        slot = np.arange(ms.size) - first[gid]

        iarr = np.zeros((Stot * 128,), dtype=np.int16)
        marr = np.zeros((128, Stot), dtype=np.float32)
        carr = np.zeros((NBLK * NCHUNK,), dtype=np.int32)
        off = 0
        bb = rk // 128
        pp = rk % 128
        for b in range(NBLK):
            selb = bb == b
            for k in range(NCHUNK):
                s = int(S[b, k])
                sel = selb & (ck == k)
                p = pp[sel]
                sl = slot[sel]
                # per-core used slot count for this (block, chunk)
                u = int(sl.max()) + 1 if sl.size else 0
                seg = np.zeros(s * 128, dtype=np.int16)
                seg[sl * 128 + p] = (srow[ms[sel]] - WBASE[k]).astype(np.int16)
                seg[u * 128:] = -1            # trailing slots: skipped by DGE
                carr[b * NCHUNK + k] = u * 128
                iarr[off * 128:(off + s) * 128] = seg
                marr[p, off + sl] = 1.0
                off += s
        idx_all.append(_wrap_idx(iarr))
        msk_all.append(marr)
        cnt_all.append(np.tile(carr.reshape(1, -1), (1, 1)))
        bl = np.full((128, NBLK), 127.0, dtype=np.float32)
        for b in range(NBLK):
            lo = b * 128
            take = min(128, SH - lo)
            bl[:take, b] = batch[c * SH + perms[c][lo:lo + take]]
        blc_all.append(bl)

    return S, Stot, trow, perms, idx_all, msk_all, blc_all, cnt_all


def build_kernel(S):
    Stot = int(S.sum())
    nc = bacc.Bacc("TRN2", target_bir_lowering=False, num_swdge_queues=4,
                   dynamic_dma_scratch_size=49152)

    def dp(name, shape, dt=F32):
        return nc.declare_dram_parameter(name, shape, dt, isOutput=False)

    xT = dp("xT", [IN, NTAB], BF16)
    xTl = dp("xTl", [IN, SHP], BF16)
    w1 = dp("w1", [IN, 128], BF16)           # [Wl1 | Wr1]
    w2 = dp("w2", [64, 64], BF16)            # [Wl2 | Wr2]
    cnts = dp("cnts", [1, NBLK * NCHUNK], mybir.dt.int32)
    att1r = dp("att1r", [128, 64])
    att2r = dp("att2r", [128, 32])
    b1r = dp("b1r", [128, 64])
    b2r = dp("b2r", [128, 32])
    g1wp = dp("g1wp", [32, 32], BF16)
    g1br = dp("g1br", [128, 32])
    g2wr = dp("g2wr", [128, 32])
    l1wp = dp("l1wp", [32, 32], BF16)
    l1br = dp("l1br", [64, 32])
    l2wr = dp("l2wr", [64, 32])
    sc4 = dp("sc4", [128, 4])                  # [g2b, lin2b, 0, 0]
    io64 = dp("io64", [128, 64])
    ones132 = dp("ones132", [1, 32])
    id128 = dp("id128", [128, 128])
    bloc = dp("bloc", [128, NBLK])
    msk = dp("msk", [128, Stot])
    eidx = dp("eidx", [128, (Stot * 128) // 16], I16)

    out_y = nc.declare_dram_parameter("y", [64, 1], F32, isOutput=True)

    add = mybir.AluOpType.add
    mult = mybir.AluOpType.mult
    iseq = mybir.AluOpType.is_equal
    byp = mybir.AluOpType.bypass
    AX = mybir.AxisListType.X
    EXP = mybir.ActivationFunctionType.Exp
    RELU = mybir.ActivationFunctionType.Relu
    LRELU = mybir.ActivationFunctionType.Lrelu

    with tile.TileContext(nc) as tc:
        with (
            tc.tile_pool(name="const", bufs=1) as cp,
            tc.tile_pool(name="gat", bufs=4) as gpool,
            tc.tile_pool(name="wk", bufs=2) as wk,
            tc.tile_pool(name="vtp", bufs=4) as vtp,
            tc.tile_pool(name="ps", bufs=2, space="PSUM") as ps,
            tc.tile_pool(name="psacc", bufs=1, space="PSUM") as psacc,
            tc.tile_pool(name="big", bufs=1) as bigp,
            tc.tile_pool(name="dram", bufs=1, space="DRAM") as dram,
        ):
            tab1 = dram.tile([NTAB, 128], BF16)
            tab2 = dram.tile([NTAB, 128], BF16)
            hT_loc = dram.tile([64, SHP], BF16)
            hT_all = dram.tile([NCORES * 64, SHP], BF16)
            pool_in = dram.tile([48, 64], F32)
            pool_all = dram.tile([48, 64], F32)
            nc.gpsimd.load_library(mlp_lib)

            def lc(t, shape, dt=F32):
                tt = cp.tile(shape, dt, tag=t.name + "_t")
                nc.sync.dma_start(tt[:], t[:])
                return tt

            w1_t = lc(w1, [IN, 128], BF16)
            w2_t = lc(w2, [64, 64], BF16)
            cnts_t = lc(cnts, [1, NBLK * NCHUNK], mybir.dt.int32)
            att1_t = lc(att1r, [128, 64])
            att1b_t = cp.tile([128, 64], BF16, tag="att1b")
            nc.vector.tensor_copy(att1b_t[:], att1_t[:])
            att2_t = lc(att2r, [128, 32])
            att2b_t = cp.tile([128, 32], BF16, tag="att2b")
            nc.vector.tensor_copy(att2b_t[:], att2_t[:])
            b1_t = lc(b1r, [128, 64])
            b2_t = lc(b2r, [128, 32])
            g1w_t = lc(g1wp, [32, 32], BF16)
            g1b_t = lc(g1br, [128, 32])
            g2w_t = lc(g2wr, [128, 32])
            l1w_t = lc(l1wp, [32, 32], BF16)
            l1b_t = lc(l1br, [64, 32])
            l2w_t = lc(l2wr, [64, 32])
            sc4_t = lc(sc4, [128, 4])
            io64_t = lc(io64, [128, 64])
            on132_t = lc(ones132, [1, 32])
            id_t = lc(id128, [128, 128])
            idb_t = cp.tile([128, 128], BF16, tag="idb")
            nc.vector.tensor_copy(idb_t[:], id_t[:])
            bloc_t = lc(bloc, [128, NBLK])
            msk_t = bigp.tile([128, Stot], F32)
            nc.sync.dma_start(msk_t[:], msk[:])

            xr1_t = bigp.tile([128, NBLK * 64], BF16)
            xr2_t = bigp.tile([128, NBLK * 32], BF16)
            hTl_t = bigp.tile([64, SHP], BF16)
            h2all = bigp.tile([128, NBLK * 32], F32)
            gcnt_regs = [nc.gpsimd.alloc_register(f"gcnt{i}")
                         for i in range(NCHUNK)]
            for _gz in range(10):
                gz = gpool.tile([128, 16, 128], BF16, tag="g", bufs=10,
                                name=f"gz{_gz}")
                nc.vector.memset(gz[:], 0.0)

            # ---------------- L1 global table + local xr1 ----------------
            for n0 in range(0, NTAB, 512):
                pt = ps.tile([128, 512], F32, tag="mm")
                xin = wk.tile([IN, 512], BF16, tag="xin")
                nc.sync.dma_start(xin[:], xT[:, n0:n0 + 512])
                for j in range(4):
                    nc.tensor.matmul(pt[:, j * 128:(j + 1) * 128],
                                     xin[:, j * 128:(j + 1) * 128], w1_t[:],
                                     start=True, stop=True)
                st = wk.tile([128, 512], BF16, tag="tsb")
                nc.scalar.copy(st[:], pt[:])
                nc.sync.dma_start(
                    tab1[n0:n0 + 512, :].rearrange("(j p) c -> p j c", p=128),
                    st[:].rearrange("p (j c) -> p j c", j=4))
            for n0 in range(0, SHP, 512):
                w_ = min(512, SHP - n0)
                pt = ps.tile([128, 512], F32, tag="mm")
                xin = wk.tile([IN, 512], BF16, tag="xin")
                nc.sync.dma_start(xin[:, :w_], xTl[:, n0:n0 + w_])
                for j in range(w_ // 128):
                    nc.tensor.matmul(pt[:, j * 128:(j + 1) * 128],
                                     xin[:, j * 128:(j + 1) * 128], w1_t[:],
                                     start=True, stop=True)
                st = wk.tile([128, 512], BF16, tag="tsb")
                nc.scalar.copy(st[:, :w_], pt[:, :w_])
                for j in range(w_ // 128):
                    b = n0 // 128 + j
                    nc.vector.tensor_copy(
                        xr1_t[:, b * 64:(b + 1) * 64],
                        st[:, j * 128 + 64:j * 128 + 128])

            tc.strict_bb_all_engine_barrier()

            def edge_layer(tab, xr_t, att_t, bias_t, F, heads, hall, gw, gdt,
                           post_block=None):
                ioff = 0
                soff = 0
                for b in range(NBLK):
                    Ss = [int(S[b, k]) for k in range(NCHUNK)]
                    ST = sum(Ss)
                    acc4 = wk.tile([128, F, NCHUNK], F32, tag="acc4")
                    ixb = wk.tile([128, ST * 8], I16, tag="ix")
                    nc.sync.dma_start(ixb[:], eidx[:, ioff:ioff + ST * 8])
                    scob = wk.tile([128, ST * heads], F32, tag="scob")
                    wexpb = wk.tile([128, ST * heads], F32, tag="wexpb")
                    nc.gpsimd.reg_load(
                        gcnt_regs,
                        cnts_t[0:1, b * NCHUNK:(b + 1) * NCHUNK])
                    gs = []
                    iof2 = 0
                    co = 0
                    for k in range(NCHUNK):
                        s = Ss[k]
                        ni = s * 128
                        g = gpool.tile([128, s, gw], gdt, tag="g", bufs=10)
                        nc.gpsimd.dma_gather(
                            g[:], tab[int(WBASE[k]):int(WBASE[k]) + WWID, 0:gw],
                            ixb[:, iof2:iof2 + ni // 16],
                            ni, gcnt_regs[k], gw,
                            single_packet=False,
                            queue_num=(b * NCHUNK + k) % 4)
                        gs.append(g)
                        iof2 += ni // 16
                        ioff += ni // 16
                        xb = xr_t[:, b * F:(b + 1) * F]
                        sadd = wk.tile([128, s, F], gdt, tag="sadd")
                        nc.vector.tensor_tensor(
                            sadd[:], g[:, :, 0:F],
                            xb.unsqueeze(1).broadcast_to([128, s, F]), op=add)
                        nc.scalar.activation(sadd[:], sadd[:], LRELU,
                                             alpha=NEG)
                        nc.vector.tensor_tensor(
                            sadd[:], sadd[:],
                            att_t[:, 0:F].unsqueeze(1)
                            .broadcast_to([128, s, F]), op=mult)
                        nc.vector.tensor_reduce(
                            scob[:, co * heads:(co + s) * heads],
                            sadd[:].rearrange("p s (h c) -> p (s h) c", c=32),
                            axis=AX, op=add)
                        co += s
                    nc.scalar.activation(wexpb[:], scob[:], EXP)
                    wv = wexpb[:].rearrange("p (s h) -> p s h", h=heads)
                    nc.vector.tensor_tensor(
                        wv, wv,
                        msk_t[:, soff:soff + ST].unsqueeze(2)
                        .broadcast_to([128, ST, heads]), op=mult)
                    den = wk.tile([128, heads], F32, tag="den")
                    nc.vector.tensor_reduce(
                        den[:], wexpb[:].rearrange("p (s h) -> p h s", h=heads),
                        axis=AX, op=add)
                    co = 0
                    for k in range(NCHUNK):
                        s = Ss[k]
                        g = gs[k]
                        vt = vtp.tile([128, F, s], F32, tag="vt")
                        wvk = wv[:, co:co + s, :]
                        for h in range(heads):
                            nc.vector.tensor_tensor(
                                vt[:, h * 32:(h + 1) * 32, :]
                                .rearrange("p c s -> p s c"),
                                g[:, :, h * 32:h * 32 + 32],
                                wvk[:, :, h:h + 1].broadcast_to([128, s, 32]),
                                op=mult)
                        nc.vector.tensor_reduce(
                            acc4[:, :, k:k + 1].rearrange("p c o -> p (c o)"),
                            vt[:], axis=AX, op=add)
                        co += s
                    acc = wk.tile([128, F], F32, tag="acc")
                    nc.vector.tensor_reduce(acc[:], acc4[:], axis=AX, op=add)
                    nc.vector.tensor_scalar_max(den[:], den[:], 1e-30)
                    nc.vector.reciprocal(den[:], den[:])
                    hb = wk.tile([128, F], F32, tag="hb")
                    for h in range(heads):
                        nc.vector.tensor_tensor(
                            hb[:, h * 32:(h + 1) * 32],
                            acc[:, h * 32:(h + 1) * 32],
                            den[:, h:h + 1].broadcast_to([128, 32]), op=mult)
                    nc.vector.tensor_tensor(
                        hb[:], hb[:],
                        bias_t[:, 0:F], op=add)
                    nc.vector.tensor_scalar_max(
                        hall[:, b * F:(b + 1) * F], hb[:], 0.0)
                    if post_block is not None:
                        post_block(b)
                    soff += ST

            # ---------------- Layer 1 ----------------
            h1all = bigp.tile([128, NBLK * 64], BF16)

            def l1_post(b):
                pt = ps.tile([64, 128], BF16, tag="mm", name="ptT")
                nc.tensor.transpose(pt[:], h1all[:, b * 64:(b + 1) * 64],
                                    idb_t[:])
                nc.scalar.copy(hTl_t[:, b * 128:(b + 1) * 128], pt[:])

            edge_layer(tab1, xr1_t, att1b_t, b1_t, 64, H, h1all, 128, BF16,
                       post_block=l1_post)
            nc.sync.dma_start(hT_loc[:], hTl_t[:])

            tc.strict_bb_all_engine_barrier()
            nc.gpsimd.collective_compute(
                "AllGather", byp,
                replica_groups=[list(range(NCORES))],
                ins=[hT_loc.opt()], outs=[hT_all.opt()])
            tc.strict_bb_all_engine_barrier()

            # ---------------- L2 table + local xr2 ----------------
            for c in range(NCORES):
                for n0 in range(0, SHP, 512):
                    w_ = min(512, SHP - n0)
                    hinb = wk.tile([64, 512], BF16, tag="hinb")
                    nc.sync.dma_start(
                        hinb[:, :w_], hT_all[c * 64:(c + 1) * 64, n0:n0 + w_])
                    pt = ps.tile([128, 256], F32, tag="mm")
                    for j in range(w_ // 128):
                        nc.tensor.matmul(pt[:, j * 64:(j + 1) * 64],
                                         hinb[:, j * 128:(j + 1) * 128],
                                         w2_t[:], start=True, stop=True)
                    st = wk.tile([128, 256], BF16, tag="t2sb")
                    nc.scalar.copy(st[:], pt[:])
                    base = c * SHP + n0
                    nc.sync.dma_start(
                        tab2[base:base + w_, 0:64]
                        .rearrange("(j p) c -> p j c", p=128),
                        st[:, :w_ // 2].rearrange("p (j c) -> p j c", c=64))

            for n0 in range(0, SHP, 512):
                w_ = min(512, SHP - n0)
                hinb = wk.tile([64, 512], BF16, tag="hinb")
                nc.vector.tensor_copy(hinb[:, :w_], hTl_t[:, n0:n0 + w_])
                pt = ps.tile([128, 256], F32, tag="mm")
                for j in range(w_ // 128):
                    nc.tensor.matmul(pt[:, j * 64:(j + 1) * 64],
                                     hinb[:, j * 128:(j + 1) * 128], w2_t[:],
                                     start=True, stop=True)
                st = wk.tile([128, 256], BF16, tag="t2sb")
                nc.scalar.copy(st[:], pt[:])
                for j in range(w_ // 128):
                    b = n0 // 128 + j
                    nc.vector.tensor_copy(xr2_t[:, b * 32:(b + 1) * 32],
                                          st[:, j * 64 + 32:j * 64 + 64])

            tc.strict_bb_all_engine_barrier()

            # ---------------- Layer 2 + fused pooling ----------------
            pp = psacc.tile([34, 64], F32)

            def l2_post(b):
                h2 = h2all[:, b * 32:(b + 1) * 32]
                pt = ps.tile([32, 128], F32, tag="mm", name="ptP")
                nc.tensor.transpose(pt[:], h2, id_t[:])
                h2T = wk.tile([32, 128], BF16, tag="h2T")
                nc.vector.tensor_copy(h2T[:], pt[:])
                gp1 = ps.tile([128, 32], F32, tag="mm", name="gp1")
                nc.tensor.matmul(gp1[:], h2T[:], g1w_t[:],
                                 start=True, stop=True)
                ga = wk.tile([128, 32], F32, tag="ga")
                nc.vector.tensor_tensor(
                    ga[:], gp1[:],
                    g1b_t[:], op=add)
                nc.vector.tensor_scalar_max(ga[:], ga[:], 0.0)
                nc.vector.tensor_tensor(
                    ga[:], ga[:],
                    g2w_t[:], op=mult)
                gt = wk.tile([128, 1], F32, tag="gt")
                nc.vector.tensor_reduce(gt[:], ga[:], axis=AX, op=add)
                nc.vector.tensor_tensor(
                    gt[:], gt[:],
                    sc4_t[:, 0:1], op=add)
                ge = wk.tile([128, 1], F32, tag="ge")
                nc.scalar.activation(ge[:], gt[:], EXP)
                vg = wk.tile([128, 34], F32, tag="vg")
                nc.vector.tensor_tensor(
                    vg[:, 0:32], h2, ge[:].broadcast_to([128, 32]), op=mult)
                nc.vector.tensor_copy(vg[:, 32:33], ge[:])
                nc.vector.memset(vg[:, 33:34], 0.0)
                ohg = wk.tile([128, 64], F32, tag="ohg")
                nc.vector.tensor_scalar(
                    ohg[:], io64_t[:],
                    bloc_t[:, b:b + 1], None, op0=iseq)
                vgb = wk.tile([128, 34], BF16, tag="vgb")
                nc.vector.tensor_copy(vgb[:], vg[:])
                ohgb = wk.tile([128, 64], BF16, tag="ohgb")
                nc.vector.tensor_copy(ohgb[:], ohg[:])
                nc.tensor.matmul(pp[:], vgb[:], ohgb[:],
                                 start=(b == 0), stop=(b == NBLK - 1))

            edge_layer(tab2, xr2_t, att2b_t, b2_t, 32, 1, h2all, 128, BF16,
                       post_block=l2_post)

            pin = wk.tile([48, 64], F32, tag="pin")
            nc.vector.memset(pin[:], 0.0)
            nc.scalar.copy(pin[0:34, :], pp[:])
            nc.sync.dma_start(pool_in[:], pin[:])

            tc.strict_bb_all_engine_barrier()
            nc.gpsimd.collective_compute(
                "AllReduce", add,
                replica_groups=[list(range(NCORES))],
                ins=[pool_in.opt()], outs=[pool_all.opt()])
            tc.strict_bb_all_engine_barrier()

            pall = wk.tile([48, 64], F32, tag="pall")
            nc.sync.dma_start(pall[:], pool_all[:])
            dn = wk.tile([1, 64], F32, tag="dn")
            nc.vector.reciprocal(dn[:], pall[32:33, :])
            dnr = ps.tile([32, 64], F32, tag="mm")
            nc.tensor.matmul(dnr[:], on132_t[:], dn[:],
                             start=True, stop=True)
            pooledT = wk.tile([32, 64], BF16, tag="pooledT")
            nc.vector.tensor_tensor(
                pooledT[:], pall[0:32, :], dnr[:], op=mult)
            zp = ps.tile([64, 32], F32, tag="mm")
            nc.tensor.matmul(zp[:], pooledT[:], l1w_t[:],
                             start=True, stop=True)
            z = wk.tile([64, 32], F32, tag="z")
            nc.vector.tensor_tensor(
                z[:], zp[:], l1b_t[:], op=add)
            nc.vector.tensor_scalar_max(z[:], z[:], 0.0)
            nc.vector.tensor_tensor(
                z[:], z[:], l2w_t[:], op=mult)
            yv = wk.tile([64, 1], F32, tag="yv")
            nc.vector.tensor_reduce(yv[:], z[:], axis=AX, op=add)
            nc.vector.tensor_tensor(
                yv[:], yv[:], sc4_t[0:64, 1:2], op=add)
            nc.sync.dma_start(out_y[:], yv[:])

    nc.compile()
    return nc


def kernel(**inputs):
    x = np.asarray(inputs["x"], dtype=np.float32)
    edge_index = np.asarray(inputs["edge_index"])
    batch = np.asarray(inputs["batch"])
    key = (int(edge_index[:, ::4099].sum()), int(batch[::997].sum()))
    if key not in _CACHE:
        prep = host_prep(edge_index, batch)
        nc = build_kernel(prep[0])
        _CACHE.clear()
        _CACHE[key] = (prep, nc)
    (S, Stot, trow, perms, idx_all, msk_all, blc_all, cnt_all), nc = _CACHE[key]

    xp = np.zeros((NTAB, IN), dtype=np.float32)
    xp[trow] = x
    xT_full = np.ascontiguousarray(xp.T).astype(ml_dtypes.bfloat16)

    w1c = np.concatenate([inputs["Wl1"], inputs["Wr1"]], 1).astype(ml_dtypes.bfloat16)
    w2c = np.concatenate([inputs["Wl2"], inputs["Wr2"]], 1).astype(ml_dtypes.bfloat16)
    common = {
        "xT": xT_full, "w1": w1c, "w2": w2c,
        "att1r": np.tile(np.asarray(inputs["att1"], np.float32).reshape(1, 64), (128, 1)),
        "att2r": np.tile(np.asarray(inputs["att2"], np.float32).reshape(1, 32), (128, 1)),
        "b1r": np.tile(np.asarray(inputs["b1"], np.float32).reshape(1, 64), (128, 1)),
        "b2r": np.tile(np.asarray(inputs["b2"], np.float32).reshape(1, 32), (128, 1)),
        "g1wp": np.asarray(inputs["g1w"]).astype(ml_dtypes.bfloat16),
        "g1br": np.tile(np.asarray(inputs["g1b"], np.float32).reshape(1, 32), (128, 1)),
        "g2wr": np.tile(np.asarray(inputs["g2w"], np.float32).reshape(1, 32), (128, 1)),
        "l1wp": np.asarray(inputs["lin1w"]).astype(ml_dtypes.bfloat16),
        "l1br": np.tile(np.asarray(inputs["lin1b"], np.float32).reshape(1, 32), (64, 1)),
        "l2wr": np.tile(np.asarray(inputs["lin2w"], np.float32).reshape(1, 32), (64, 1)),
        "sc4": np.tile(np.array([[float(np.ravel(inputs["g2b"])[0]),
                          float(np.ravel(inputs["lin2b"])[0]), 0.0, 0.0]],
                        np.float32), (128, 1)),
        "io64": np.tile(np.arange(64, dtype=np.float32).reshape(1, 64), (128, 1)),
        "ones132": np.ones((1, 32), np.float32),
        "id128": np.eye(128, dtype=np.float32),
    }
    in_maps = []
    for c in range(NCORES):
        m = dict(common)
        m["xTl"] = np.ascontiguousarray(xT_full[:, c * SHP:(c + 1) * SHP])
        m["bloc"] = blc_all[c]
        m["msk"] = msk_all[c]
        m["eidx"] = idx_all[c]
        m["cnts"] = cnt_all[c]
        in_maps.append(m)

    res = run_bass_kernel_spmd(nc, in_maps, list(range(NCORES)))
    return res.results[0]["y"].reshape(G).astype(np.float32)



# revision 52
# speedup vs baseline: 1.6334x; 1.0997x over previous
"""GATv2 regressor on 8 Trainium2 NeuronCores (Bass).

Sharding: core c owns dst nodes [c*12500, (c+1)*12500), relabeled locally in
decreasing in-degree order so fixed-slot padding per 128-dst block is tight.
All per-edge indexing is host-precomputed; the device does dense DMA,
dma_gather by src table row, broadcast adds, free-dim reductions and small
matmuls. Softmax skips max-subtraction (scores are O(1) by construction);
denominators are reduced from the masked exp tile and divided at the end.

Both layers' node tables are bf16 with 256B rows so each per-edge gather
descriptor moves one full row. Per-core gather lists end in a -1 suffix with
the exact valid count supplied at runtime through a gpsimd register, so cores
with fewer edges in a (block, chunk) cell skip the cross-core padding rows.
Activations are batched per 128-dst block (one Lrelu span, one Exp span) to
avoid per-chunk activation-table reloads.
"""
import numpy as np
import ml_dtypes

import concourse.bacc as bacc
import concourse.mybir as mybir
import concourse.tile as tile
from concourse.bass_utils import run_bass_kernel_spmd
from concourse.library_config import mlp as mlp_lib

F32 = mybir.dt.float32
BF16 = mybir.dt.bfloat16
I16 = mybir.dt.int16

N, E, IN, C, H, G = 100000, 1600000, 128, 32, 2, 64
NEG = 0.2
NCORES = 8
SH = 12500
SHP = 12544              # 98*128
NBLK = SHP // 128        # 98
NTAB = SHP * NCORES      # 100352
NCHUNK = 4
WWID = 32768             # gather window width (int16 index reach)
WBASE = np.array([0, 22528, 45056, 67584])   # overlapping window bases
_WB = np.array([22528, 32768, 45056, 55296, 67584, 77824])  # region bounds

_CACHE = {}


def _wrap_idx(idx):
    n = idx.shape[0]
    w = idx.reshape(n // 16, 16).T
    return np.tile(w, (8, 1)).astype(np.int16)


def _assign_windows(sr, rk):
    """2-choice balance: edges in window overlaps go to the less-loaded
    window of their dst, flattening per-(dst, window) counts."""
    reg = np.searchsorted(_WB, sr, side="right")
    C7 = np.zeros((SH, 7), np.int64)
    np.add.at(C7, (rk, reg), 1)
    e = C7[:, [0, 2, 4, 6]].astype(np.float64)
    o = C7[:, [1, 3, 5]].astype(np.float64)
    a = o / 2
    for _ in range(8):
        l0 = e[:, 0] + a[:, 0]
        l1 = e[:, 1] + (o[:, 0] - a[:, 0]) + a[:, 1]
        l2 = e[:, 2] + (o[:, 1] - a[:, 1]) + a[:, 2]
        l3 = e[:, 3] + (o[:, 2] - a[:, 2])
        a[:, 0] = np.clip(a[:, 0] + (l1 - l0) / 2, 0, o[:, 0])
        a[:, 1] = np.clip(a[:, 1] + (l2 - l1) / 2, 0, o[:, 1])
        a[:, 2] = np.clip(a[:, 2] + (l3 - l2) / 2, 0, o[:, 2])
    A = np.minimum(np.rint(a).astype(np.int64), C7[:, [1, 3, 5]])
    key = rk * 7 + reg
    order = np.argsort(key, kind="stable")
    first = np.zeros(SH * 7 + 1, np.int64)
    np.cumsum(np.bincount(key, minlength=SH * 7), out=first[1:])
    rig = np.empty(sr.size, np.int64)
    rig[order] = np.arange(sr.size) - first[key[order]]
    w = np.empty(sr.size, np.int64)
    excl = (reg % 2 == 0)
    w[excl] = reg[excl] // 2
    ov = ~excl
    ovi = (reg[ov] - 1) // 2
    left = rig[ov] < A[rk[ov], ovi]
    w[ov] = np.where(left, ovi, ovi + 1)
    return w


def host_prep(edge_index, batch):
    src = edge_index[0].astype(np.int64)
    dst = edge_index[1].astype(np.int64)
    core = dst // SH
    dloc = dst % SH

    perms, ranks = [], []
    for c in range(NCORES):
        deg = np.bincount(dloc[core == c], minlength=SH)
        p = np.argsort(-deg, kind="stable")
        r = np.empty(SH, dtype=np.int64)
        r[p] = np.arange(SH)
        perms.append(p)
        ranks.append(r)

    ncore = np.arange(N) // SH
    nloc = np.arange(N) % SH
    trow = np.empty(N, dtype=np.int64)
    for c in range(NCORES):
        m = ncore == c
        trow[m] = c * SHP + ranks[c][nloc[m]]

    erow = np.empty(E, dtype=np.int64)
    for c in range(NCORES):
        m = core == c
        erow[m] = ranks[c][dloc[m]]
    srow = trow[src]
    wofe = np.empty(E, dtype=np.int64)
    for c in range(NCORES):
        m = core == c
        wofe[m] = _assign_windows(srow[m], erow[m])

    S = np.ones((NBLK, NCHUNK), dtype=np.int64)
    for c in range(NCORES):
        m = core == c
        cnt = np.bincount((erow[m] * NCHUNK + wofe[m]).astype(np.int64),
                          minlength=SH * NCHUNK).reshape(SH, NCHUNK)
        full = np.zeros((SHP, NCHUNK), dtype=np.int64)
        full[:SH] = cnt
        S = np.maximum(S, full.reshape(NBLK, 128, NCHUNK).max(axis=1))
    Stot = int(S.sum())

    idx_all, msk_all, blc_all, cnt_all = [], [], [], []
    for c in range(NCORES):
        m = np.nonzero(core == c)[0]
        key = erow[m] * NCHUNK + wofe[m]
        order = np.argsort(key, kind="stable")
        ms = m[order]
        rk, ck = erow[ms], wofe[ms]
        gid = (rk * NCHUNK + ck).astype(np.int64)
        first = np.zeros(SH * NCHUNK + 1, dtype=np.int64)
        np.cumsum(np.bincount(gid, minlength=SH * NCHUNK), out=first[1:])
        slot = np.arange(ms.size) - first[gid]

        iarr = np.zeros((Stot * 128,), dtype=np.int16)
        marr = np.zeros((128, Stot), dtype=np.float32)
        carr = np.zeros((NBLK * NCHUNK,), dtype=np.int32)
        off = 0
        bb = rk // 128
        pp = rk % 128
        for b in range(NBLK):
            selb = bb == b
            for k in range(NCHUNK):
                s = int(S[b, k])
                sel = selb & (ck == k)
                p = pp[sel]
                sl = slot[sel]
                # per-core used slot count for this (block, chunk)
                u = int(sl.max()) + 1 if sl.size else 0
                seg = np.zeros(s * 128, dtype=np.int16)
                seg[sl * 128 + p] = (srow[ms[sel]] - WBASE[k]).astype(np.int16)
                seg[u * 128:] = -1            # trailing slots: skipped by DGE
                carr[b * NCHUNK + k] = u * 128
                iarr[off * 128:(off + s) * 128] = seg
                marr[p, off + sl] = 1.0
                off += s
        idx_all.append(_wrap_idx(iarr))
        msk_all.append(marr)
        cnt_all.append(np.tile(carr.reshape(1, -1), (1, 1)))
        bl = np.full((128, NBLK), 127.0, dtype=np.float32)
        for b in range(NBLK):
            lo = b * 128
            take = min(128, SH - lo)
            bl[:take, b] = batch[c * SH + perms[c][lo:lo + take]]
        blc_all.append(bl)

    return S, Stot, trow, perms, idx_all, msk_all, blc_all, cnt_all


def build_kernel(S):
    Stot = int(S.sum())
    nc = bacc.Bacc("TRN2", target_bir_lowering=False, num_swdge_queues=4,
                   dynamic_dma_scratch_size=57344)

    def dp(name, shape, dt=F32):
        return nc.declare_dram_parameter(name, shape, dt, isOutput=False)

    xT = dp("xT", [IN, NTAB], BF16)
    xTl = dp("xTl", [IN, SHP], BF16)
    w1 = dp("w1", [IN, 128], BF16)           # [Wl1 | Wr1]
    w2 = dp("w2", [64, 64], BF16)            # [Wl2 | Wr2]
    cnts = dp("cnts", [1, NBLK * NCHUNK], mybir.dt.int32)
    att1r = dp("att1r", [128, 64])
    att2r = dp("att2r", [128, 32])
    b1r = dp("b1r", [128, 64])
    b2r = dp("b2r", [128, 32])
    g1wp = dp("g1wp", [32, 32], BF16)
    g1br = dp("g1br", [128, 32])
    g2wr = dp("g2wr", [128, 32])
    l1wp = dp("l1wp", [32, 32], BF16)
    l1br = dp("l1br", [64, 32])
    l2wr = dp("l2wr", [64, 32])
    sc4 = dp("sc4", [128, 4])                  # [g2b, lin2b, 0, 0]
    io64 = dp("io64", [128, 64])
    ones132 = dp("ones132", [1, 32])
    id128 = dp("id128", [128, 128])
    bloc = dp("bloc", [128, NBLK])
    msk = dp("msk", [128, Stot], BF16)
    eidx = dp("eidx", [128, (Stot * 128) // 16], I16)

    out_y = nc.declare_dram_parameter("y", [64, 1], F32, isOutput=True)

    add = mybir.AluOpType.add
    mult = mybir.AluOpType.mult
    iseq = mybir.AluOpType.is_equal
    byp = mybir.AluOpType.bypass
    AX = mybir.AxisListType.X
    EXP = mybir.ActivationFunctionType.Exp
    RELU = mybir.ActivationFunctionType.Relu
    LRELU = mybir.ActivationFunctionType.Lrelu

    with tile.TileContext(nc) as tc:
        with (
            tc.tile_pool(name="const", bufs=1) as cp,
            tc.tile_pool(name="gat", bufs=4) as gpool,
            tc.tile_pool(name="wk", bufs=2) as wk,
            tc.tile_pool(name="vtp", bufs=5) as vtp,
            tc.tile_pool(name="ps", bufs=2, space="PSUM") as ps,
            tc.tile_pool(name="psacc", bufs=1, space="PSUM") as psacc,
            tc.tile_pool(name="big", bufs=1) as bigp,
            tc.tile_pool(name="dram", bufs=1, space="DRAM") as dram,
        ):
            tab1 = dram.tile([NTAB, 128], BF16)
            tab2 = dram.tile([NTAB, 128], BF16)
            hT_loc = dram.tile([64, SHP], BF16)
            hT_all = dram.tile([NCORES * 64, SHP], BF16)
            pool_in = dram.tile([48, 64], F32)
            pool_all = dram.tile([48, 64], F32)
            nc.gpsimd.load_library(mlp_lib)

            def lc(t, shape, dt=F32):
                tt = cp.tile(shape, dt, tag=t.name + "_t")
                nc.sync.dma_start(tt[:], t[:])
                return tt

            w1_t = lc(w1, [IN, 128], BF16)
            w2_t = lc(w2, [64, 64], BF16)
            cnts_t = lc(cnts, [1, NBLK * NCHUNK], mybir.dt.int32)
            att1_t = lc(att1r, [128, 64])
            att1b_t = cp.tile([128, 64], BF16, tag="att1b")
            nc.vector.tensor_copy(att1b_t[:], att1_t[:])
            att2_t = lc(att2r, [128, 32])
            att2b_t = cp.tile([128, 32], BF16, tag="att2b")
            nc.vector.tensor_copy(att2b_t[:], att2_t[:])
            b1_t = lc(b1r, [128, 64])
            b2_t = lc(b2r, [128, 32])
            g1w_t = lc(g1wp, [32, 32], BF16)
            g1b_t = lc(g1br, [128, 32])
            g2w_t = lc(g2wr, [128, 32])
            l1w_t = lc(l1wp, [32, 32], BF16)
            l1b_t = lc(l1br, [64, 32])
            l2w_t = lc(l2wr, [64, 32])
            sc4_t = lc(sc4, [128, 4])
            io64_t = lc(io64, [128, 64])
            on132_t = lc(ones132, [1, 32])
            id_t = lc(id128, [128, 128])
            idb_t = cp.tile([128, 128], BF16, tag="idb")
            nc.vector.tensor_copy(idb_t[:], id_t[:])
            bloc_t = lc(bloc, [128, NBLK])
            msk_t = bigp.tile([128, Stot], BF16)
            nc.sync.dma_start(msk_t[:], msk[:])

            xr1_t = bigp.tile([128, NBLK * 64], BF16)
            xr2_t = bigp.tile([128, NBLK * 32], BF16)
            hTl_t = bigp.tile([64, SHP], BF16)
            h2all = bigp.tile([128, NBLK * 32], F32)
            gcnt_regs = [nc.gpsimd.alloc_register(f"gcnt{i}")
                         for i in range(NCHUNK)]
            for _gz in range(12):
                gz = gpool.tile([128, 16, 128], BF16, tag="g", bufs=12,
                                name=f"gz{_gz}")
                nc.vector.memset(gz[:], 0.0)

            # ---------------- L1 global table + local xr1 ----------------
            for n0 in range(0, NTAB, 512):
                pt = ps.tile([128, 512], F32, tag="mm")
                xin = wk.tile([IN, 512], BF16, tag="xin")
                nc.sync.dma_start(xin[:], xT[:, n0:n0 + 512])
                for j in range(4):
                    nc.tensor.matmul(pt[:, j * 128:(j + 1) * 128],
                                     xin[:, j * 128:(j + 1) * 128], w1_t[:],
                                     start=True, stop=True)
                st = wk.tile([128, 512], BF16, tag="tsb")
                nc.scalar.copy(st[:], pt[:])
                nc.sync.dma_start(
                    tab1[n0:n0 + 512, :].rearrange("(j p) c -> p j c", p=128),
                    st[:].rearrange("p (j c) -> p j c", j=4))
            for n0 in range(0, SHP, 512):
                w_ = min(512, SHP - n0)
                pt = ps.tile([128, 512], F32, tag="mm")
                xin = wk.tile([IN, 512], BF16, tag="xin")
                nc.sync.dma_start(xin[:, :w_], xTl[:, n0:n0 + w_])
                for j in range(w_ // 128):
                    nc.tensor.matmul(pt[:, j * 128:(j + 1) * 128],
                                     xin[:, j * 128:(j + 1) * 128], w1_t[:],
                                     start=True, stop=True)
                st = wk.tile([128, 512], BF16, tag="tsb")
                nc.scalar.copy(st[:, :w_], pt[:, :w_])
                for j in range(w_ // 128):
                    b = n0 // 128 + j
                    nc.vector.tensor_copy(
                        xr1_t[:, b * 64:(b + 1) * 64],
                        st[:, j * 128 + 64:j * 128 + 128])

            tc.strict_bb_all_engine_barrier()

            def edge_layer(tab, xr_t, att_t, bias_t, F, heads, hall, gw, gdt,
                           post_block=None):
                ioff = 0
                soff = 0
                for b in range(NBLK):
                    Ss = [int(S[b, k]) for k in range(NCHUNK)]
                    ST = sum(Ss)
                    acc4 = wk.tile([128, F, NCHUNK], F32, tag="acc4")
                    ixb = wk.tile([128, ST * 8], I16, tag="ix", bufs=3)
                    nc.sync.dma_start(ixb[:], eidx[:, ioff:ioff + ST * 8])
                    scob = wk.tile([128, ST * heads], F32, tag="scob")
                    wexpb = wk.tile([128, ST * heads], F32, tag="wexpb")
                    nc.gpsimd.reg_load(
                        gcnt_regs,
                        cnts_t[0:1, b * NCHUNK:(b + 1) * NCHUNK])
                    gs = []
                    iof2 = 0
                    co = 0
                    for k in range(NCHUNK):
                        s = Ss[k]
                        ni = s * 128
                        g = gpool.tile([128, s, gw], gdt, tag="g", bufs=12)
                        nc.gpsimd.dma_gather(
                            g[:], tab[int(WBASE[k]):int(WBASE[k]) + WWID, 0:gw],
                            ixb[:, iof2:iof2 + ni // 16],
                            ni, gcnt_regs[k], gw,
                            single_packet=False,
                            queue_num=(b * NCHUNK + k) % 4)
                        gs.append(g)
                        iof2 += ni // 16
                        ioff += ni // 16
                        xb = xr_t[:, b * F:(b + 1) * F]
                        sadd = wk.tile([128, s, F], gdt, tag="sadd")
                        nc.vector.tensor_tensor(
                            sadd[:], g[:, :, 0:F],
                            xb.unsqueeze(1).broadcast_to([128, s, F]), op=add)
                        nc.scalar.activation(sadd[:], sadd[:], LRELU,
                                             alpha=NEG)
                        nc.vector.tensor_tensor(
                            sadd[:], sadd[:],
                            att_t[:, 0:F].unsqueeze(1)
                            .broadcast_to([128, s, F]), op=mult)
                        nc.vector.tensor_reduce(
                            scob[:, co * heads:(co + s) * heads],
                            sadd[:].rearrange("p s (h c) -> p (s h) c", c=32),
                            axis=AX, op=add)
                        co += s
                    nc.scalar.activation(wexpb[:], scob[:], EXP)
                    wv = wexpb[:].rearrange("p (s h) -> p s h", h=heads)
                    nc.vector.tensor_tensor(
                        wv, wv,
                        msk_t[:, soff:soff + ST].unsqueeze(2)
                        .broadcast_to([128, ST, heads]), op=mult)
                    den = wk.tile([128, heads], F32, tag="den")
                    nc.vector.tensor_reduce(
                        den[:], wexpb[:].rearrange("p (s h) -> p h s", h=heads),
                        axis=AX, op=add)
                    co = 0
                    for k in range(NCHUNK):
                        s = Ss[k]
                        g = gs[k]
                        vt = vtp.tile([128, F, s], F32, tag="vt")
                        wvk = wv[:, co:co + s, :]
                        for h in range(heads):
                            nc.vector.tensor_tensor(
                                vt[:, h * 32:(h + 1) * 32, :]
                                .rearrange("p c s -> p s c"),
                                g[:, :, h * 32:h * 32 + 32],
                                wvk[:, :, h:h + 1].broadcast_to([128, s, 32]),
                                op=mult)
                        nc.vector.tensor_reduce(
                            acc4[:, :, k:k + 1].rearrange("p c o -> p (c o)"),
                            vt[:], axis=AX, op=add)
                        co += s
                    acc = wk.tile([128, F], F32, tag="acc")
                    nc.vector.tensor_reduce(acc[:], acc4[:], axis=AX, op=add)
                    nc.vector.tensor_scalar_max(den[:], den[:], 1e-30)
                    nc.vector.reciprocal(den[:], den[:])
                    hb = wk.tile([128, F], F32, tag="hb")
                    for h in range(heads):
                        nc.vector.tensor_tensor(
                            hb[:, h * 32:(h + 1) * 32],
                            acc[:, h * 32:(h + 1) * 32],
                            den[:, h:h + 1].broadcast_to([128, 32]), op=mult)
                    nc.vector.tensor_tensor(
                        hb[:], hb[:],
                        bias_t[:, 0:F], op=add)
                    nc.vector.tensor_scalar_max(
                        hall[:, b * F:(b + 1) * F], hb[:], 0.0)
                    if post_block is not None:
                        post_block(b)
                    soff += ST

            # ---------------- Layer 1 ----------------
            h1all = bigp.tile([128, NBLK * 64], BF16)

            def l1_post(b):
                pt = ps.tile([64, 128], BF16, tag="mm", name="ptT")
                nc.tensor.transpose(pt[:], h1all[:, b * 64:(b + 1) * 64],
                                    idb_t[:])
                nc.scalar.copy(hTl_t[:, b * 128:(b + 1) * 128], pt[:])

            edge_layer(tab1, xr1_t, att1b_t, b1_t, 64, H, h1all, 128, BF16,
                       post_block=l1_post)
            nc.sync.dma_start(hT_loc[:], hTl_t[:])

            tc.strict_bb_all_engine_barrier()
            nc.gpsimd.collective_compute(
                "AllGather", byp,
                replica_groups=[list(range(NCORES))],
                ins=[hT_loc.opt()], outs=[hT_all.opt()])
            tc.strict_bb_all_engine_barrier()

            # ---------------- L2 table + local xr2 ----------------
            for c in range(NCORES):
                for n0 in range(0, SHP, 512):
                    w_ = min(512, SHP - n0)
                    hinb = wk.tile([64, 512], BF16, tag="hinb")
                    nc.sync.dma_start(
                        hinb[:, :w_], hT_all[c * 64:(c + 1) * 64, n0:n0 + w_])
                    pt = ps.tile([128, 256], F32, tag="mm")
                    for j in range(w_ // 128):
                        nc.tensor.matmul(pt[:, j * 64:(j + 1) * 64],
                                         hinb[:, j * 128:(j + 1) * 128],
                                         w2_t[:], start=True, stop=True)
                    st = wk.tile([128, 256], BF16, tag="t2sb")
                    nc.scalar.copy(st[:], pt[:])
                    base = c * SHP + n0
                    nc.sync.dma_start(
                        tab2[base:base + w_, 0:64]
                        .rearrange("(j p) c -> p j c", p=128),
                        st[:, :w_ // 2].rearrange("p (j c) -> p j c", c=64))

            for n0 in range(0, SHP, 512):
                w_ = min(512, SHP - n0)
                hinb = wk.tile([64, 512], BF16, tag="hinb")
                nc.vector.tensor_copy(hinb[:, :w_], hTl_t[:, n0:n0 + w_])
                pt = ps.tile([128, 256], F32, tag="mm")
                for j in range(w_ // 128):
                    nc.tensor.matmul(pt[:, j * 64:(j + 1) * 64],
                                     hinb[:, j * 128:(j + 1) * 128], w2_t[:],
                                     start=True, stop=True)
                st = wk.tile([128, 256], BF16, tag="t2sb")
                nc.scalar.copy(st[:], pt[:])
                for j in range(w_ // 128):
                    b = n0 // 128 + j
                    nc.vector.tensor_copy(xr2_t[:, b * 32:(b + 1) * 32],
                                          st[:, j * 64 + 32:j * 64 + 64])

            tc.strict_bb_all_engine_barrier()

            # ---------------- Layer 2 + fused pooling ----------------
            pp = psacc.tile([34, 64], F32)

            def l2_post(b):
                h2 = h2all[:, b * 32:(b + 1) * 32]
                pt = ps.tile([32, 128], F32, tag="mm", name="ptP")
                nc.tensor.transpose(pt[:], h2, id_t[:])
                h2T = wk.tile([32, 128], BF16, tag="h2T")
                nc.vector.tensor_copy(h2T[:], pt[:])
                gp1 = ps.tile([128, 32], F32, tag="mm", name="gp1")
                nc.tensor.matmul(gp1[:], h2T[:], g1w_t[:],
                                 start=True, stop=True)
                ga = wk.tile([128, 32], F32, tag="ga")
                nc.vector.tensor_tensor(
                    ga[:], gp1[:],
                    g1b_t[:], op=add)
                nc.vector.tensor_scalar_max(ga[:], ga[:], 0.0)
                nc.vector.tensor_tensor(
                    ga[:], ga[:],
                    g2w_t[:], op=mult)
                gt = wk.tile([128, 1], F32, tag="gt")
                nc.vector.tensor_reduce(gt[:], ga[:], axis=AX, op=add)
                nc.vector.tensor_tensor(
                    gt[:], gt[:],
                    sc4_t[:, 0:1], op=add)
                ge = wk.tile([128, 1], F32, tag="ge")
                nc.scalar.activation(ge[:], gt[:], EXP)
                vg = wk.tile([128, 34], F32, tag="vg")
                nc.vector.tensor_tensor(
                    vg[:, 0:32], h2, ge[:].broadcast_to([128, 32]), op=mult)
                nc.vector.tensor_copy(vg[:, 32:33], ge[:])
                nc.vector.memset(vg[:, 33:34], 0.0)
                ohg = wk.tile([128, 64], F32, tag="ohg")
                nc.vector.tensor_scalar(
                    ohg[:], io64_t[:],
                    bloc_t[:, b:b + 1], None, op0=iseq)
                vgb = wk.tile([128, 34], BF16, tag="vgb")
                nc.vector.tensor_copy(vgb[:], vg[:])
                ohgb = wk.tile([128, 64], BF16, tag="ohgb")
                nc.vector.tensor_copy(ohgb[:], ohg[:])
                nc.tensor.matmul(pp[:], vgb[:], ohgb[:],
                                 start=(b == 0), stop=(b == NBLK - 1))

            edge_layer(tab2, xr2_t, att2b_t, b2_t, 32, 1, h2all, 128, BF16,
                       post_block=l2_post)

            pin = wk.tile([48, 64], F32, tag="pin")
            nc.vector.memset(pin[:], 0.0)
            nc.scalar.copy(pin[0:34, :], pp[:])
            nc.sync.dma_start(pool_in[:], pin[:])

            tc.strict_bb_all_engine_barrier()
            nc.gpsimd.collective_compute(
                "AllReduce", add,
                replica_groups=[list(range(NCORES))],
                ins=[pool_in.opt()], outs=[pool_all.opt()])
            tc.strict_bb_all_engine_barrier()

            pall = wk.tile([48, 64], F32, tag="pall")
            nc.sync.dma_start(pall[:], pool_all[:])
            dn = wk.tile([1, 64], F32, tag="dn")
            nc.vector.reciprocal(dn[:], pall[32:33, :])
            dnr = ps.tile([32, 64], F32, tag="mm")
            nc.tensor.matmul(dnr[:], on132_t[:], dn[:],
                             start=True, stop=True)
            pooledT = wk.tile([32, 64], BF16, tag="pooledT")
            nc.vector.tensor_tensor(
                pooledT[:], pall[0:32, :], dnr[:], op=mult)
            zp = ps.tile([64, 32], F32, tag="mm")
            nc.tensor.matmul(zp[:], pooledT[:], l1w_t[:],
                             start=True, stop=True)
            z = wk.tile([64, 32], F32, tag="z")
            nc.vector.tensor_tensor(
                z[:], zp[:], l1b_t[:], op=add)
            nc.vector.tensor_scalar_max(z[:], z[:], 0.0)
            nc.vector.tensor_tensor(
                z[:], z[:], l2w_t[:], op=mult)
            yv = wk.tile([64, 1], F32, tag="yv")
            nc.vector.tensor_reduce(yv[:], z[:], axis=AX, op=add)
            nc.vector.tensor_tensor(
                yv[:], yv[:], sc4_t[0:64, 1:2], op=add)
            nc.sync.dma_start(out_y[:], yv[:])

    nc.compile()
    return nc


def kernel(**inputs):
    x = np.asarray(inputs["x"], dtype=np.float32)
    edge_index = np.asarray(inputs["edge_index"])
    batch = np.asarray(inputs["batch"])
    key = (int(edge_index[:, ::4099].sum()), int(batch[::997].sum()))
    if key not in _CACHE:
        prep = host_prep(edge_index, batch)
        nc = build_kernel(prep[0])
        _CACHE.clear()
        _CACHE[key] = (prep, nc)
    (S, Stot, trow, perms, idx_all, msk_all, blc_all, cnt_all), nc = _CACHE[key]

    xp = np.zeros((NTAB, IN), dtype=np.float32)
    xp[trow] = x
    xT_full = np.ascontiguousarray(xp.T).astype(ml_dtypes.bfloat16)

    w1c = np.concatenate([inputs["Wl1"], inputs["Wr1"]], 1).astype(ml_dtypes.bfloat16)
    w2c = np.concatenate([inputs["Wl2"], inputs["Wr2"]], 1).astype(ml_dtypes.bfloat16)
    common = {
        "xT": xT_full, "w1": w1c, "w2": w2c,
        "att1r": np.tile(np.asarray(inputs["att1"], np.float32).reshape(1, 64), (128, 1)),
        "att2r": np.tile(np.asarray(inputs["att2"], np.float32).reshape(1, 32), (128, 1)),
        "b1r": np.tile(np.asarray(inputs["b1"], np.float32).reshape(1, 64), (128, 1)),
        "b2r": np.tile(np.asarray(inputs["b2"], np.float32).reshape(1, 32), (128, 1)),
        "g1wp": np.asarray(inputs["g1w"]).astype(ml_dtypes.bfloat16),
        "g1br": np.tile(np.asarray(inputs["g1b"], np.float32).reshape(1, 32), (128, 1)),
        "g2wr": np.tile(np.asarray(inputs["g2w"], np.float32).reshape(1, 32), (128, 1)),
        "l1wp": np.asarray(inputs["lin1w"]).astype(ml_dtypes.bfloat16),
        "l1br": np.tile(np.asarray(inputs["lin1b"], np.float32).reshape(1, 32), (64, 1)),
        "l2wr": np.tile(np.asarray(inputs["lin2w"], np.float32).reshape(1, 32), (64, 1)),
        "sc4": np.tile(np.array([[float(np.ravel(inputs["g2b"])[0]),
                          float(np.ravel(inputs["lin2b"])[0]), 0.0, 0.0]],
                        np.float32), (128, 1)),
        "io64": np.tile(np.arange(64, dtype=np.float32).reshape(1, 64), (128, 1)),
        "ones132": np.ones((1, 32), np.float32),
        "id128": np.eye(128, dtype=np.float32),
    }
    in_maps = []
    for c in range(NCORES):
        m = dict(common)
        m["xTl"] = np.ascontiguousarray(xT_full[:, c * SHP:(c + 1) * SHP])
        m["bloc"] = blc_all[c]
        m["msk"] = msk_all[c].astype(ml_dtypes.bfloat16)
        m["eidx"] = idx_all[c]
        m["cnts"] = cnt_all[c]
        in_maps.append(m)

    res = run_bass_kernel_spmd(nc, in_maps, list(range(NCORES)))
    return res.results[0]["y"].reshape(G).astype(np.float32)

